# revision 35
# baseline (speedup 1.0000x reference)
"""BinaryNet MLP forward on 8 TRN2 NeuronCores.

Strategy: data-parallel over batch (2048 rows/core), feature-major on-chip
layout (activations stored [channel, batch]).  For layers 1-3 the positive
per-row weight scales and the BatchNorm variance cancel inside sign(), so
each layer reduces to:  g_l = 1{ A_l >= mean_batch(A_l) }  where
A_l = sign(W_l) @ h_{l-1} is an exact small integer computed with fp8 {+-1}
activations x fp8 {+-1} weights on the TensorEngine.  h_l is produced by
the Activation engine as Sign(A - mean) (integer margins >= 1/16384 make
the fp32 subtract sign-safe).  Layers 2-4 run fp8 DoubleRow (two
K-subtiles per pass).  Layer 1 (continuous x) uses a 2-term fp16 split of
x that reproduces the reference's fp32 sign decisions; the hi and lo
tails of the 7th (7/8-padding) k-tile share one mixed k-tile, so layer 1
runs 13 K-passes instead of 14 while the six full-width sign-weight tiles
stay shared between the hi and lo passes.  Layer 4 applies the real
BatchNorm with weight scales.

Batch means: colsum(A_l) = sigma_l @ rowsum(h_{l-1}) distributes over the
AllReduce, so each core computes a LOCAL transposed matvec
phi = rowsum_local^T @ sigma  (rowsum stationary: LDWEIGHTS ~free; sigma
planes stream as the moving operand, DoubleRow pairs for layers 2/3) and
the AllReduce carries the phi rows instead of the rowsums -- similar
payload, but the LDWEIGHTS-bound PE matvec of the old formulation
disappears and the matvec no longer sits between the AllReduce and the
drains.  Theta is read back from the AR buffer with transposing DMAs
("r (t p) -> p t") and the phi rows are combined after, in
partition-aligned DVE ops.  For layers 2/3 the local rowsums are exact
integers |r| <= 2048, shipped to the PE as 3 balanced base-16 fp8
digits, so the threshold psums accumulate exact small integers; layer
1's theta uses an 8-digit base-16 fixed-point split of the local x
rowsums (2^-17 resolution, hierarchically extracted with exact fp32
steps) for the same reason -- the sign margins bottom out at ~1e-6 and
ANY accumulation-ordering dice in theta flips h1 entries, which the
binary net amplifies ~36x per layer (10 flips => 6% final error).

Engine-queue discipline at the AllReduce boundaries: the ACT/DVE queues
are strict in-order FIFOs, so theta-dependent ops must not be enqueued
ahead of ready work.  Each layer emits: matmuls for the first BRIDGE_T
j-tiles with psum->SBUF bridge copies (fp16 for the integer DR layers,
f32 for layer 1; even tiles copy on ACT, odd on DVE), THEN the theta
combine/scale algebra, THEN the deferred drains -- so the bridges free
psum banks while the collective is still in flight and the PE never
head-of-line blocks (worth ~30us over the naive order).  PSUM runs with
7 main accumulation banks + 1 for the theta matvecs/layer 4.

All sign-weights ship pre-signed from the host as fp8 {+-1} planes
(pre-paired for DoubleRow), so the ACT engine never runs sign-prep and
the DMA queue never serializes weight staging against the AllReduce path.

Measured (8-core axon TRN2, slope-of-n_rep method): ~477-490us vs the
538.8us/519.5us baseline; rel err 3.09e-07, zero flipped rows.
"""
import sys, os
sys.path.insert(0, '/opt/trn_rl_repo')
import numpy as np
import ml_dtypes

import concourse.bass as bass
import concourse.bacc as bacc
import concourse.tile as tile
import concourse.mybir as mybir
from concourse import bass_utils

F32 = mybir.dt.float32
BF16 = mybir.dt.bfloat16
FP16 = mybir.dt.float16
FP8 = mybir.dt.float8e4
AF = mybir.ActivationFunctionType
ALU = mybir.AluOpType
AX = mybir.AxisListType
DR = mybir.MatmulPerfMode.DoubleRow

NP8 = mybir.dt.np(mybir.dt.float8e4)

N_CORES = 8
D_IN, H, C = 784, 2048, 10
KF = 6                 # full 128-row k-tiles of x (768 rows)
KT1 = 2 * KF + 1       # 13 layer-1 K-passes (6 hi + 6 lo + 1 mixed)
NW1 = KF + 1           # 7 distinct layer-1 weight planes
KT = H // 128          # 16 k-tiles for layers 2-4
NP = KT // 2           # 8 DoubleRow k-pairs
JT = H // 128          # 16 output-channel tiles
CHUNK = 512
NCHNK = H // CHUNK     # 4 j-chunks of 512 for the theta matvecs
BRIDGE_BUFS = 12       # SBUF bridge slots for psum->sbuf theta-decoupling
BRIDGE_T = 3           # j-tiles per layer whose psums get bridged
MAGIC = float(3 << 22)  # fp32 round-to-nearest-int magic constant

# Timing-experiment knobs (leave defaults for correct results)
SKIP_DMA_REP = False   # skip input DMAs for rep>0 (garbage data, timing only)
SKIP_AR = False        # replace AllReduces with local DMA copies (wrong
                       # results on 8 cores, timing only)
DMA_ONLY = False       # emit only the input DMAs (timing the DMA floor)
NO_THETA = False       # constant thresholds: no matvec/AR/theta path at all
N_EXTRA = 0            # extra dummy DR layers after L3 (timing calibration)
MM_ONLY_EXTRA = False  # dummy layers emit only the matmuls (no drains)
N_EXTRA_L1 = 0         # extra MM-only replays of the L1 loop (timing)
LDW_OPT = False        # pass --enable-ldw-opt=true to walrus (dedup LDWs)


def _install_ldw_opt_patch():
    from concourse import bass_utils as _bu
    if getattr(_bu, "_ldw_patch", None):
        return
    _orig = _bu.run_command

    def _patched(argv, **kw):
        if LDW_OPT and any("walrus" in str(a) for a in argv[:1]):
            argv = ["--enable-ldw-opt=true" if a == "--enable-ldw-opt=false"
                    else a for a in argv]
        return _orig(argv, **kw)

    _bu.run_command = _patched
    _bu._ldw_patch = True


_install_ldw_opt_patch()


def build(n_loc: int, single: bool = False, n_rep: int = 1):
    """Emit the SPMD program for one core (all 8 run it on their own shard).

    single=True builds a 1-core variant with AllReduces replaced by plain
    copies (for cost-model timeline analysis).  n_rep repeats the whole
    forward pass back-to-back (device-time benchmarking)."""
    nch = n_loc // CHUNK
    assert n_loc % CHUNK == 0
    inv_B = 1.0 / float(n_loc * N_CORES)   # exact: power of two
    inv_H = 1.0 / float(H)

    nc = bacc.Bacc("TRN2", target_bir_lowering=False, debug=False,
                   num_devices=1 if single else N_CORES)
    nc._single_fake_ar = single

    xh = nc.dram_tensor("xh", [KF * 128, n_loc], FP16, kind="ExternalInput")
    xl = nc.dram_tensor("xl", [KF * 128, n_loc], FP16, kind="ExternalInput")
    xm = nc.dram_tensor("xm", [128, n_loc], FP16, kind="ExternalInput")
    sg1 = nc.dram_tensor("sg1", [KF * 128, H], FP8, kind="ExternalInput")
    sgm = nc.dram_tensor("sgm", [128, H], FP8, kind="ExternalInput")
    sg2 = nc.dram_tensor("sg2", [H, H], FP8, kind="ExternalInput")
    sg3 = nc.dram_tensor("sg3", [H, H], FP8, kind="ExternalInput")
    sg4 = nc.dram_tensor("sg4", [H, 16], FP8, kind="ExternalInput")
    w4n = nc.dram_tensor("w4n", [C, H], F32, kind="ExternalInput")
    g4v = nc.dram_tensor("g4v", [C, 1], F32, kind="ExternalInput")
    b4v = nc.dram_tensor("b4v", [C, 1], F32, kind="ExternalInput")
    yout = nc.dram_tensor("yout", [C, n_loc], F32, kind="ExternalOutput")

    xh_t = xh[:].rearrange("(t p) i -> t p i", p=128)
    xl_t = xl[:].rearrange("(t p) i -> t p i", p=128)
    sg1_t = sg1[:].rearrange("(t p) j -> t p j", p=128)
    sg2_t = sg2[:].rearrange("(m k p) j -> m k p j", k=128, p=2)
    sg3_t = sg3[:].rearrange("(m k p) j -> m k p j", k=128, p=2)
    sg4_t = sg4[:].rearrange("(m k p) j -> m k p j", k=128, p=2)

    salt = os.environ.get("BASS_SALT", "")
    if salt:
        nc.dram_tensor(f"salt_{salt}", [1, 4], F32, kind="Internal")
    with tile.TileContext(nc) as tc:
        for _rep in range(n_rep):
            _emit(tc, nc, n_loc, nch, inv_B, inv_H,
                  xh_t, xl_t, xm, sg1_t, sgm, sg2_t, sg3_t, sg4_t,
                  w4n, g4v, b4v, yout, _rep)
    nc.compile()
    return nc


def _emit(tc, nc, n_loc, nch, inv_B, inv_H,
          xh_t, xl_t, xm, sg1_t, sgm, sg2_t, sg3_t, sg4_t,
          w4n, g4v, b4v, yout, rep_idx=0):
    skip_in = SKIP_DMA_REP and rep_idx > 0
    def dma_in(dst, src):
        if not skip_in:
            nc.sync.dma_start(dst, src)
    import contextlib
    es = contextlib.ExitStack()
    with es:
        misc = es.enter_context(tc.tile_pool(name="misc", bufs=1))
        dram = es.enter_context(tc.tile_pool(name="dram", bufs=1, space="DRAM"))
        ps_main = es.enter_context(tc.tile_pool(name="ps_main", bufs=7, space="PSUM"))
        ps_small = es.enter_context(tc.tile_pool(name="ps_small", bufs=1, space="PSUM"))
        # paired activation tiles [128, 2, n_loc]; g1/g3 rotate through p_gA,
        # g2 lives in p_gB (opened after the layer-1 pool closes)
        p_gA = es.enter_context(tc.tile_pool(name="p_gA", bufs=8))
        # sigma pool A: s1 (7x [128,H] fp8) + s2 (8 pairs); s4 has own pool
        p_sA = es.enter_context(tc.tile_pool(name="p_sA", bufs=1))
        p_s4 = es.enter_context(tc.tile_pool(name="p_s4", bufs=1))
        p_l4c = es.enter_context(tc.tile_pool(name="p_l4c", bufs=1))

        def allreduce_rows(phi_src, nrows, name):
            # AllReduce a [nrows, H] f32 block of local colsum rows, then
            # read it back transposed as [128, JT, nrows] in one DMA.
            bi = dram.tile([nrows, H], F32, name=f"{name}_bi", tag=f"{name}_bi")
            nc.sync.dma_start(bi[:], phi_src)
            dst = misc.tile([128, JT, nrows], F32, name=f"{name}_ar",
                            tag=f"{name}_ar")
            if getattr(nc, "_single_fake_ar", False) or SKIP_AR:
                src = bi
            else:
                bo = dram.tile([nrows, H], F32, addr_space="Shared",
                               name=f"{name}_bo", tag=f"{name}_bo")
                nc.gpsimd.collective_compute(
                    "AllReduce", ALU.add,
                    replica_groups=[list(range(N_CORES))],
                    ins=[bi.opt()], outs=[bo.opt()],
                )
                src = bo
            for rrow in range(nrows):
                nc.sync.dma_start(
                    dst[:, :, rrow],
                    src[rrow:rrow+1, :].rearrange("r (t p) -> (r p) t", p=128))
            return dst

        def allreduce(sbuf_src, shape, name):
            # plain AllReduce (layer-4 BN stats)
            bi = dram.tile(shape, F32, name=f"{name}_bi", tag=f"{name}_bi")
            nc.sync.dma_start(bi[:], sbuf_src)
            dst = misc.tile(shape, F32, name=f"{name}_ar", tag=f"{name}_ar")
            if getattr(nc, "_single_fake_ar", False) or SKIP_AR:
                nc.sync.dma_start(dst[:], bi[:])
                return dst
            bo = dram.tile(shape, F32, addr_space="Shared",
                           name=f"{name}_bo", tag=f"{name}_bo")
            nc.gpsimd.collective_compute(
                "AllReduce", ALU.add,
                replica_groups=[list(range(N_CORES))],
                ins=[bi.opt()], outs=[bo.opt()],
            )
            nc.sync.dma_start(dst[:], bo[:])
            return dst

        def theta_scales(raw, name):
            # negated global mean (ACT Sign bias) and positive mean (DVE
            # is_ge threshold) from the combined colsum vector [128, JT]
            thn = misc.tile([128, JT], F32, name=f"{name}_n", tag=f"{name}_n")
            nc.vector.tensor_scalar_mul(thn[:], raw, -inv_B)
            thp = misc.tile([128, JT], F32, name=f"{name}_p", tag=f"{name}_p")
            nc.vector.tensor_scalar_mul(thp[:], raw, inv_B)
            return thn, thp

        def digitize_r(r, name):
            # local rowsums (exact ints, |r|<=2048) -> 3 balanced base-16
            # fp8 digits laid out [128, KT, 16] (digit dim padded to 16 so
            # the DR weight AP step is 16-byte aligned)
            rd = misc.tile([128, KT, 16], FP8, name=f"{name}_d", tag=f"{name}_d")
            t2 = misc.tile([128, KT], F32, name=f"{name}_t2", tag=f"{name}_t2")
            nc.vector.tensor_scalar(t2[:], r[:], 1.0 / 256.0, MAGIC,
                                    ALU.mult, ALU.add)
            q2 = misc.tile([128, KT], F32, name=f"{name}_q2", tag=f"{name}_q2")
            nc.vector.tensor_scalar(q2[:], t2[:], MAGIC, None, ALU.subtract)
            rem = misc.tile([128, KT], F32, name=f"{name}_rm", tag=f"{name}_rm")
            nc.vector.scalar_tensor_tensor(rem[:], q2[:], -256.0, r[:],
                                           ALU.mult, ALU.add)
            t1 = misc.tile([128, KT], F32, name=f"{name}_t1", tag=f"{name}_t1")
            nc.vector.tensor_scalar(t1[:], rem[:], 1.0 / 16.0, MAGIC,
                                    ALU.mult, ALU.add)
            q1 = misc.tile([128, KT], F32, name=f"{name}_q1", tag=f"{name}_q1")
            nc.vector.tensor_scalar(q1[:], t1[:], MAGIC, None, ALU.subtract)
            q0 = misc.tile([128, KT], F32, name=f"{name}_q0", tag=f"{name}_q0")
            nc.vector.scalar_tensor_tensor(q0[:], q1[:], -16.0, rem[:],
                                           ALU.mult, ALU.add)
            nc.vector.tensor_copy(rd[:, :, 0], q0[:])
            nc.vector.tensor_copy(rd[:, :, 1], q1[:])
            nc.vector.tensor_copy(rd[:, :, 2], q2[:])
            return rd

        def phi_tile(name):
            # one shared [8, H] row-block for all three theta matvecs (each
            # is DMA'd to the AR input buffer long before the next layer's
            # matvec runs, so a single buffer is WAR-safe)
            return misc.tile([8, H], F32, name=name, tag="phi", bufs=1)

        def matvec_T_dr(rd, sig_pairs, name):
            # local transposed matvec, DoubleRow: phi digit rows
            # phi[d, j] = sum_k digit_d(r[k]) * sig[j, k]
            phi = phi_tile(f"{name}_phi")
            for cj in range(NCHNK):
                ps = ps_small.tile([3, CHUNK], F32, name=f"mvT_{name}_{cj}",
                                   tag="ps_small")
                for m in range(NP):
                    nc.tensor.matmul(ps[:], rd[:, 2*m:2*m+2, 0:3],
                                     sig_pairs[m][:, :, CHUNK*cj:CHUNK*(cj+1)],
                                     start=(m == 0), stop=(m == NP - 1),
                                     perf_mode=DR)
                nc.vector.tensor_copy(phi[0:3, CHUNK*cj:CHUNK*(cj+1)],
                                      ps[:])
            return phi

        def combine_digits(dst, name):
            # dst: [128, JT, 3] AR'd digit planes -> [128, JT] colsums
            t01 = misc.tile([128, JT], F32, name=f"{name}_c1", tag=f"{name}_c1")
            nc.vector.scalar_tensor_tensor(t01[:], dst[:, :, 2], 16.0,
                                           dst[:, :, 1], ALU.mult, ALU.add)
            raw = misc.tile([128, JT], F32, name=f"{name}_c0", tag=f"{name}_c0")
            nc.vector.scalar_tensor_tensor(raw[:], t01[:], 16.0,
                                           dst[:, :, 0], ALU.mult, ALU.add)
            return raw

        def drains(gp_of, t, srcs, thn, r, accs_tag, lname, thp=None):
            # h = Sign(A - mean) in {-1,+1} fp8, on the Activation engine
            # (margins are >= 1/16384 with |A| << 1024, so the fp32 subtract
            # never rounds to exactly 0 and Sign never emits 0).
            # Odd j-tiles drain on DVE as (h+1)/2 in {0,1} via is_ge: the
            # per-tile affine encoding cancels in every downstream
            # mean-compare (and in the final BatchNorm).
            on_dve = thp is not None and (t % 2 == 1)
            accs = misc.tile([128, nch], F32, name=f"acc_{lname}_{t}",
                             tag=accs_tag, bufs=4) if r is not None else None
            for c in range(nch):
                sl = gp_of(t, c)
                if on_dve:
                    if r is not None:
                        nc.vector.tensor_scalar(sl, srcs[c], thp[:, t:t+1],
                                                None, ALU.is_ge, ALU.add,
                                                accum_out=accs[:, c:c+1])
                    else:
                        nc.vector.tensor_scalar(sl, srcs[c], thp[:, t:t+1],
                                                None, ALU.is_ge)
                elif r is not None:
                    nc.scalar.activation(sl, srcs[c], AF.Sign,
                                         bias=thn[:, t:t+1],
                                         accum_out=accs[:, c:c+1])
                else:
                    nc.scalar.activation(sl, srcs[c], AF.Sign,
                                         bias=thn[:, t:t+1])
            if r is not None:
                nc.vector.tensor_reduce(r[:, t:t+1], accs[:], axis=AX.X,
                                        op=ALU.add)

        def alloc_g_pairs(pool, tag, lname):
            return [pool.tile([128, 2, n_loc], FP8, name=f"g_{lname}_{m}",
                              tag=tag) for m in range(NP)]

        def bridge_tile(t, pss, lname, bridge_pool, bridge_bufs,
                        bdt=F32):
            # copy psums to SBUF right away (no theta dep) so the banks
            # free up while the AllReduce for theta is still in flight.
            # Even tiles copy on ACT, odd on DVE, matching the engine that
            # will drain them -- and these copies are emitted BEFORE any
            # theta-dependent op so the in-order queues never head-of-line
            # block on the collective.
            srcs = []
            for c in range(nch):
                tb = bridge_pool.tile([128, CHUNK], bdt,
                                      name=f"br_{lname}_{t}_{c}",
                                      tag="bridge", bufs=bridge_bufs)
                if t % 2 == 0:
                    nc.scalar.activation(tb[:], pss[c], AF.Identity)
                else:
                    nc.vector.tensor_copy(tb[:], pss[c])
                srcs.append(tb)
            return srcs

        def layer_dr(sig_pairs, gin_pairs, theta_cb, gout_pairs, want_r,
                     lname, bridge_pool=None, bridge_t=None,
                     bridge_bufs=None):
            if bridge_t is None:
                bridge_t = BRIDGE_T
            if bridge_bufs is None:
                bridge_bufs = BRIDGE_BUFS
            # DoubleRow fp8 layer: A = sigma @ (prev g), drained via ACT Sign
            r = misc.tile([128, JT], F32, name=f"r_{lname}", tag=f"r_{lname}") \
                if want_r else None
            gp_of = lambda tt, cc: gout_pairs[tt//2][:, tt % 2,
                                                     CHUNK*cc:CHUNK*(cc+1)]
            pend = []
            theta = thp = None
            for t in range(JT):
                pss = [ps_main.tile([128, CHUNK], F32,
                                    name=f"ps_{lname}_{t}_{c}", tag="ps_main")
                       for c in range(nch)]
                for m in range(NP):
                    lhs = sig_pairs[m][:, :, 128*t:128*(t+1)]
                    for c in range(nch):
                        nc.tensor.matmul(pss[c], lhs,
                                         gin_pairs[m][:, :, CHUNK*c:CHUNK*(c+1)],
                                         start=(m == 0), stop=(m == NP - 1),
                                         perf_mode=DR)
                if t < bridge_t:
                    pend.append((t, bridge_tile(t, pss, lname, bridge_pool,
                                                bridge_bufs, bdt=FP16)))
                    continue
                if theta is None:
                    # theta algebra lands in the queues only now, after all
                    # bridge copies, then the deferred drains
                    theta, thp = theta_cb()
                    for (tt, ss) in pend:
                        drains(gp_of, tt, ss, theta, r, "accs", lname,
                               thp=thp)
                    pend = []
                drains(gp_of, t, pss, theta, r, "accs", lname, thp=thp)
            return r

        # ---------------- layer 1: x load (chunked), sums, weight DMAs ------
        pl1_cm = tc.tile_pool(name="pl1", bufs=1)
        pl1 = pl1_cm.__enter__()

        # startup-critical DMA order: sg1 k-tile 0 first so the PE can begin
        # the first j-tile as soon as x k-tile 0 lands; x tiles next (they
        # pace the psum accumulation); bulk sigma tiles after.
        # Layer-1 K-pass kt: 2w -> (s1[w], xh[w]); 2w+1 -> (s1[w], xl[w]);
        # 12 -> (sgm, xm) mixed hi/lo tail tile.  The interleaved hi/lo
        # order keeps the psum accumulation close to the reference's
        # k-major fp32 summation order.
        s1_tiles = []
        xs_loc = misc.tile([128, KT1], F32, name="xs_loc", tag="xs_loc")
        x_tiles = []
        for w in range(KF):
            sgt = p_sA.tile([128, H], FP8, name=f"s1_{w}", tag="s1",
                            bufs=NW1)
            dma_in(sgt[:], sg1_t[w])
            s1_tiles.append(sgt)
            hi = pl1.tile([128, n_loc], FP16, name=f"xt0_{w}", tag="xhl",
                          bufs=KT1)
            dma_in(hi[:], xh_t[w])
            nc.vector.tensor_reduce(xs_loc[:, 2*w:2*w+1], hi[:], axis=AX.X,
                                    op=ALU.add)
            x_tiles.append(hi)
            lo = pl1.tile([128, n_loc], FP16, name=f"xt1_{w}", tag="xhl",
                          bufs=KT1)
            dma_in(lo[:], xl_t[w])
            nc.vector.tensor_reduce(xs_loc[:, 2*w+1:2*w+2], lo[:],
                                    axis=AX.X, op=ALU.add)
            x_tiles.append(lo)
        sgt = p_sA.tile([128, H], FP8, name="s1_m", tag="s1", bufs=NW1)
        dma_in(sgt[:], sgm[:])
        s1_tiles.append(sgt)          # s1_tiles[KF] = mixed weight plane
        xmt = pl1.tile([128, n_loc], FP16, name="xt_m", tag="xhl", bufs=KT1)
        dma_in(xmt[:], xm[:])
        nc.vector.tensor_reduce(xs_loc[:, 2*KF:2*KF+1], xmt[:], axis=AX.X,
                                op=ALU.add)
        x_tiles.append(xmt)

        if DMA_ONLY:
            # land every input, then bail out with a token output write
            for m in range(NP):
                sg = p_sA.tile([128, 2, H], FP8, name=f"s2_{m}", tag="s2",
                               bufs=NP)
                dma_in(sg[:], sg2_t[m])
            for m in range(NP):
                sg = p_sA.tile([128, 2, H], FP8, name=f"s3_{m}", tag="s3",
                               bufs=NP)
                dma_in(sg[:], sg3_t[m])
            for m in range(NP):
                sg = p_s4.tile([128, 2, 16], FP8, name=f"s4_{m}", tag="s4",
                               bufs=NP)
                dma_in(sg[:], sg4_t[m])
            w4sb = p_l4c.tile([C, H], F32, name="w4sb", tag="w4sb")
            dma_in(w4sb[:], w4n[:])
            nc.sync.dma_start(yout[0:10, 0:13], xs_loc[0:10, :])
            pl1_cm.__exit__(None, None, None)
            return

        def w1_of(kt):
            return s1_tiles[kt // 2] if kt < 2*KF else s1_tiles[KF]

        # per-weight-plane x rowsums: the hi and lo k-tiles of plane w
        # share sign weights, so their rowsums sum before the theta matvec
        xsv = misc.tile([128, NW1], F32, name="xsv", tag="xsv")
        xsi = xs_loc[:, 0:2*KF].rearrange("p (a b) -> p a b", b=2)
        nc.vector.tensor_add(xsv[:, 0:KF], xsi[:, :, 0], xsi[:, :, 1])
        nc.vector.tensor_copy(xsv[:, KF:NW1], xs_loc[:, 2*KF:2*KF+1])

        # exact fixed-point digitization: 8 balanced base-16 fp8 digits of
        # xsv * 2^17 (hierarchical RN extraction; every step exact in fp32,
        # only the final fractional drop rounds: <= 2^-18 per value).  The
        # theta matvec psums then accumulate exact small integers, so the
        # threshold has no accumulation-order rounding dice at all.
        xs2 = misc.tile([128, NW1, 8], FP8, name="xs2", tag="xs2")
        rk = misc.tile([128, NW1], F32, name="th1_rk", tag="th1_rk")
        nc.vector.tensor_scalar_mul(rk[:], xsv[:], float(2.0 ** -11))
        for d in range(7, -1, -1):
            tm = misc.tile([128, NW1], F32, name=f"th1_t{d}", tag="th1_tm",
                           bufs=2)
            nc.vector.tensor_scalar(tm[:], rk[:], MAGIC, None, ALU.add)
            dg = misc.tile([128, NW1], F32, name=f"th1_d{d}", tag="th1_dg",
                           bufs=2)
            nc.vector.tensor_scalar(dg[:], tm[:], MAGIC, None, ALU.subtract)
            nc.vector.tensor_copy(xs2[:, :, d], dg[:])
            if d > 0:
                rem = misc.tile([128, NW1], F32, name=f"th1_r{d}",
                                tag="th1_rm", bufs=2)
                nc.vector.scalar_tensor_tensor(rem[:], dg[:], -1.0, rk[:],
                                               ALU.mult, ALU.add)
                rk = misc.tile([128, NW1], F32, name=f"th1_k{d}",
                               tag="th1_rk2", bufs=2)
                nc.vector.tensor_scalar_mul(rk[:], rem[:], 16.0)

        def zero_theta(name):
            z = misc.tile([128, JT], F32, name=name, tag=name)
            nc.vector.tensor_scalar_mul(z[:, 0:KT1], xs_loc[:], 0.0)
            nc.vector.tensor_scalar_mul(z[:, KT1:JT], xs_loc[:, 0:JT-KT1], 0.0)
            return z

        # local transposed matvec for theta1 over the 7 weight planes
        phi1 = phi_tile("phi1")
        for cj in range(NCHNK):
            ps = ps_small.tile([8, CHUNK], F32, name=f"mvT_th1_{cj}",
                               tag="ps_small")
            for w in range(NW1):
                nc.tensor.matmul(ps[:], xs2[:, w, :],
                                 s1_tiles[w][:, CHUNK*cj:CHUNK*(cj+1)],
                                 start=(w == 0), stop=(w == NW1 - 1))
            nc.vector.tensor_copy(phi1[0:8, CHUNK*cj:CHUNK*(cj+1)], ps[:])
        if NO_THETA:
            def th1_cb():
                z1 = zero_theta("zth1")
                return theta_scales(z1[:], "th1")
        else:
            D1 = allreduce_rows(phi1[0:8, :], 8, "phi1")

            def th1_cb():
                # theta = sum_d D1[..d] * 16^d * 2^-17 / B, smallest digit
                # first so the rounding stays at ~2 ulp of the result
                acc = misc.tile([128, JT], F32, name="th1_a0", tag="th1_acc",
                                bufs=2)
                nc.vector.tensor_scalar_mul(acc[:], D1[:, :, 0],
                                            float(2.0 ** -17) * inv_B)
                for d in range(1, 8):
                    nxt = misc.tile([128, JT], F32, name=f"th1_a{d}",
                                    tag="th1_acc", bufs=2)
                    nc.vector.scalar_tensor_tensor(
                        nxt[:], D1[:, :, d], float(2.0 ** (4*d - 17)) * inv_B,
                        acc[:], ALU.mult, ALU.add)
                    acc = nxt
                thn = misc.tile([128, JT], F32, name="th1_n", tag="th1_n")
                nc.vector.tensor_scalar_mul(thn[:], acc[:], -1.0)
                return thn, acc

        # sigma2 pair tiles: direct DMA of host-signed fp8 planes
        s2_pairs = []
        for m in range(NP):
            sg = p_sA.tile([128, 2, H], FP8, name=f"s2_{m}", tag="s2", bufs=NP)
            dma_in(sg[:], sg2_t[m])
            s2_pairs.append(sg)

        # layer-4 statics (tiny): land them during layer 1 so the DMA queue
        # is empty around every AllReduce
        s4_pairs = []
        for m in range(NP):
            sg = p_s4.tile([128, 2, 16], FP8, name=f"s4_{m}", tag="s4", bufs=NP)
            dma_in(sg[:], sg4_t[m])
            s4_pairs.append(sg)
        w4sb = p_l4c.tile([C, H], F32, name="w4sb", tag="w4sb")
        dma_in(w4sb[:], w4n[:])
        g4sb = misc.tile([C, 1], F32, name="g4sb", tag="g4sb")
        dma_in(g4sb[:], g4v[:])
        b4sb = misc.tile([C, 1], F32, name="b4sb", tag="b4sb")
        dma_in(b4sb[:], b4v[:])

        s4raw = misc.tile([C, 1], F32, name="s4raw", tag="s4raw")
        nc.vector.tensor_reduce(s4raw[:], w4sb[:], axis=AX.X, op=ALU.add,
                                apply_absolute_value=True)
        s4 = misc.tile([C, 1], F32, name="s4", tag="s4sc")
        nc.vector.tensor_scalar_mul(s4[:], s4raw[:], inv_H)     # mean|w4|
        ns4sq = misc.tile([C, 1], F32, name="ns4sq", tag="ns4sq")
        nc.vector.tensor_scalar(ns4sq[:], s4[:], s4[:], -1.0,
                                ALU.mult, ALU.mult)              # -s4^2

        # ---------------- layer 1 main (13-pass 2-term fp16 x fp8 sign) -----
        g1_pairs = alloc_g_pairs(p_gA, "gA", "l1")
        r1 = misc.tile([128, JT], F32, name="r_l1", tag="r_l1")
        gp1 = lambda tt, cc: g1_pairs[tt//2][:, tt % 2,
                                             CHUNK*cc:CHUNK*(cc+1)]
        pend1 = []
        theta1 = thp1 = None
        for t in range(JT):
            pss = [ps_main.tile([128, CHUNK], F32, name=f"ps_l1_{t}_{c}",
                                tag="ps_main") for c in range(nch)]
            for kt in range(KT1):
                lhs = w1_of(kt)[:, 128*t:128*(t+1)]
                for c in range(nch):
                    nc.tensor.matmul(pss[c], lhs,
                                     x_tiles[kt][:, CHUNK*c:CHUNK*(c+1)],
                                     start=(kt == 0), stop=(kt == KT1 - 1))
            if t < BRIDGE_T:
                pend1.append((t, bridge_tile(t, pss, "l1", pl1,
                                             BRIDGE_BUFS)))
            else:
                if theta1 is None:
                    theta1, thp1 = th1_cb()
                    for (tt, ss) in pend1:
                        drains(gp1, tt, ss, theta1, r1, "accs", "l1",
                               thp=thp1)
                    pend1 = []
                drains(gp1, t, pss, theta1, r1, "accs", "l1", thp=thp1)
            if t == 3:
                # sigma3 pair tiles: DMA mid-layer-1 (queue is idle then;
                # landing them early keeps the phi AllReduce DMAs and the
                # ACT drain stream unblocked at the layer boundaries)
                s3_pairs = []
                for m in range(NP):
                    sg = p_sA.tile([128, 2, H], FP8, name=f"s3_{m}",
                                   tag="s3", bufs=NP)
                    dma_in(sg[:], sg3_t[m])
                    s3_pairs.append(sg)

        for _e in range(N_EXTRA_L1):
            for t in range(JT):
                pss = [ps_main.tile([128, CHUNK], F32,
                                    name=f"px1_{_e}_{t}_{c}", tag="ps_main")
                       for c in range(nch)]
                for kt in range(KT1):
                    lhs = w1_of(kt)[:, 128*t:128*(t+1)]
                    for c in range(nch):
                        nc.tensor.matmul(pss[c], lhs,
                                         x_tiles[kt][:, CHUNK*c:CHUNK*(c+1)],
                                         start=(kt == 0),
                                         stop=(kt == KT1 - 1))

        pl1_cm.__exit__(None, None, None)

        p_gB = es.enter_context(tc.tile_pool(name="p_gB", bufs=8))
        brB_cm = tc.tile_pool(name="brB", bufs=1)
        brB = brB_cm.__enter__()

        if NO_THETA:
            th2_cb = lambda: theta_scales(zero_theta("zth2")[:], "th2")
        else:
            rd1 = digitize_r(r1, "rd1")
            phi2 = matvec_T_dr(rd1, s2_pairs, "th2")
            D2 = allreduce_rows(phi2[0:3, :], 3, "phi2")
            th2_cb = lambda: theta_scales(combine_digits(D2, "th2")[:],
                                          "th2")

        g2_pairs = alloc_g_pairs(p_gB, "gB", "l2")
        r2 = layer_dr(s2_pairs, g1_pairs, th2_cb, g2_pairs, True, "l2",
                      bridge_pool=brB, bridge_t=5, bridge_bufs=20)

        if NO_THETA:
            th3_cb = lambda: theta_scales(zero_theta("zth3")[:], "th3")
        else:
            rd2 = digitize_r(r2, "rd2")
            phi3 = matvec_T_dr(rd2, s3_pairs, "th3")
            D3 = allreduce_rows(phi3[0:3, :], 3, "phi3")
            th3_cb = lambda: theta_scales(combine_digits(D3, "th3")[:],
                                          "th3")

        g3_pairs = alloc_g_pairs(p_gA, "gA", "l3")
        th3_memo = []

        def th3_once():
            if not th3_memo:
                th3_memo.append(th3_cb())
            return th3_memo[0]

        layer_dr(s3_pairs, g2_pairs, th3_once, g3_pairs, False, "l3",
                 bridge_pool=brB, bridge_t=5, bridge_bufs=20)

        if N_EXTRA:
            # dummy layers recycle dead pools: s2 (dead after L2) and
            # gB/g2 (dead once dummy0 replaces g3 as the consumer)
            assert N_EXTRA <= 2
            gin = g3_pairs
            for e in range(N_EXTRA):
                if MM_ONLY_EXTRA:
                    for t in range(JT):
                        pss = [ps_main.tile([128, CHUNK], F32,
                                            name=f"px_{e}_{t}_{c}",
                                            tag="ps_main")
                               for c in range(nch)]
                        for m in range(NP):
                            lhs = s3_pairs[m][:, :, 128*t:128*(t+1)]
                            for c in range(nch):
                                nc.tensor.matmul(
                                    pss[c], lhs,
                                    gin[m][:, :, CHUNK*c:CHUNK*(c+1)],
                                    start=(m == 0), stop=(m == NP - 1),
                                    perf_mode=DR)
                    continue
                pool, tag = (p_sA, "s2") if e == 0 else (p_gB, "gB")
                gx = [pool.tile([128, 2, n_loc], FP8, name=f"g_x{e}_{m}",
                                tag=tag, bufs=NP) for m in range(NP)]
                layer_dr(s3_pairs, gin, th3_once, gx, False, f"lx{e}")
                gin = gx

        brB_cm.__exit__(None, None, None)
        p_l4 = es.enter_context(tc.tile_pool(name="p_l4", bufs=1))

        # ---------------- layer 4 + BatchNorm ----------------
        # k4 = sign(W4) @ h3 (exact small integers); BN statistics of
        # y4 = s4*k4 are folded into per-channel affine coefficients so the
        # only full-width op after the AllReduce is one tensor_scalar.
        k4 = p_l4.tile([C, n_loc], F32, name="k4", tag="k4")
        ksum = misc.tile([C, nch], F32, name="ksum", tag="ksum")
        ksq = misc.tile([C, nch], F32, name="ksq", tag="ksq")
        for c in range(nch):
            ps = ps_small.tile([16, CHUNK], F32, name=f"ps_l4_{c}", tag="ps_small")
            for m in range(NP):
                nc.tensor.matmul(ps[:], s4_pairs[m][:],
                                 g3_pairs[m][:, :, CHUNK*c:CHUNK*(c+1)],
                                 start=(m == 0), stop=(m == NP - 1),
                                 perf_mode=DR)
            nc.vector.tensor_scalar(k4[:, CHUNK*c:CHUNK*(c+1)], ps[0:C, :],
                                    1.0, None, ALU.mult, ALU.add,
                                    accum_out=ksum[:, c:c+1])
            sqsc = p_l4.tile([C, CHUNK], F32, name=f"sq_{c}", tag="sqsc",
                             bufs=2)
            nc.scalar.activation(sqsc[:], ps[0:C, :], AF.Square,
                                 accum_out=ksq[:, c:c+1])

        # pre-scale the stats by 1/B before the AllReduce: the reduced
        # result is then directly (mu_k, E[k^2])
        p4 = misc.tile([C, 2], F32, name="p4", tag="p4")
        nc.vector.tensor_reduce(p4[:, 0:1], ksum[:], axis=AX.X, op=ALU.add)
        nc.vector.tensor_reduce(p4[:, 1:2], ksq[:], axis=AX.X, op=ALU.add)
        p4s = misc.tile([C, 2], F32, name="p4s", tag="p4s")
        nc.vector.tensor_scalar_mul(p4s[:], p4[:], inv_B)

        G4 = allreduce(p4s[:], [C, 2], "p4")
        # -var = mu^2 - E[k^2];  veps = (-var)*(-s4^2) + eps
        nvar = misc.tile([C, 1], F32, name="nvar", tag="nvar")
        nc.vector.tensor_scalar(nvar[:], G4[:, 0:1], G4[:, 0:1], G4[:, 1:2],
                                ALU.mult, ALU.subtract)
        veps = misc.tile([C, 1], F32, name="veps", tag="veps")
        nc.vector.tensor_scalar(veps[:], nvar[:], ns4sq[:], 1e-5,
                                ALU.mult, ALU.add)
        sd = misc.tile([C, 1], F32, name="sd", tag="sd")
        nc.scalar.activation(sd[:], veps[:], AF.Sqrt)
        inv_sd = misc.tile([C, 1], F32, name="inv_sd", tag="inv_sd")
        nc.vector.reciprocal(inv_sd[:], sd[:])
        # A = g*s4/sd(y);  B0 = b - mu_k*A
        A4 = misc.tile([C, 1], F32, name="A4", tag="A4")
        nc.vector.tensor_scalar(A4[:], inv_sd[:], g4sb[:], s4[:],
                                ALU.mult, ALU.mult)
        muA = misc.tile([C, 1], F32, name="muA", tag="muA")
        nc.vector.tensor_mul(muA[:], G4[:, 0:1], A4[:])
        B4 = misc.tile([C, 1], F32, name="B4", tag="B4")
        nc.vector.tensor_sub(B4[:], b4sb[:], muA[:])

        # final affine: split halves across DVE and ACT, DMA out per half
        yo = p_l4.tile([C, n_loc], F32, name="yo", tag="yo")
        hn = n_loc // 2
        nc.vector.tensor_scalar(yo[:, 0:hn], k4[:, 0:hn], A4[:], B4[:],
                                ALU.mult, ALU.add)
        nc.sync.dma_start(yout[:, 0:hn], yo[:, 0:hn])
        nc.scalar.activation(yo[:, hn:], k4[:, hn:], AF.Identity,
                             bias=B4[:], scale=A4[:])
        nc.sync.dma_start(yout[:, hn:], yo[:, hn:])


# --------------------------------------------------------------------------
def prep_inputs(x, w1, w2, w3, w4, g4, b4, n_loc):
    n_cores = N_CORES
    B = x.shape[0]
    assert B == n_loc * n_cores

    f16 = np.float16

    def signs(w):
        return np.where(np.asarray(w) >= 0, np.float32(1.0),
                        np.float32(-1.0))

    # sg1: [768, H], row k = sign(w1[:, k]); sgm: [128, H] mixed tail plane
    # (rows 0-15 = sign rows 768-783, rows 16-31 the same, rest zero)
    s1 = signs(w1).T.astype(NP8)          # [784, 2048]
    sg1 = np.ascontiguousarray(s1[:KF*128])
    sgm = np.zeros((128, H), dtype=NP8)
    sgm[0:D_IN - KF*128] = s1[KF*128:]
    sgm[16:16 + D_IN - KF*128] = s1[KF*128:]

    def paired(w, width):
        # [H, width]: rows (m, k, p) -> sign(w[j, 128*(2m+p)+k]), j < width.
        # Odd input-tile planes (p=1) are scaled x2: those tiles' activations
        # are stored as (h+1)/2 in {0,1} by the DVE drain path.
        s = signs(w)                       # [out, H]
        out = np.zeros((NP, 128, 2, width), dtype=NP8)
        for m in range(NP):
            for p in (0, 1):
                blk = s[:, 128*(2*m+p):128*(2*m+p+1)] * (1.0 + p)
                out[m, :, p, :blk.shape[0]] = blk.T.astype(NP8)
        return out.reshape(H, width)

    sg2 = paired(w2, H)
    sg3 = paired(w3, H)
    sg4 = paired(w4, 16)
    w4n = np.ascontiguousarray(np.asarray(w4).astype(np.float32))
    g4v = np.ascontiguousarray(np.asarray(g4).reshape(C, 1).astype(np.float32))
    b4v = np.ascontiguousarray(np.asarray(b4).reshape(C, 1).astype(np.float32))

    in_maps = []
    ntail = D_IN - KF*128                  # 16 tail rows
    for cidx in range(n_cores):
        xs = x[n_loc*cidx:n_loc*(cidx+1)]
        xT = np.ascontiguousarray(xs.T.astype(np.float32))   # [784, n_loc]
        xhp = xT.astype(f16)
        xlp = (xT - xhp.astype(np.float32)).astype(f16)
        xmv = np.zeros((128, n_loc), dtype=f16)
        xmv[0:ntail] = xhp[KF*128:]
        xmv[16:16 + ntail] = xlp[KF*128:]
        m = {
            "xh": np.ascontiguousarray(xhp[:KF*128]),
            "xl": np.ascontiguousarray(xlp[:KF*128]),
            "xm": xmv,
            "sg1": sg1, "sgm": sgm, "sg2": sg2, "sg3": sg3,
            "sg4": sg4, "w4n": w4n, "g4v": g4v, "b4v": b4v,
        }
        in_maps.append(m)
    return in_maps


_NC_CACHE = {}


def kernel(x, w1, w2, w3, w4, g1, b1, g2, b2, g3, b3, g4, b4):
    x = np.asarray(x); w1 = np.asarray(w1); w2 = np.asarray(w2)
    w3 = np.asarray(w3); w4 = np.asarray(w4)
    g4 = np.asarray(g4); b4 = np.asarray(b4)
    # layers 1-3 BN params: scales cancel inside sign() only when gamma>0, beta=0
    for g in (g1, g2, g3):
        assert np.all(np.asarray(g) > 0), "kernel assumes gamma > 0 for hidden BNs"
    for b in (b1, b2, b3):
        assert np.all(np.asarray(b) == 0), "kernel assumes beta == 0 for hidden BNs"
    for w in (w1, w2, w3, w4):
        assert not np.any(w == 0.0), "exact-zero weight would break Sign()"

    n_loc = x.shape[0] // N_CORES
    if n_loc not in _NC_CACHE:
        _NC_CACHE[n_loc] = build(n_loc)
    nc = _NC_CACHE[n_loc]

    in_maps = prep_inputs(x, w1, w2, w3, w4, g4, b4, n_loc)
    res = bass_utils.run_bass_kernel_spmd(nc, in_maps,
                                          core_ids=list(range(N_CORES)))
    out = np.concatenate([res.results[c]["yout"].T for c in range(N_CORES)],
                         axis=0)
    return out.astype(np.float32)


# revision 37
# speedup vs baseline: 1.0225x; 1.0225x over previous
"""BinaryNet MLP forward on 8 TRN2 NeuronCores.

Strategy: data-parallel over batch (2048 rows/core), feature-major on-chip
layout (activations stored [channel, batch]).  For layers 1-3 the positive
per-row weight scales and the BatchNorm variance cancel inside sign(), so
each layer reduces to:  g_l = 1{ A_l >= mean_batch(A_l) }  where
A_l = sign(W_l) @ h_{l-1} is an exact small integer computed with fp8 {+-1}
activations x fp8 {+-1} weights on the TensorEngine.  h_l is produced by
the Activation engine as Sign(A - mean) (integer margins >= 1/16384 make
the fp32 subtract sign-safe).  Layers 2-4 run fp8 DoubleRow (two
K-subtiles per pass).  Layer 1 (continuous x) uses a 2-term fp16 split of
x that reproduces the reference's fp32 sign decisions; the hi and lo
tails of the 7th (7/8-padding) k-tile share one mixed k-tile, so layer 1
runs 13 K-passes instead of 14 while the six full-width sign-weight tiles
stay shared between the hi and lo passes.  Layer 4 applies the real
BatchNorm with weight scales.

Batch means: colsum(A_l) = sigma_l @ rowsum(h_{l-1}) distributes over the
AllReduce, so each core computes a LOCAL transposed matvec
phi = rowsum_local^T @ sigma  (rowsum stationary: LDWEIGHTS ~free; sigma
planes stream as the moving operand, DoubleRow pairs for layers 2/3) and
the AllReduce carries the phi rows instead of the rowsums -- similar
payload, but the LDWEIGHTS-bound PE matvec of the old formulation
disappears and the matvec no longer sits between the AllReduce and the
drains.  Theta is read back from the AR buffer with transposing DMAs
("r (t p) -> p t") and the phi rows are combined after, in
partition-aligned DVE ops.  For layers 2/3 the local rowsums are exact
integers |r| <= 2048, shipped to the PE as 3 balanced base-16 fp8
digits, so the threshold psums accumulate exact small integers; layer
1's theta uses an 8-digit base-16 fixed-point split of the local x
rowsums (2^-17 resolution, hierarchically extracted with exact fp32
steps) for the same reason -- the sign margins bottom out at ~1e-6 and
ANY accumulation-ordering dice in theta flips h1 entries, which the
binary net amplifies ~36x per layer (10 flips => 6% final error).

Engine-queue discipline at the AllReduce boundaries: the ACT/DVE queues
are strict in-order FIFOs, so theta-dependent ops must not be enqueued
ahead of ready work.  Each layer emits: matmuls for the first BRIDGE_T
j-tiles with psum->SBUF bridge copies (fp16 for the integer DR layers,
f32 for layer 1; even tiles copy on ACT, odd on DVE), THEN the theta
combine/scale algebra, THEN the deferred drains -- so the bridges free
psum banks while the collective is still in flight and the PE never
head-of-line blocks (worth ~30us over the naive order).  PSUM runs with
7 main accumulation banks + 1 for the theta matvecs/layer 4.

All sign-weights ship pre-signed from the host as fp8 {+-1} planes
(pre-paired for DoubleRow), so the ACT engine never runs sign-prep and
the DMA queue never serializes weight staging against the AllReduce path.

Measured (8-core axon TRN2, slope-of-n_rep method): ~477-490us vs the
538.8us/519.5us baseline; rel err 3.09e-07, zero flipped rows.
"""
import sys, os
sys.path.insert(0, '/opt/trn_rl_repo')
import numpy as np
import ml_dtypes

import concourse.bass as bass
import concourse.bacc as bacc
import concourse.tile as tile
import concourse.mybir as mybir
from concourse import bass_utils

F32 = mybir.dt.float32
BF16 = mybir.dt.bfloat16
FP16 = mybir.dt.float16
FP8 = mybir.dt.float8e4
AF = mybir.ActivationFunctionType
ALU = mybir.AluOpType
AX = mybir.AxisListType
DR = mybir.MatmulPerfMode.DoubleRow

NP8 = mybir.dt.np(mybir.dt.float8e4)

N_CORES = 8
D_IN, H, C = 784, 2048, 10
KF = 6                 # full 128-row k-tiles of x (768 rows)
KT1 = 2 * KF + 1       # 13 layer-1 K-passes (6 hi + 6 lo + 1 mixed)
NW1 = KF + 1           # 7 distinct layer-1 weight planes
KT = H // 128          # 16 k-tiles for layers 2-4
NP = KT // 2           # 8 DoubleRow k-pairs
JT = H // 128          # 16 output-channel tiles
CHUNK = 512
NCHNK = H // CHUNK     # 4 j-chunks of 512 for the theta matvecs
BRIDGE_BUFS = 12       # SBUF bridge slots for psum->sbuf theta-decoupling
BRIDGE_T = 3           # j-tiles per layer whose psums get bridged
MAGIC = float(3 << 22)  # fp32 round-to-nearest-int magic constant

# Timing-experiment knobs (leave defaults for correct results)
SKIP_DMA_REP = False   # skip input DMAs for rep>0 (garbage data, timing only)
SKIP_AR = False        # replace AllReduces with local DMA copies (wrong
                       # results on 8 cores, timing only)
DMA_ONLY = False       # emit only the input DMAs (timing the DMA floor)
NO_THETA = False       # constant thresholds: no matvec/AR/theta path at all
N_EXTRA = 0            # extra dummy DR layers after L3 (timing calibration)
MM_ONLY_EXTRA = False  # dummy layers emit only the matmuls (no drains)
N_EXTRA_L1 = 0         # extra MM-only replays of the L1 loop (timing)
LDW_OPT = False        # pass --enable-ldw-opt=true to walrus (dedup LDWs)


def _install_ldw_opt_patch():
    from concourse import bass_utils as _bu
    if getattr(_bu, "_ldw_patch", None):
        return
    _orig = _bu.run_command

    def _patched(argv, **kw):
        if LDW_OPT and any("walrus" in str(a) for a in argv[:1]):
            argv = ["--enable-ldw-opt=true" if a == "--enable-ldw-opt=false"
                    else a for a in argv]
        return _orig(argv, **kw)

    _bu.run_command = _patched
    _bu._ldw_patch = True


_install_ldw_opt_patch()


def build(n_loc: int, single: bool = False, n_rep: int = 1):
    """Emit the SPMD program for one core (all 8 run it on their own shard).

    single=True builds a 1-core variant with AllReduces replaced by plain
    copies (for cost-model timeline analysis).  n_rep repeats the whole
    forward pass back-to-back (device-time benchmarking)."""
    nch = n_loc // CHUNK
    assert n_loc % CHUNK == 0
    inv_B = 1.0 / float(n_loc * N_CORES)   # exact: power of two
    inv_H = 1.0 / float(H)

    nc = bacc.Bacc("TRN2", target_bir_lowering=False, debug=False,
                   num_devices=1 if single else N_CORES)
    nc._single_fake_ar = single

    xh = nc.dram_tensor("xh", [KF * 128, n_loc], FP16, kind="ExternalInput")
    xl = nc.dram_tensor("xl", [KF * 128, n_loc], FP16, kind="ExternalInput")
    xm = nc.dram_tensor("xm", [128, n_loc], FP16, kind="ExternalInput")
    sg1 = nc.dram_tensor("sg1", [KF * 128, H], FP8, kind="ExternalInput")
    sgm = nc.dram_tensor("sgm", [128, H], FP8, kind="ExternalInput")
    sg2 = nc.dram_tensor("sg2", [H, H], FP8, kind="ExternalInput")
    sg3 = nc.dram_tensor("sg3", [H, H], FP8, kind="ExternalInput")
    sg4 = nc.dram_tensor("sg4", [H, 16], FP8, kind="ExternalInput")
    w4n = nc.dram_tensor("w4n", [C, H], F32, kind="ExternalInput")
    g4v = nc.dram_tensor("g4v", [C, 1], F32, kind="ExternalInput")
    b4v = nc.dram_tensor("b4v", [C, 1], F32, kind="ExternalInput")
    yout = nc.dram_tensor("yout", [C, n_loc], F32, kind="ExternalOutput")

    xh_t = xh[:].rearrange("(t p) i -> t p i", p=128)
    xl_t = xl[:].rearrange("(t p) i -> t p i", p=128)
    sg1_t = sg1[:].rearrange("(t p) j -> t p j", p=128)
    sg2_t = sg2[:].rearrange("(m k p) j -> m k p j", k=128, p=2)
    sg3_t = sg3[:].rearrange("(m k p) j -> m k p j", k=128, p=2)
    sg4_t = sg4[:].rearrange("(m k p) j -> m k p j", k=128, p=2)

    salt = os.environ.get("BASS_SALT", "")
    if salt:
        nc.dram_tensor(f"salt_{salt}", [1, 4], F32, kind="Internal")
    with tile.TileContext(nc) as tc:
        for _rep in range(n_rep):
            _emit(tc, nc, n_loc, nch, inv_B, inv_H,
                  xh_t, xl_t, xm, sg1_t, sgm, sg2_t, sg3_t, sg4_t,
                  w4n, g4v, b4v, yout, _rep)
    nc.compile()
    return nc


def _emit(tc, nc, n_loc, nch, inv_B, inv_H,
          xh_t, xl_t, xm, sg1_t, sgm, sg2_t, sg3_t, sg4_t,
          w4n, g4v, b4v, yout, rep_idx=0):
    skip_in = SKIP_DMA_REP and rep_idx > 0
    def dma_in(dst, src):
        if not skip_in:
            nc.sync.dma_start(dst, src)
    import contextlib
    es = contextlib.ExitStack()
    with es:
        misc = es.enter_context(tc.tile_pool(name="misc", bufs=1))
        dram = es.enter_context(tc.tile_pool(name="dram", bufs=1, space="DRAM"))
        ps_main = es.enter_context(tc.tile_pool(name="ps_main", bufs=8, space="PSUM"))
        ps_small = ps_main
        # paired activation tiles [128, 2, n_loc]; g1/g3 rotate through p_gA,
        # g2 lives in p_gB (opened after the layer-1 pool closes)
        p_gA = es.enter_context(tc.tile_pool(name="p_gA", bufs=8))
        # sigma pool A: s1 (7x [128,H] fp8) + s2 (8 pairs); s4 has own pool
        p_sA = es.enter_context(tc.tile_pool(name="p_sA", bufs=1))
        p_s4 = es.enter_context(tc.tile_pool(name="p_s4", bufs=1))
        p_l4c = es.enter_context(tc.tile_pool(name="p_l4c", bufs=1))

        def allreduce_rows(phi_src, nrows, name):
            # AllReduce a [nrows, H] f32 block of local colsum rows, then
            # read it back transposed as [128, JT, nrows] in one DMA.
            bi = dram.tile([nrows, H], F32, name=f"{name}_bi", tag=f"{name}_bi")
            nc.sync.dma_start(bi[:], phi_src)
            dst = misc.tile([128, JT, nrows], F32, name=f"{name}_ar",
                            tag=f"{name}_ar")
            if getattr(nc, "_single_fake_ar", False) or SKIP_AR:
                src = bi
            else:
                bo = dram.tile([nrows, H], F32, addr_space="Shared",
                               name=f"{name}_bo", tag=f"{name}_bo")
                nc.gpsimd.collective_compute(
                    "AllReduce", ALU.add,
                    replica_groups=[list(range(N_CORES))],
                    ins=[bi.opt()], outs=[bo.opt()],
                )
                src = bo
            for rrow in range(nrows):
                nc.sync.dma_start(
                    dst[:, :, rrow],
                    src[rrow:rrow+1, :].rearrange("r (t p) -> (r p) t", p=128))
            return dst

        def allreduce(sbuf_src, shape, name):
            # plain AllReduce (layer-4 BN stats)
            bi = dram.tile(shape, F32, name=f"{name}_bi", tag=f"{name}_bi")
            nc.sync.dma_start(bi[:], sbuf_src)
            dst = misc.tile(shape, F32, name=f"{name}_ar", tag=f"{name}_ar")
            if getattr(nc, "_single_fake_ar", False) or SKIP_AR:
                nc.sync.dma_start(dst[:], bi[:])
                return dst
            bo = dram.tile(shape, F32, addr_space="Shared",
                           name=f"{name}_bo", tag=f"{name}_bo")
            nc.gpsimd.collective_compute(
                "AllReduce", ALU.add,
                replica_groups=[list(range(N_CORES))],
                ins=[bi.opt()], outs=[bo.opt()],
            )
            nc.sync.dma_start(dst[:], bo[:])
            return dst

        def theta_scales(raw, name):
            # negated global mean (ACT Sign bias) and positive mean (DVE
            # is_ge threshold) from the combined colsum vector [128, JT]
            thn = misc.tile([128, JT], F32, name=f"{name}_n", tag=f"{name}_n")
            nc.vector.tensor_scalar_mul(thn[:], raw, -inv_B)
            thp = misc.tile([128, JT], F32, name=f"{name}_p", tag=f"{name}_p")
            nc.vector.tensor_scalar_mul(thp[:], raw, inv_B)
            return thn, thp

        def digitize_r(r, name):
            # local rowsums (exact ints, |r|<=2048) -> 3 balanced base-16
            # fp8 digits laid out [128, KT, 16] (digit dim padded to 16 so
            # the DR weight AP step is 16-byte aligned)
            rd = misc.tile([128, KT, 16], FP8, name=f"{name}_d", tag=f"{name}_d")
            t2 = misc.tile([128, KT], F32, name=f"{name}_t2", tag=f"{name}_t2")
            nc.vector.tensor_scalar(t2[:], r[:], 1.0 / 256.0, MAGIC,
                                    ALU.mult, ALU.add)
            q2 = misc.tile([128, KT], F32, name=f"{name}_q2", tag=f"{name}_q2")
            nc.vector.tensor_scalar(q2[:], t2[:], MAGIC, None, ALU.subtract)
            rem = misc.tile([128, KT], F32, name=f"{name}_rm", tag=f"{name}_rm")
            nc.vector.scalar_tensor_tensor(rem[:], q2[:], -256.0, r[:],
                                           ALU.mult, ALU.add)
            t1 = misc.tile([128, KT], F32, name=f"{name}_t1", tag=f"{name}_t1")
            nc.vector.tensor_scalar(t1[:], rem[:], 1.0 / 16.0, MAGIC,
                                    ALU.mult, ALU.add)
            q1 = misc.tile([128, KT], F32, name=f"{name}_q1", tag=f"{name}_q1")
            nc.vector.tensor_scalar(q1[:], t1[:], MAGIC, None, ALU.subtract)
            q0 = misc.tile([128, KT], F32, name=f"{name}_q0", tag=f"{name}_q0")
            nc.vector.scalar_tensor_tensor(q0[:], q1[:], -16.0, rem[:],
                                           ALU.mult, ALU.add)
            nc.vector.tensor_copy(rd[:, :, 0], q0[:])
            nc.vector.tensor_copy(rd[:, :, 1], q1[:])
            nc.vector.tensor_copy(rd[:, :, 2], q2[:])
            return rd

        def phi_tile(name):
            # one shared [8, H] row-block for all three theta matvecs (each
            # is DMA'd to the AR input buffer long before the next layer's
            # matvec runs, so a single buffer is WAR-safe)
            return misc.tile([8, H], F32, name=name, tag="phi", bufs=1)

        def matvec_T_dr(rd, sig_pairs, name):
            # local transposed matvec, DoubleRow: phi digit rows
            # phi[d, j] = sum_k digit_d(r[k]) * sig[j, k]
            phi = phi_tile(f"{name}_phi")
            for cj in range(NCHNK):
                ps = ps_small.tile([3, CHUNK], F32, name=f"mvT_{name}_{cj}",
                                   tag="ps_main", bufs=8)
                for m in range(NP):
                    nc.tensor.matmul(ps[:], rd[:, 2*m:2*m+2, 0:3],
                                     sig_pairs[m][:, :, CHUNK*cj:CHUNK*(cj+1)],
                                     start=(m == 0), stop=(m == NP - 1),
                                     perf_mode=DR)
                nc.vector.tensor_copy(phi[0:3, CHUNK*cj:CHUNK*(cj+1)],
                                      ps[:])
            return phi

        def combine_digits(dst, name):
            # dst: [128, JT, 3] AR'd digit planes -> [128, JT] colsums
            t01 = misc.tile([128, JT], F32, name=f"{name}_c1", tag=f"{name}_c1")
            nc.vector.scalar_tensor_tensor(t01[:], dst[:, :, 2], 16.0,
                                           dst[:, :, 1], ALU.mult, ALU.add)
            raw = misc.tile([128, JT], F32, name=f"{name}_c0", tag=f"{name}_c0")
            nc.vector.scalar_tensor_tensor(raw[:], t01[:], 16.0,
                                           dst[:, :, 0], ALU.mult, ALU.add)
            return raw

        def drains(gp_of, t, srcs, thn, r, accs_tag, lname, thp=None):
            # h = Sign(A - mean) in {-1,+1} fp8, on the Activation engine
            # (margins are >= 1/16384 with |A| << 1024, so the fp32 subtract
            # never rounds to exactly 0 and Sign never emits 0).
            # Odd j-tiles drain on DVE as (h+1)/2 in {0,1} via is_ge: the
            # per-tile affine encoding cancels in every downstream
            # mean-compare (and in the final BatchNorm).
            on_dve = thp is not None and (t % 2 == 1)
            accs = misc.tile([128, nch], F32, name=f"acc_{lname}_{t}",
                             tag=accs_tag, bufs=4) if r is not None else None
            for c in range(nch):
                sl = gp_of(t, c)
                if on_dve:
                    if r is not None:
                        nc.vector.tensor_scalar(sl, srcs[c], thp[:, t:t+1],
                                                None, ALU.is_ge, ALU.add,
                                                accum_out=accs[:, c:c+1])
                    else:
                        nc.vector.tensor_scalar(sl, srcs[c], thp[:, t:t+1],
                                                None, ALU.is_ge)
                elif r is not None:
                    nc.scalar.activation(sl, srcs[c], AF.Sign,
                                         bias=thn[:, t:t+1],
                                         accum_out=accs[:, c:c+1])
                else:
                    nc.scalar.activation(sl, srcs[c], AF.Sign,
                                         bias=thn[:, t:t+1])
            if r is not None:
                nc.vector.tensor_reduce(r[:, t:t+1], accs[:], axis=AX.X,
                                        op=ALU.add)

        def alloc_g_pairs(pool, tag, lname):
            return [pool.tile([128, 2, n_loc], FP8, name=f"g_{lname}_{m}",
                              tag=tag) for m in range(NP)]

        def bridge_tile(t, pss, lname, bridge_pool, bridge_bufs,
                        bdt=F32):
            # copy psums to SBUF right away (no theta dep) so the banks
            # free up while the AllReduce for theta is still in flight.
            # Even tiles copy on ACT, odd on DVE, matching the engine that
            # will drain them -- and these copies are emitted BEFORE any
            # theta-dependent op so the in-order queues never head-of-line
            # block on the collective.
            srcs = []
            for c in range(nch):
                tb = bridge_pool.tile([128, CHUNK], bdt,
                                      name=f"br_{lname}_{t}_{c}",
                                      tag="bridge", bufs=bridge_bufs)
                if t % 2 == 0:
                    nc.scalar.activation(tb[:], pss[c], AF.Identity)
                else:
                    nc.vector.tensor_copy(tb[:], pss[c])
                srcs.append(tb)
            return srcs

        def layer_dr(sig_pairs, gin_pairs, theta_cb, gout_pairs, want_r,
                     lname, bridge_pool=None, bridge_t=None,
                     bridge_bufs=None):
            if bridge_t is None:
                bridge_t = BRIDGE_T
            if bridge_bufs is None:
                bridge_bufs = BRIDGE_BUFS
            # DoubleRow fp8 layer: A = sigma @ (prev g), drained via ACT Sign
            r = misc.tile([128, JT], F32, name=f"r_{lname}", tag=f"r_{lname}") \
                if want_r else None
            gp_of = lambda tt, cc: gout_pairs[tt//2][:, tt % 2,
                                                     CHUNK*cc:CHUNK*(cc+1)]
            pend = []
            theta = thp = None
            for t in range(JT):
                pss = [ps_main.tile([128, CHUNK], F32,
                                    name=f"ps_{lname}_{t}_{c}", tag="ps_main")
                       for c in range(nch)]
                for m in range(NP):
                    lhs = sig_pairs[m][:, :, 128*t:128*(t+1)]
                    for c in range(nch):
                        nc.tensor.matmul(pss[c], lhs,
                                         gin_pairs[m][:, :, CHUNK*c:CHUNK*(c+1)],
                                         start=(m == 0), stop=(m == NP - 1),
                                         perf_mode=DR)
                if t < bridge_t:
                    pend.append((t, bridge_tile(t, pss, lname, bridge_pool,
                                                bridge_bufs, bdt=FP16)))
                    continue
                if theta is None:
                    # theta algebra lands in the queues only now, after all
                    # bridge copies, then the deferred drains
                    theta, thp = theta_cb()
                    for (tt, ss) in pend:
                        drains(gp_of, tt, ss, theta, r, "accs", lname,
                               thp=thp)
                    pend = []
                drains(gp_of, t, pss, theta, r, "accs", lname, thp=thp)
            return r

        # ---------------- layer 1: x load (chunked), sums, weight DMAs ------
        pl1_cm = tc.tile_pool(name="pl1", bufs=1)
        pl1 = pl1_cm.__enter__()

        # startup-critical DMA order: sg1 k-tile 0 first so the PE can begin
        # the first j-tile as soon as x k-tile 0 lands; x tiles next (they
        # pace the psum accumulation); bulk sigma tiles after.
        # Layer-1 K-pass kt: 2w -> (s1[w], xh[w]); 2w+1 -> (s1[w], xl[w]);
        # 12 -> (sgm, xm) mixed hi/lo tail tile.  The interleaved hi/lo
        # order keeps the psum accumulation close to the reference's
        # k-major fp32 summation order.
        s1_tiles = []
        xs_loc = misc.tile([128, KT1], F32, name="xs_loc", tag="xs_loc")
        x_tiles = []
        for w in range(KF):
            sgt = p_sA.tile([128, H], FP8, name=f"s1_{w}", tag="s1",
                            bufs=NW1)
            dma_in(sgt[:], sg1_t[w])
            s1_tiles.append(sgt)
            hi = pl1.tile([128, n_loc], FP16, name=f"xt0_{w}", tag="xhl",
                          bufs=KT1)
            dma_in(hi[:], xh_t[w])
            nc.vector.tensor_reduce(xs_loc[:, 2*w:2*w+1], hi[:], axis=AX.X,
                                    op=ALU.add)
            x_tiles.append(hi)
            lo = pl1.tile([128, n_loc], FP16, name=f"xt1_{w}", tag="xhl",
                          bufs=KT1)
            dma_in(lo[:], xl_t[w])
            nc.vector.tensor_reduce(xs_loc[:, 2*w+1:2*w+2], lo[:],
                                    axis=AX.X, op=ALU.add)
            x_tiles.append(lo)
        sgt = p_sA.tile([128, H], FP8, name="s1_m", tag="s1", bufs=NW1)
        dma_in(sgt[:], sgm[:])
        s1_tiles.append(sgt)          # s1_tiles[KF] = mixed weight plane
        xmt = pl1.tile([128, n_loc], FP16, name="xt_m", tag="xhl", bufs=KT1)
        dma_in(xmt[:], xm[:])
        nc.vector.tensor_reduce(xs_loc[:, 2*KF:2*KF+1], xmt[:], axis=AX.X,
                                op=ALU.add)
        x_tiles.append(xmt)

        if DMA_ONLY:
            # land every input, then bail out with a token output write
            for m in range(NP):
                sg = p_sA.tile([128, 2, H], FP8, name=f"s2_{m}", tag="s2",
                               bufs=NP)
                dma_in(sg[:], sg2_t[m])
            for m in range(NP):
                sg = p_sA.tile([128, 2, H], FP8, name=f"s3_{m}", tag="s3",
                               bufs=NP)
                dma_in(sg[:], sg3_t[m])
            for m in range(NP):
                sg = p_s4.tile([128, 2, 16], FP8, name=f"s4_{m}", tag="s4",
                               bufs=NP)
                dma_in(sg[:], sg4_t[m])
            w4sb = p_l4c.tile([C, H], F32, name="w4sb", tag="w4sb")
            dma_in(w4sb[:], w4n[:])
            nc.sync.dma_start(yout[0:10, 0:13], xs_loc[0:10, :])
            pl1_cm.__exit__(None, None, None)
            return

        def w1_of(kt):
            return s1_tiles[kt // 2] if kt < 2*KF else s1_tiles[KF]

        # per-weight-plane x rowsums: the hi and lo k-tiles of plane w
        # share sign weights, so their rowsums sum before the theta matvec
        xsv = misc.tile([128, NW1], F32, name="xsv", tag="xsv")
        xsi = xs_loc[:, 0:2*KF].rearrange("p (a b) -> p a b", b=2)
        nc.vector.tensor_add(xsv[:, 0:KF], xsi[:, :, 0], xsi[:, :, 1])
        nc.vector.tensor_copy(xsv[:, KF:NW1], xs_loc[:, 2*KF:2*KF+1])

        # exact fixed-point digitization: 8 balanced base-16 fp8 digits of
        # xsv * 2^17 (hierarchical RN extraction; every step exact in fp32,
        # only the final fractional drop rounds: <= 2^-18 per value).  The
        # theta matvec psums then accumulate exact small integers, so the
        # threshold has no accumulation-order rounding dice at all.
        xs2 = misc.tile([128, NW1, 8], FP8, name="xs2", tag="xs2")
        rk = misc.tile([128, NW1], F32, name="th1_rk", tag="th1_rk")
        nc.vector.tensor_scalar_mul(rk[:], xsv[:], float(2.0 ** -11))
        for d in range(7, -1, -1):
            tm = misc.tile([128, NW1], F32, name=f"th1_t{d}", tag="th1_tm",
                           bufs=2)
            nc.vector.tensor_scalar(tm[:], rk[:], MAGIC, None, ALU.add)
            dg = misc.tile([128, NW1], F32, name=f"th1_d{d}", tag="th1_dg",
                           bufs=2)
            nc.vector.tensor_scalar(dg[:], tm[:], MAGIC, None, ALU.subtract)
            nc.vector.tensor_copy(xs2[:, :, d], dg[:])
            if d > 0:
                rem = misc.tile([128, NW1], F32, name=f"th1_r{d}",
                                tag="th1_rm", bufs=2)
                nc.vector.scalar_tensor_tensor(rem[:], dg[:], -1.0, rk[:],
                                               ALU.mult, ALU.add)
                rk = misc.tile([128, NW1], F32, name=f"th1_k{d}",
                               tag="th1_rk2", bufs=2)
                nc.vector.tensor_scalar_mul(rk[:], rem[:], 16.0)

        def zero_theta(name):
            z = misc.tile([128, JT], F32, name=name, tag=name)
            nc.vector.tensor_scalar_mul(z[:, 0:KT1], xs_loc[:], 0.0)
            nc.vector.tensor_scalar_mul(z[:, KT1:JT], xs_loc[:, 0:JT-KT1], 0.0)
            return z

        # local transposed matvec for theta1 over the 7 weight planes
        phi1 = phi_tile("phi1")
        for cj in range(NCHNK):
            ps = ps_small.tile([8, CHUNK], F32, name=f"mvT_th1_{cj}",
                               tag="ps_main", bufs=8)
            for w in range(NW1):
                nc.tensor.matmul(ps[:], xs2[:, w, :],
                                 s1_tiles[w][:, CHUNK*cj:CHUNK*(cj+1)],
                                 start=(w == 0), stop=(w == NW1 - 1))
            nc.vector.tensor_copy(phi1[0:8, CHUNK*cj:CHUNK*(cj+1)], ps[:])
        if NO_THETA:
            def th1_cb():
                z1 = zero_theta("zth1")
                return theta_scales(z1[:], "th1")
        else:
            D1 = allreduce_rows(phi1[0:8, :], 8, "phi1")

            def th1_cb():
                # theta = sum_d D1[..d] * 16^d * 2^-17 / B, smallest digit
                # first so the rounding stays at ~2 ulp of the result
                acc = misc.tile([128, JT], F32, name="th1_a0", tag="th1_acc",
                                bufs=2)
                nc.vector.tensor_scalar_mul(acc[:], D1[:, :, 0],
                                            float(2.0 ** -17) * inv_B)
                for d in range(1, 8):
                    nxt = misc.tile([128, JT], F32, name=f"th1_a{d}",
                                    tag="th1_acc", bufs=2)
                    nc.vector.scalar_tensor_tensor(
                        nxt[:], D1[:, :, d], float(2.0 ** (4*d - 17)) * inv_B,
                        acc[:], ALU.mult, ALU.add)
                    acc = nxt
                thn = misc.tile([128, JT], F32, name="th1_n", tag="th1_n")
                nc.vector.tensor_scalar_mul(thn[:], acc[:], -1.0)
                return thn, acc

        # sigma2 pair tiles: direct DMA of host-signed fp8 planes
        s2_pairs = []
        for m in range(NP):
            sg = p_sA.tile([128, 2, H], FP8, name=f"s2_{m}", tag="s2", bufs=NP)
            dma_in(sg[:], sg2_t[m])
            s2_pairs.append(sg)

        # layer-4 statics (tiny): land them during layer 1 so the DMA queue
        # is empty around every AllReduce
        s4_pairs = []
        for m in range(NP):
            sg = p_s4.tile([128, 2, 16], FP8, name=f"s4_{m}", tag="s4", bufs=NP)
            dma_in(sg[:], sg4_t[m])
            s4_pairs.append(sg)
        w4sb = p_l4c.tile([C, H], F32, name="w4sb", tag="w4sb")
        dma_in(w4sb[:], w4n[:])
        g4sb = misc.tile([C, 1], F32, name="g4sb", tag="g4sb")
        dma_in(g4sb[:], g4v[:])
        b4sb = misc.tile([C, 1], F32, name="b4sb", tag="b4sb")
        dma_in(b4sb[:], b4v[:])

        s4raw = misc.tile([C, 1], F32, name="s4raw", tag="s4raw")
        nc.vector.tensor_reduce(s4raw[:], w4sb[:], axis=AX.X, op=ALU.add,
                                apply_absolute_value=True)
        s4 = misc.tile([C, 1], F32, name="s4", tag="s4sc")
        nc.vector.tensor_scalar_mul(s4[:], s4raw[:], inv_H)     # mean|w4|
        ns4sq = misc.tile([C, 1], F32, name="ns4sq", tag="ns4sq")
        nc.vector.tensor_scalar(ns4sq[:], s4[:], s4[:], -1.0,
                                ALU.mult, ALU.mult)              # -s4^2

        # ---------------- layer 1 main (13-pass 2-term fp16 x fp8 sign) -----
        g1_pairs = alloc_g_pairs(p_gA, "gA", "l1")
        r1 = misc.tile([128, JT], F32, name="r_l1", tag="r_l1")
        gp1 = lambda tt, cc: g1_pairs[tt//2][:, tt % 2,
                                             CHUNK*cc:CHUNK*(cc+1)]
        pend1 = []
        theta1 = thp1 = None
        for t in range(JT):
            pss = [ps_main.tile([128, CHUNK], F32, name=f"ps_l1_{t}_{c}",
                                tag="ps_main") for c in range(nch)]
            for kt in range(KT1):
                lhs = w1_of(kt)[:, 128*t:128*(t+1)]
                for c in range(nch):
                    nc.tensor.matmul(pss[c], lhs,
                                     x_tiles[kt][:, CHUNK*c:CHUNK*(c+1)],
                                     start=(kt == 0), stop=(kt == KT1 - 1))
            if t < BRIDGE_T:
                pend1.append((t, bridge_tile(t, pss, "l1", pl1,
                                             BRIDGE_BUFS)))
            else:
                if theta1 is None:
                    theta1, thp1 = th1_cb()
                    for (tt, ss) in pend1:
                        drains(gp1, tt, ss, theta1, r1, "accs", "l1",
                               thp=thp1)
                    pend1 = []
                drains(gp1, t, pss, theta1, r1, "accs", "l1", thp=thp1)
            if t == 3:
                # sigma3 pair tiles: DMA mid-layer-1 (queue is idle then;
                # landing them early keeps the phi AllReduce DMAs and the
                # ACT drain stream unblocked at the layer boundaries)
                s3_pairs = []
                for m in range(NP):
                    sg = p_sA.tile([128, 2, H], FP8, name=f"s3_{m}",
                                   tag="s3", bufs=NP)
                    dma_in(sg[:], sg3_t[m])
                    s3_pairs.append(sg)

        for _e in range(N_EXTRA_L1):
            for t in range(JT):
                pss = [ps_main.tile([128, CHUNK], F32,
                                    name=f"px1_{_e}_{t}_{c}", tag="ps_main")
                       for c in range(nch)]
                for kt in range(KT1):
                    lhs = w1_of(kt)[:, 128*t:128*(t+1)]
                    for c in range(nch):
                        nc.tensor.matmul(pss[c], lhs,
                                         x_tiles[kt][:, CHUNK*c:CHUNK*(c+1)],
                                         start=(kt == 0),
                                         stop=(kt == KT1 - 1))

        pl1_cm.__exit__(None, None, None)

        p_gB = es.enter_context(tc.tile_pool(name="p_gB", bufs=8))
        brB_cm = tc.tile_pool(name="brB", bufs=1)
        brB = brB_cm.__enter__()

        if NO_THETA:
            th2_cb = lambda: theta_scales(zero_theta("zth2")[:], "th2")
        else:
            rd1 = digitize_r(r1, "rd1")
            phi2 = matvec_T_dr(rd1, s2_pairs, "th2")
            D2 = allreduce_rows(phi2[0:3, :], 3, "phi2")
            th2_cb = lambda: theta_scales(combine_digits(D2, "th2")[:],
                                          "th2")

        g2_pairs = alloc_g_pairs(p_gB, "gB", "l2")
        r2 = layer_dr(s2_pairs, g1_pairs, th2_cb, g2_pairs, True, "l2",
                      bridge_pool=brB, bridge_t=5, bridge_bufs=20)

        if NO_THETA:
            th3_cb = lambda: theta_scales(zero_theta("zth3")[:], "th3")
        else:
            rd2 = digitize_r(r2, "rd2")
            phi3 = matvec_T_dr(rd2, s3_pairs, "th3")
            D3 = allreduce_rows(phi3[0:3, :], 3, "phi3")
            th3_cb = lambda: theta_scales(combine_digits(D3, "th3")[:],
                                          "th3")

        g3_pairs = alloc_g_pairs(p_gA, "gA", "l3")
        th3_memo = []

        def th3_once():
            if not th3_memo:
                th3_memo.append(th3_cb())
            return th3_memo[0]

        layer_dr(s3_pairs, g2_pairs, th3_once, g3_pairs, False, "l3",
                 bridge_pool=brB, bridge_t=5, bridge_bufs=20)

        if N_EXTRA:
            # dummy layers recycle dead pools: s2 (dead after L2) and
            # gB/g2 (dead once dummy0 replaces g3 as the consumer)
            assert N_EXTRA <= 2
            gin = g3_pairs
            for e in range(N_EXTRA):
                if MM_ONLY_EXTRA:
                    for t in range(JT):
                        pss = [ps_main.tile([128, CHUNK], F32,
                                            name=f"px_{e}_{t}_{c}",
                                            tag="ps_main")
                               for c in range(nch)]
                        for m in range(NP):
                            lhs = s3_pairs[m][:, :, 128*t:128*(t+1)]
                            for c in range(nch):
                                nc.tensor.matmul(
                                    pss[c], lhs,
                                    gin[m][:, :, CHUNK*c:CHUNK*(c+1)],
                                    start=(m == 0), stop=(m == NP - 1),
                                    perf_mode=DR)
                    continue
                pool, tag = (p_sA, "s2") if e == 0 else (p_gB, "gB")
                gx = [pool.tile([128, 2, n_loc], FP8, name=f"g_x{e}_{m}",
                                tag=tag, bufs=NP) for m in range(NP)]
                layer_dr(s3_pairs, gin, th3_once, gx, False, f"lx{e}")
                gin = gx

        brB_cm.__exit__(None, None, None)
        p_l4 = es.enter_context(tc.tile_pool(name="p_l4", bufs=1))

        # ---------------- layer 4 + BatchNorm ----------------
        # k4 = sign(W4) @ h3 (exact small integers); BN statistics of
        # y4 = s4*k4 are folded into per-channel affine coefficients so the
        # only full-width op after the AllReduce is one tensor_scalar.
        k4 = p_l4.tile([C, n_loc], F32, name="k4", tag="k4")
        ksum = misc.tile([C, nch], F32, name="ksum", tag="ksum")
        ksq = misc.tile([C, nch], F32, name="ksq", tag="ksq")
        for c in range(nch):
            ps = ps_small.tile([16, CHUNK], F32, name=f"ps_l4_{c}", tag="ps_main", bufs=8)
            for m in range(NP):
                nc.tensor.matmul(ps[:], s4_pairs[m][:],
                                 g3_pairs[m][:, :, CHUNK*c:CHUNK*(c+1)],
                                 start=(m == 0), stop=(m == NP - 1),
                                 perf_mode=DR)
            nc.vector.tensor_scalar(k4[:, CHUNK*c:CHUNK*(c+1)], ps[0:C, :],
                                    1.0, None, ALU.mult, ALU.add,
                                    accum_out=ksum[:, c:c+1])
            sqsc = p_l4.tile([C, CHUNK], F32, name=f"sq_{c}", tag="sqsc",
                             bufs=2)
            nc.scalar.activation(sqsc[:], ps[0:C, :], AF.Square,
                                 accum_out=ksq[:, c:c+1])

        # pre-scale the stats by 1/B before the AllReduce: the reduced
        # result is then directly (mu_k, E[k^2])
        p4 = misc.tile([C, 2], F32, name="p4", tag="p4")
        nc.vector.tensor_reduce(p4[:, 0:1], ksum[:], axis=AX.X, op=ALU.add)
        nc.vector.tensor_reduce(p4[:, 1:2], ksq[:], axis=AX.X, op=ALU.add)
        p4s = misc.tile([C, 2], F32, name="p4s", tag="p4s")
        nc.vector.tensor_scalar_mul(p4s[:], p4[:], inv_B)

        G4 = allreduce(p4s[:], [C, 2], "p4")
        # -var = mu^2 - E[k^2];  veps = (-var)*(-s4^2) + eps
        nvar = misc.tile([C, 1], F32, name="nvar", tag="nvar")
        nc.vector.tensor_scalar(nvar[:], G4[:, 0:1], G4[:, 0:1], G4[:, 1:2],
                                ALU.mult, ALU.subtract)
        veps = misc.tile([C, 1], F32, name="veps", tag="veps")
        nc.vector.tensor_scalar(veps[:], nvar[:], ns4sq[:], 1e-5,
                                ALU.mult, ALU.add)
        sd = misc.tile([C, 1], F32, name="sd", tag="sd")
        nc.scalar.activation(sd[:], veps[:], AF.Sqrt)
        inv_sd = misc.tile([C, 1], F32, name="inv_sd", tag="inv_sd")
        nc.vector.reciprocal(inv_sd[:], sd[:])
        # A = g*s4/sd(y);  B0 = b - mu_k*A
        A4 = misc.tile([C, 1], F32, name="A4", tag="A4")
        nc.vector.tensor_scalar(A4[:], inv_sd[:], g4sb[:], s4[:],
                                ALU.mult, ALU.mult)
        muA = misc.tile([C, 1], F32, name="muA", tag="muA")
        nc.vector.tensor_mul(muA[:], G4[:, 0:1], A4[:])
        B4 = misc.tile([C, 1], F32, name="B4", tag="B4")
        nc.vector.tensor_sub(B4[:], b4sb[:], muA[:])

        # final affine: split halves across DVE and ACT, DMA out per half
        yo = p_l4.tile([C, n_loc], F32, name="yo", tag="yo")
        hn = n_loc // 2
        nc.vector.tensor_scalar(yo[:, 0:hn], k4[:, 0:hn], A4[:], B4[:],
                                ALU.mult, ALU.add)
        nc.sync.dma_start(yout[:, 0:hn], yo[:, 0:hn])
        nc.scalar.activation(yo[:, hn:], k4[:, hn:], AF.Identity,
                             bias=B4[:], scale=A4[:])
        nc.sync.dma_start(yout[:, hn:], yo[:, hn:])


# --------------------------------------------------------------------------
def prep_inputs(x, w1, w2, w3, w4, g4, b4, n_loc):
    n_cores = N_CORES
    B = x.shape[0]
    assert B == n_loc * n_cores

    f16 = np.float16

    def signs(w):
        return np.where(np.asarray(w) >= 0, np.float32(1.0),
                        np.float32(-1.0))

    # sg1: [768, H], row k = sign(w1[:, k]); sgm: [128, H] mixed tail plane
    # (rows 0-15 = sign rows 768-783, rows 16-31 the same, rest zero)
    s1 = signs(w1).T.astype(NP8)          # [784, 2048]
    sg1 = np.ascontiguousarray(s1[:KF*128])
    sgm = np.zeros((128, H), dtype=NP8)
    sgm[0:D_IN - KF*128] = s1[KF*128:]
    sgm[16:16 + D_IN - KF*128] = s1[KF*128:]

    def paired(w, width):
        # [H, width]: rows (m, k, p) -> sign(w[j, 128*(2m+p)+k]), j < width.
        # Odd input-tile planes (p=1) are scaled x2: those tiles' activations
        # are stored as (h+1)/2 in {0,1} by the DVE drain path.
        s = signs(w)                       # [out, H]
        out = np.zeros((NP, 128, 2, width), dtype=NP8)
        for m in range(NP):
            for p in (0, 1):
                blk = s[:, 128*(2*m+p):128*(2*m+p+1)] * (1.0 + p)
                out[m, :, p, :blk.shape[0]] = blk.T.astype(NP8)
        return out.reshape(H, width)

    sg2 = paired(w2, H)
    sg3 = paired(w3, H)
    sg4 = paired(w4, 16)
    w4n = np.ascontiguousarray(np.asarray(w4).astype(np.float32))
    g4v = np.ascontiguousarray(np.asarray(g4).reshape(C, 1).astype(np.float32))
    b4v = np.ascontiguousarray(np.asarray(b4).reshape(C, 1).astype(np.float32))

    in_maps = []
    ntail = D_IN - KF*128                  # 16 tail rows
    for cidx in range(n_cores):
        xs = x[n_loc*cidx:n_loc*(cidx+1)]
        xT = np.ascontiguousarray(xs.T.astype(np.float32))   # [784, n_loc]
        xhp = xT.astype(f16)
        xlp = (xT - xhp.astype(np.float32)).astype(f16)
        xmv = np.zeros((128, n_loc), dtype=f16)
        xmv[0:ntail] = xhp[KF*128:]
        xmv[16:16 + ntail] = xlp[KF*128:]
        m = {
            "xh": np.ascontiguousarray(xhp[:KF*128]),
            "xl": np.ascontiguousarray(xlp[:KF*128]),
            "xm": xmv,
            "sg1": sg1, "sgm": sgm, "sg2": sg2, "sg3": sg3,
            "sg4": sg4, "w4n": w4n, "g4v": g4v, "b4v": b4v,
        }
        in_maps.append(m)
    return in_maps


_NC_CACHE = {}


def kernel(x, w1, w2, w3, w4, g1, b1, g2, b2, g3, b3, g4, b4):
    x = np.asarray(x); w1 = np.asarray(w1); w2 = np.asarray(w2)
    w3 = np.asarray(w3); w4 = np.asarray(w4)
    g4 = np.asarray(g4); b4 = np.asarray(b4)
    # layers 1-3 BN params: scales cancel inside sign() only when gamma>0, beta=0
    for g in (g1, g2, g3):
        assert np.all(np.asarray(g) > 0), "kernel assumes gamma > 0 for hidden BNs"
    for b in (b1, b2, b3):
        assert np.all(np.asarray(b) == 0), "kernel assumes beta == 0 for hidden BNs"
    for w in (w1, w2, w3, w4):
        assert not np.any(w == 0.0), "exact-zero weight would break Sign()"

    n_loc = x.shape[0] // N_CORES
    if n_loc not in _NC_CACHE:
        _NC_CACHE[n_loc] = build(n_loc)
    nc = _NC_CACHE[n_loc]

    in_maps = prep_inputs(x, w1, w2, w3, w4, g4, b4, n_loc)
    res = bass_utils.run_bass_kernel_spmd(nc, in_maps,
                                          core_ids=list(range(N_CORES)))
    out = np.concatenate([res.results[c]["yout"].T for c in range(N_CORES)],
                         axis=0)
    return out.astype(np.float32)


# revision 38
# speedup vs baseline: 1.0260x; 1.0035x over previous
"""BinaryNet MLP forward on 8 TRN2 NeuronCores.

Strategy: data-parallel over batch (2048 rows/core), feature-major on-chip
layout (activations stored [channel, batch]).  For layers 1-3 the positive
per-row weight scales and the BatchNorm variance cancel inside sign(), so
each layer reduces to:  g_l = 1{ A_l >= mean_batch(A_l) }  where
A_l = sign(W_l) @ h_{l-1} is an exact small integer computed with fp8 {+-1}
activations x fp8 {+-1} weights on the TensorEngine.  h_l is produced by
the Activation engine as Sign(A - mean) (integer margins >= 1/16384 make
the fp32 subtract sign-safe).  Layers 2-4 run fp8 DoubleRow (two
K-subtiles per pass).  Layer 1 (continuous x) uses a 2-term fp16 split of
x that reproduces the reference's fp32 sign decisions; the hi and lo
tails of the 7th (7/8-padding) k-tile share one mixed k-tile, so layer 1
runs 13 K-passes instead of 14 while the six full-width sign-weight tiles
stay shared between the hi and lo passes.  Layer 4 applies the real
BatchNorm with weight scales.

Batch means: colsum(A_l) = sigma_l @ rowsum(h_{l-1}) distributes over the
AllReduce, so each core computes a LOCAL transposed matvec
phi = rowsum_local^T @ sigma  (rowsum stationary: LDWEIGHTS ~free; sigma
planes stream as the moving operand, DoubleRow pairs for layers 2/3) and
the AllReduce carries the phi rows instead of the rowsums -- similar
payload, but the LDWEIGHTS-bound PE matvec of the old formulation
disappears and the matvec no longer sits between the AllReduce and the
drains.  Theta is read back from the AR buffer with transposing DMAs
("r (t p) -> p t") and the phi rows are combined after, in
partition-aligned DVE ops.  For layers 2/3 the local rowsums are exact
integers |r| <= 2048, shipped to the PE as 3 balanced base-16 fp8
digits, so the threshold psums accumulate exact small integers; layer
1's theta uses an 8-digit base-16 fixed-point split of the local x
rowsums (2^-17 resolution, hierarchically extracted with exact fp32
steps) for the same reason -- the sign margins bottom out at ~1e-6 and
ANY accumulation-ordering dice in theta flips h1 entries, which the
binary net amplifies ~36x per layer (10 flips => 6% final error).

Engine-queue discipline at the AllReduce boundaries: the ACT/DVE queues
are strict in-order FIFOs, so theta-dependent ops must not be enqueued
ahead of ready work.  Each layer emits: matmuls for the first BRIDGE_T
j-tiles with psum->SBUF bridge copies (fp16 for the integer DR layers,
f32 for layer 1; even tiles copy on ACT, odd on DVE), THEN the theta
combine/scale algebra, THEN the deferred drains -- so the bridges free
psum banks while the collective is still in flight and the PE never
head-of-line blocks (worth ~30us over the naive order).  PSUM runs with
7 main accumulation banks + 1 for the theta matvecs/layer 4.

All sign-weights ship pre-signed from the host as fp8 {+-1} planes
(pre-paired for DoubleRow), so the ACT engine never runs sign-prep and
the DMA queue never serializes weight staging against the AllReduce path.

Measured (8-core axon TRN2, slope-of-n_rep method): ~477-490us vs the
538.8us/519.5us baseline; rel err 3.09e-07, zero flipped rows.
"""
import sys, os
sys.path.insert(0, '/opt/trn_rl_repo')
import numpy as np
import ml_dtypes

import concourse.bass as bass
import concourse.bacc as bacc
import concourse.tile as tile
import concourse.mybir as mybir
from concourse import bass_utils

F32 = mybir.dt.float32
BF16 = mybir.dt.bfloat16
FP16 = mybir.dt.float16
FP8 = mybir.dt.float8e4
AF = mybir.ActivationFunctionType
ALU = mybir.AluOpType
AX = mybir.AxisListType
DR = mybir.MatmulPerfMode.DoubleRow

NP8 = mybir.dt.np(mybir.dt.float8e4)

N_CORES = 8
D_IN, H, C = 784, 2048, 10
KF = 6                 # full 128-row k-tiles of x (768 rows)
KT1 = 2 * KF + 1       # 13 layer-1 K-passes (6 hi + 6 lo + 1 mixed)
NW1 = KF + 1           # 7 distinct layer-1 weight planes
KT = H // 128          # 16 k-tiles for layers 2-4
NP = KT // 2           # 8 DoubleRow k-pairs
JT = H // 128          # 16 output-channel tiles
CHUNK = 512
NCHNK = H // CHUNK     # 4 j-chunks of 512 for the theta matvecs
BRIDGE_BUFS = 12       # SBUF bridge slots for psum->sbuf theta-decoupling
BRIDGE_T = 3           # j-tiles per layer whose psums get bridged
MAGIC = float(3 << 22)  # fp32 round-to-nearest-int magic constant

# Timing-experiment knobs (leave defaults for correct results)
SKIP_DMA_REP = False   # skip input DMAs for rep>0 (garbage data, timing only)
SKIP_AR = False        # replace AllReduces with local DMA copies (wrong
                       # results on 8 cores, timing only)
DMA_ONLY = False       # emit only the input DMAs (timing the DMA floor)
NO_THETA = False       # constant thresholds: no matvec/AR/theta path at all
N_EXTRA = 0            # extra dummy DR layers after L3 (timing calibration)
MM_ONLY_EXTRA = False  # dummy layers emit only the matmuls (no drains)
N_EXTRA_L1 = 0         # extra MM-only replays of the L1 loop (timing)
LDW_OPT = False        # pass --enable-ldw-opt=true to walrus (dedup LDWs)


def _install_ldw_opt_patch():
    from concourse import bass_utils as _bu
    if getattr(_bu, "_ldw_patch", None):
        return
    _orig = _bu.run_command

    def _patched(argv, **kw):
        if LDW_OPT and any("walrus" in str(a) for a in argv[:1]):
            argv = ["--enable-ldw-opt=true" if a == "--enable-ldw-opt=false"
                    else a for a in argv]
        return _orig(argv, **kw)

    _bu.run_command = _patched
    _bu._ldw_patch = True


_install_ldw_opt_patch()


def build(n_loc: int, single: bool = False, n_rep: int = 1):
    """Emit the SPMD program for one core (all 8 run it on their own shard).

    single=True builds a 1-core variant with AllReduces replaced by plain
    copies (for cost-model timeline analysis).  n_rep repeats the whole
    forward pass back-to-back (device-time benchmarking)."""
    nch = n_loc // CHUNK
    assert n_loc % CHUNK == 0
    inv_B = 1.0 / float(n_loc * N_CORES)   # exact: power of two
    inv_H = 1.0 / float(H)

    nc = bacc.Bacc("TRN2", target_bir_lowering=False, debug=False,
                   num_devices=1 if single else N_CORES)
    nc._single_fake_ar = single

    xh = nc.dram_tensor("xh", [KF * 128, n_loc], FP16, kind="ExternalInput")
    xl = nc.dram_tensor("xl", [KF * 128, n_loc], FP16, kind="ExternalInput")
    xm = nc.dram_tensor("xm", [128, n_loc], FP16, kind="ExternalInput")
    sg1 = nc.dram_tensor("sg1", [KF * 128, H], FP8, kind="ExternalInput")
    sgm = nc.dram_tensor("sgm", [128, H], FP8, kind="ExternalInput")
    sg2 = nc.dram_tensor("sg2", [H, H], FP8, kind="ExternalInput")
    sg3 = nc.dram_tensor("sg3", [H, H], FP8, kind="ExternalInput")
    sg4 = nc.dram_tensor("sg4", [H, 16], FP8, kind="ExternalInput")
    w4n = nc.dram_tensor("w4n", [C, H], F32, kind="ExternalInput")
    g4v = nc.dram_tensor("g4v", [C, 1], F32, kind="ExternalInput")
    b4v = nc.dram_tensor("b4v", [C, 1], F32, kind="ExternalInput")
    yout = nc.dram_tensor("yout", [C, n_loc], F32, kind="ExternalOutput")

    xh_t = xh[:].rearrange("(t p) i -> t p i", p=128)
    xl_t = xl[:].rearrange("(t p) i -> t p i", p=128)
    sg1_t = sg1[:].rearrange("(t p) j -> t p j", p=128)
    sg2_t = sg2[:].rearrange("(m k p) j -> m k p j", k=128, p=2)
    sg3_t = sg3[:].rearrange("(m k p) j -> m k p j", k=128, p=2)
    sg4_t = sg4[:].rearrange("(m k p) j -> m k p j", k=128, p=2)

    salt = os.environ.get("BASS_SALT", "")
    if salt:
        nc.dram_tensor(f"salt_{salt}", [1, 4], F32, kind="Internal")
    with tile.TileContext(nc) as tc:
        for _rep in range(n_rep):
            _emit(tc, nc, n_loc, nch, inv_B, inv_H,
                  xh_t, xl_t, xm, sg1_t, sgm, sg2_t, sg3_t, sg4_t,
                  w4n, g4v, b4v, yout, _rep)
    nc.compile()
    return nc


def _emit(tc, nc, n_loc, nch, inv_B, inv_H,
          xh_t, xl_t, xm, sg1_t, sgm, sg2_t, sg3_t, sg4_t,
          w4n, g4v, b4v, yout, rep_idx=0):
    skip_in = SKIP_DMA_REP and rep_idx > 0
    def dma_in(dst, src):
        if not skip_in:
            nc.sync.dma_start(dst, src)
    import contextlib
    es = contextlib.ExitStack()
    with es:
        misc = es.enter_context(tc.tile_pool(name="misc", bufs=1))
        dram = es.enter_context(tc.tile_pool(name="dram", bufs=1, space="DRAM"))
        ps_main = es.enter_context(tc.tile_pool(name="ps_main", bufs=8, space="PSUM"))
        ps_small = ps_main
        # paired activation tiles [128, 2, n_loc]; g1/g3 rotate through p_gA,
        # g2 lives in p_gB (opened after the layer-1 pool closes)
        p_gA = es.enter_context(tc.tile_pool(name="p_gA", bufs=8))
        # sigma pool A: s1 (7x [128,H] fp8) + s2 (8 pairs); s4 has own pool
        p_sA = es.enter_context(tc.tile_pool(name="p_sA", bufs=1))
        p_s4 = es.enter_context(tc.tile_pool(name="p_s4", bufs=1))
        p_l4c = es.enter_context(tc.tile_pool(name="p_l4c", bufs=1))

        def allreduce_rows(phi_src, nrows, name):
            # AllReduce a [nrows, H] f32 block of local colsum rows, then
            # read it back transposed as [128, JT, nrows] in one DMA.
            bi = dram.tile([nrows, H], F32, name=f"{name}_bi", tag=f"{name}_bi")
            nc.sync.dma_start(bi[:], phi_src)
            dst = misc.tile([128, JT, nrows], F32, name=f"{name}_ar",
                            tag=f"{name}_ar")
            if getattr(nc, "_single_fake_ar", False) or SKIP_AR:
                src = bi
            else:
                bo = dram.tile([nrows, H], F32, addr_space="Shared",
                               name=f"{name}_bo", tag=f"{name}_bo")
                nc.gpsimd.collective_compute(
                    "AllReduce", ALU.add,
                    replica_groups=[list(range(N_CORES))],
                    ins=[bi.opt()], outs=[bo.opt()],
                )
                src = bo
            for rrow in range(nrows):
                nc.sync.dma_start(
                    dst[:, :, rrow],
                    src[rrow:rrow+1, :].rearrange("r (t p) -> (r p) t", p=128))
            return dst

        def allreduce(sbuf_src, shape, name):
            # plain AllReduce (layer-4 BN stats)
            bi = dram.tile(shape, F32, name=f"{name}_bi", tag=f"{name}_bi")
            nc.sync.dma_start(bi[:], sbuf_src)
            dst = misc.tile(shape, F32, name=f"{name}_ar", tag=f"{name}_ar")
            if getattr(nc, "_single_fake_ar", False) or SKIP_AR:
                nc.sync.dma_start(dst[:], bi[:])
                return dst
            bo = dram.tile(shape, F32, addr_space="Shared",
                           name=f"{name}_bo", tag=f"{name}_bo")
            nc.gpsimd.collective_compute(
                "AllReduce", ALU.add,
                replica_groups=[list(range(N_CORES))],
                ins=[bi.opt()], outs=[bo.opt()],
            )
            nc.sync.dma_start(dst[:], bo[:])
            return dst

        def theta_scales(raw, name):
            # negated global mean (ACT Sign bias) and positive mean (DVE
            # is_ge threshold) from the combined colsum vector [128, JT]
            thn = misc.tile([128, JT], F32, name=f"{name}_n", tag=f"{name}_n")
            nc.vector.tensor_scalar_mul(thn[:], raw, -inv_B)
            thp = misc.tile([128, JT], F32, name=f"{name}_p", tag=f"{name}_p")
            nc.vector.tensor_scalar_mul(thp[:], raw, inv_B)
            return thn, thp

        def digitize_r(r, name):
            # local rowsums (exact ints, |r|<=2048) -> 3 balanced base-16
            # fp8 digits laid out [128, KT, 16] (digit dim padded to 16 so
            # the DR weight AP step is 16-byte aligned)
            rd = misc.tile([128, KT, 16], FP8, name=f"{name}_d", tag=f"{name}_d")
            t2 = misc.tile([128, KT], F32, name=f"{name}_t2", tag=f"{name}_t2")
            nc.vector.tensor_scalar(t2[:], r[:], 1.0 / 256.0, MAGIC,
                                    ALU.mult, ALU.add)
            q2 = misc.tile([128, KT], F32, name=f"{name}_q2", tag=f"{name}_q2")
            nc.vector.tensor_scalar(q2[:], t2[:], MAGIC, None, ALU.subtract)
            rem = misc.tile([128, KT], F32, name=f"{name}_rm", tag=f"{name}_rm")
            nc.vector.scalar_tensor_tensor(rem[:], q2[:], -256.0, r[:],
                                           ALU.mult, ALU.add)
            t1 = misc.tile([128, KT], F32, name=f"{name}_t1", tag=f"{name}_t1")
            nc.vector.tensor_scalar(t1[:], rem[:], 1.0 / 16.0, MAGIC,
                                    ALU.mult, ALU.add)
            q1 = misc.tile([128, KT], F32, name=f"{name}_q1", tag=f"{name}_q1")
            nc.vector.tensor_scalar(q1[:], t1[:], MAGIC, None, ALU.subtract)
            q0 = misc.tile([128, KT], F32, name=f"{name}_q0", tag=f"{name}_q0")
            nc.vector.scalar_tensor_tensor(q0[:], q1[:], -16.0, rem[:],
                                           ALU.mult, ALU.add)
            nc.vector.tensor_copy(rd[:, :, 0], q0[:])
            nc.vector.tensor_copy(rd[:, :, 1], q1[:])
            nc.vector.tensor_copy(rd[:, :, 2], q2[:])
            return rd

        def phi_tile(name):
            # one shared [8, H] row-block for all three theta matvecs (each
            # is DMA'd to the AR input buffer long before the next layer's
            # matvec runs, so a single buffer is WAR-safe)
            return misc.tile([8, H], F32, name=name, tag="phi", bufs=1)

        def matvec_T_dr(rd, sig_pairs, name):
            # local transposed matvec, DoubleRow: phi digit rows
            # phi[d, j] = sum_k digit_d(r[k]) * sig[j, k]
            phi = phi_tile(f"{name}_phi")
            for cj in range(NCHNK):
                ps = ps_small.tile([3, CHUNK], F32, name=f"mvT_{name}_{cj}",
                                   tag="ps_main", bufs=8)
                for m in range(NP):
                    nc.tensor.matmul(ps[:], rd[:, 2*m:2*m+2, 0:3],
                                     sig_pairs[m][:, :, CHUNK*cj:CHUNK*(cj+1)],
                                     start=(m == 0), stop=(m == NP - 1),
                                     perf_mode=DR)
                nc.vector.tensor_copy(phi[0:3, CHUNK*cj:CHUNK*(cj+1)],
                                      ps[:])
            return phi

        def combine_digits(dst, name):
            # dst: [128, JT, 3] AR'd digit planes -> [128, JT] colsums
            t01 = misc.tile([128, JT], F32, name=f"{name}_c1", tag=f"{name}_c1")
            nc.vector.scalar_tensor_tensor(t01[:], dst[:, :, 2], 16.0,
                                           dst[:, :, 1], ALU.mult, ALU.add)
            raw = misc.tile([128, JT], F32, name=f"{name}_c0", tag=f"{name}_c0")
            nc.vector.scalar_tensor_tensor(raw[:], t01[:], 16.0,
                                           dst[:, :, 0], ALU.mult, ALU.add)
            return raw

        def drains(gp_of, t, srcs, thn, r, accs_tag, lname, thp=None):
            # h = Sign(A - mean) in {-1,+1} fp8, on the Activation engine
            # (margins are >= 1/16384 with |A| << 1024, so the fp32 subtract
            # never rounds to exactly 0 and Sign never emits 0).
            # Odd j-tiles drain on DVE as (h+1)/2 in {0,1} via is_ge: the
            # per-tile affine encoding cancels in every downstream
            # mean-compare (and in the final BatchNorm).
            on_dve = thp is not None and (t % 2 == 1)
            accs = misc.tile([128, nch], F32, name=f"acc_{lname}_{t}",
                             tag=accs_tag, bufs=4) if r is not None else None
            for c in range(nch):
                sl = gp_of(t, c)
                if on_dve:
                    if r is not None:
                        nc.vector.tensor_scalar(sl, srcs[c], thp[:, t:t+1],
                                                None, ALU.is_ge, ALU.add,
                                                accum_out=accs[:, c:c+1])
                    else:
                        nc.vector.tensor_scalar(sl, srcs[c], thp[:, t:t+1],
                                                None, ALU.is_ge)
                elif r is not None:
                    nc.scalar.activation(sl, srcs[c], AF.Sign,
                                         bias=thn[:, t:t+1],
                                         accum_out=accs[:, c:c+1])
                else:
                    nc.scalar.activation(sl, srcs[c], AF.Sign,
                                         bias=thn[:, t:t+1])
            if r is not None:
                nc.vector.tensor_reduce(r[:, t:t+1], accs[:], axis=AX.X,
                                        op=ALU.add)

        def alloc_g_pairs(pool, tag, lname):
            return [pool.tile([128, 2, n_loc], FP8, name=f"g_{lname}_{m}",
                              tag=tag) for m in range(NP)]

        def bridge_tile(t, pss, lname, bridge_pool, bridge_bufs,
                        bdt=F32):
            # copy psums to SBUF right away (no theta dep) so the banks
            # free up while the AllReduce for theta is still in flight.
            # Even tiles copy on ACT, odd on DVE, matching the engine that
            # will drain them -- and these copies are emitted BEFORE any
            # theta-dependent op so the in-order queues never head-of-line
            # block on the collective.
            srcs = []
            for c in range(nch):
                tb = bridge_pool.tile([128, CHUNK], bdt,
                                      name=f"br_{lname}_{t}_{c}",
                                      tag="bridge", bufs=bridge_bufs)
                if t % 2 == 0:
                    nc.scalar.activation(tb[:], pss[c], AF.Identity)
                else:
                    nc.vector.tensor_copy(tb[:], pss[c])
                srcs.append(tb)
            return srcs

        def layer_dr(sig_pairs, gin_pairs, theta_cb, gout_pairs, want_r,
                     lname, bridge_pool=None, bridge_t=None,
                     bridge_bufs=None):
            if bridge_t is None:
                bridge_t = BRIDGE_T
            if bridge_bufs is None:
                bridge_bufs = BRIDGE_BUFS
            # DoubleRow fp8 layer: A = sigma @ (prev g), drained via ACT Sign
            r = misc.tile([128, JT], F32, name=f"r_{lname}", tag=f"r_{lname}") \
                if want_r else None
            gp_of = lambda tt, cc: gout_pairs[tt//2][:, tt % 2,
                                                     CHUNK*cc:CHUNK*(cc+1)]
            pend = []
            theta = thp = None
            for t in range(JT):
                pss = [ps_main.tile([128, CHUNK], F32,
                                    name=f"ps_{lname}_{t}_{c}", tag="ps_main")
                       for c in range(nch)]
                for m in range(NP):
                    lhs = sig_pairs[m][:, :, 128*t:128*(t+1)]
                    for c in range(nch):
                        nc.tensor.matmul(pss[c], lhs,
                                         gin_pairs[m][:, :, CHUNK*c:CHUNK*(c+1)],
                                         start=(m == 0), stop=(m == NP - 1),
                                         perf_mode=DR)
                if t < bridge_t:
                    pend.append((t, bridge_tile(t, pss, lname, bridge_pool,
                                                bridge_bufs, bdt=FP16)))
                    continue
                if theta is None:
                    # theta algebra lands in the queues only now, after all
                    # bridge copies, then the deferred drains
                    theta, thp = theta_cb()
                    for (tt, ss) in pend:
                        drains(gp_of, tt, ss, theta, r, "accs", lname,
                               thp=thp)
                    pend = []
                drains(gp_of, t, pss, theta, r, "accs", lname, thp=thp)
            return r

        # ---------------- layer 1: x load (chunked), sums, weight DMAs ------
        pl1_cm = tc.tile_pool(name="pl1", bufs=1)
        pl1 = pl1_cm.__enter__()

        # startup-critical DMA order: sg1 k-tile 0 first so the PE can begin
        # the first j-tile as soon as x k-tile 0 lands; x tiles next (they
        # pace the psum accumulation); bulk sigma tiles after.
        # Layer-1 K-pass kt: 2w -> (s1[w], xh[w]); 2w+1 -> (s1[w], xl[w]);
        # 12 -> (sgm, xm) mixed hi/lo tail tile.  The interleaved hi/lo
        # order keeps the psum accumulation close to the reference's
        # k-major fp32 summation order.
        s1_tiles = []
        xs_loc = misc.tile([128, KT1], F32, name="xs_loc", tag="xs_loc")
        x_tiles = []
        for w in range(KF):
            sgt = p_sA.tile([128, H], FP8, name=f"s1_{w}", tag="s1",
                            bufs=NW1)
            dma_in(sgt[:], sg1_t[w])
            s1_tiles.append(sgt)
            hi = pl1.tile([128, n_loc], FP16, name=f"xt0_{w}", tag="xhl",
                          bufs=KT1)
            dma_in(hi[:], xh_t[w])
            nc.vector.tensor_reduce(xs_loc[:, 2*w:2*w+1], hi[:], axis=AX.X,
                                    op=ALU.add)
            x_tiles.append(hi)
            lo = pl1.tile([128, n_loc], FP16, name=f"xt1_{w}", tag="xhl",
                          bufs=KT1)
            dma_in(lo[:], xl_t[w])
            nc.vector.tensor_reduce(xs_loc[:, 2*w+1:2*w+2], lo[:],
                                    axis=AX.X, op=ALU.add)
            x_tiles.append(lo)
        sgt = p_sA.tile([128, H], FP8, name="s1_m", tag="s1", bufs=NW1)
        dma_in(sgt[:], sgm[:])
        s1_tiles.append(sgt)          # s1_tiles[KF] = mixed weight plane
        xmt = pl1.tile([128, n_loc], FP16, name="xt_m", tag="xhl", bufs=KT1)
        dma_in(xmt[:], xm[:])
        nc.vector.tensor_reduce(xs_loc[:, 2*KF:2*KF+1], xmt[:], axis=AX.X,
                                op=ALU.add)
        x_tiles.append(xmt)

        if DMA_ONLY:
            # land every input, then bail out with a token output write
            for m in range(NP):
                sg = p_sA.tile([128, 2, H], FP8, name=f"s2_{m}", tag="s2",
                               bufs=NP)
                dma_in(sg[:], sg2_t[m])
            for m in range(NP):
                sg = p_sA.tile([128, 2, H], FP8, name=f"s3_{m}", tag="s3",
                               bufs=NP)
                dma_in(sg[:], sg3_t[m])
            for m in range(NP):
                sg = p_s4.tile([128, 2, 16], FP8, name=f"s4_{m}", tag="s4",
                               bufs=NP)
                dma_in(sg[:], sg4_t[m])
            w4sb = p_l4c.tile([C, H], F32, name="w4sb", tag="w4sb")
            dma_in(w4sb[:], w4n[:])
            nc.sync.dma_start(yout[0:10, 0:13], xs_loc[0:10, :])
            pl1_cm.__exit__(None, None, None)
            return

        def w1_of(kt):
            return s1_tiles[kt // 2] if kt < 2*KF else s1_tiles[KF]

        # per-weight-plane x rowsums: the hi and lo k-tiles of plane w
        # share sign weights, so their rowsums sum before the theta matvec
        xsv = misc.tile([128, NW1], F32, name="xsv", tag="xsv")
        xsi = xs_loc[:, 0:2*KF].rearrange("p (a b) -> p a b", b=2)
        nc.vector.tensor_add(xsv[:, 0:KF], xsi[:, :, 0], xsi[:, :, 1])
        nc.vector.tensor_copy(xsv[:, KF:NW1], xs_loc[:, 2*KF:2*KF+1])

        # exact fixed-point digitization: 8 balanced base-16 fp8 digits of
        # xsv * 2^17 (hierarchical RN extraction; every step exact in fp32,
        # only the final fractional drop rounds: <= 2^-18 per value).  The
        # theta matvec psums then accumulate exact small integers, so the
        # threshold has no accumulation-order rounding dice at all.
        xs2 = misc.tile([128, NW1, 8], FP8, name="xs2", tag="xs2")
        rk = misc.tile([128, NW1], F32, name="th1_rk", tag="th1_rk")
        nc.vector.tensor_scalar_mul(rk[:], xsv[:], float(2.0 ** -11))
        for d in range(7, -1, -1):
            tm = misc.tile([128, NW1], F32, name=f"th1_t{d}", tag="th1_tm",
                           bufs=2)
            nc.vector.tensor_scalar(tm[:], rk[:], MAGIC, None, ALU.add)
            dg = misc.tile([128, NW1], F32, name=f"th1_d{d}", tag="th1_dg",
                           bufs=2)
            nc.vector.tensor_scalar(dg[:], tm[:], MAGIC, None, ALU.subtract)
            nc.vector.tensor_copy(xs2[:, :, d], dg[:])
            if d > 0:
                rem = misc.tile([128, NW1], F32, name=f"th1_r{d}",
                                tag="th1_rm", bufs=2)
                nc.vector.scalar_tensor_tensor(rem[:], dg[:], -1.0, rk[:],
                                               ALU.mult, ALU.add)
                rk = misc.tile([128, NW1], F32, name=f"th1_k{d}",
                               tag="th1_rk2", bufs=2)
                nc.vector.tensor_scalar_mul(rk[:], rem[:], 16.0)

        def zero_theta(name):
            z = misc.tile([128, JT], F32, name=name, tag=name)
            nc.vector.tensor_scalar_mul(z[:, 0:KT1], xs_loc[:], 0.0)
            nc.vector.tensor_scalar_mul(z[:, KT1:JT], xs_loc[:, 0:JT-KT1], 0.0)
            return z

        # local transposed matvec for theta1 over the 7 weight planes.
        # Emitted from inside the layer-1 main loop (after tile 2's
        # matmuls) so the in-order PE queue does not head-of-line block
        # layer 1's first tiles -- which can pace with the x DMAs --
        # behind a matvec that needs ALL x tiles landed.
        th1_state = {}

        def emit_phi1():
            phi1 = phi_tile("phi1")
            for cj in range(NCHNK):
                ps = ps_small.tile([8, CHUNK], F32, name=f"mvT_th1_{cj}",
                                   tag="ps_main", bufs=8)
                for w in range(NW1):
                    nc.tensor.matmul(ps[:], xs2[:, w, :],
                                     s1_tiles[w][:, CHUNK*cj:CHUNK*(cj+1)],
                                     start=(w == 0), stop=(w == NW1 - 1))
                nc.vector.tensor_copy(phi1[0:8, CHUNK*cj:CHUNK*(cj+1)],
                                      ps[:])
            if not NO_THETA:
                th1_state["D1"] = allreduce_rows(phi1[0:8, :], 8, "phi1")

        if NO_THETA:
            def th1_cb():
                z1 = zero_theta("zth1")
                return theta_scales(z1[:], "th1")
        else:
            def th1_cb():
                D1 = th1_state["D1"]
                # theta = sum_d D1[..d] * 16^d * 2^-17 / B, smallest digit
                # first so the rounding stays at ~2 ulp of the result
                acc = misc.tile([128, JT], F32, name="th1_a0", tag="th1_acc",
                                bufs=2)
                nc.vector.tensor_scalar_mul(acc[:], D1[:, :, 0],
                                            float(2.0 ** -17) * inv_B)
                for d in range(1, 8):
                    nxt = misc.tile([128, JT], F32, name=f"th1_a{d}",
                                    tag="th1_acc", bufs=2)
                    nc.vector.scalar_tensor_tensor(
                        nxt[:], D1[:, :, d], float(2.0 ** (4*d - 17)) * inv_B,
                        acc[:], ALU.mult, ALU.add)
                    acc = nxt
                thn = misc.tile([128, JT], F32, name="th1_n", tag="th1_n")
                nc.vector.tensor_scalar_mul(thn[:], acc[:], -1.0)
                return thn, acc

        # sigma2 pair tiles: direct DMA of host-signed fp8 planes
        s2_pairs = []
        for m in range(NP):
            sg = p_sA.tile([128, 2, H], FP8, name=f"s2_{m}", tag="s2", bufs=NP)
            dma_in(sg[:], sg2_t[m])
            s2_pairs.append(sg)

        # layer-4 statics (tiny): land them during layer 1 so the DMA queue
        # is empty around every AllReduce
        s4_pairs = []
        for m in range(NP):
            sg = p_s4.tile([128, 2, 16], FP8, name=f"s4_{m}", tag="s4", bufs=NP)
            dma_in(sg[:], sg4_t[m])
            s4_pairs.append(sg)
        w4sb = p_l4c.tile([C, H], F32, name="w4sb", tag="w4sb")
        dma_in(w4sb[:], w4n[:])
        g4sb = misc.tile([C, 1], F32, name="g4sb", tag="g4sb")
        dma_in(g4sb[:], g4v[:])
        b4sb = misc.tile([C, 1], F32, name="b4sb", tag="b4sb")
        dma_in(b4sb[:], b4v[:])

        s4raw = misc.tile([C, 1], F32, name="s4raw", tag="s4raw")
        nc.vector.tensor_reduce(s4raw[:], w4sb[:], axis=AX.X, op=ALU.add,
                                apply_absolute_value=True)
        s4 = misc.tile([C, 1], F32, name="s4", tag="s4sc")
        nc.vector.tensor_scalar_mul(s4[:], s4raw[:], inv_H)     # mean|w4|
        ns4sq = misc.tile([C, 1], F32, name="ns4sq", tag="ns4sq")
        nc.vector.tensor_scalar(ns4sq[:], s4[:], s4[:], -1.0,
                                ALU.mult, ALU.mult)              # -s4^2

        # ---------------- layer 1 main (13-pass 2-term fp16 x fp8 sign) -----
        g1_pairs = alloc_g_pairs(p_gA, "gA", "l1")
        r1 = misc.tile([128, JT], F32, name="r_l1", tag="r_l1")
        gp1 = lambda tt, cc: g1_pairs[tt//2][:, tt % 2,
                                             CHUNK*cc:CHUNK*(cc+1)]
        pend1 = []
        theta1 = thp1 = None
        for t in range(JT):
            pss = [ps_main.tile([128, CHUNK], F32, name=f"ps_l1_{t}_{c}",
                                tag="ps_main") for c in range(nch)]
            for kt in range(KT1):
                lhs = w1_of(kt)[:, 128*t:128*(t+1)]
                for c in range(nch):
                    nc.tensor.matmul(pss[c], lhs,
                                     x_tiles[kt][:, CHUNK*c:CHUNK*(c+1)],
                                     start=(kt == 0), stop=(kt == KT1 - 1))
            if t == 2:
                emit_phi1()
            if t < BRIDGE_T:
                pend1.append((t, bridge_tile(t, pss, "l1", pl1,
                                             BRIDGE_BUFS)))
            else:
                if theta1 is None:
                    theta1, thp1 = th1_cb()
                    for (tt, ss) in pend1:
                        drains(gp1, tt, ss, theta1, r1, "accs", "l1",
                               thp=thp1)
                    pend1 = []
                drains(gp1, t, pss, theta1, r1, "accs", "l1", thp=thp1)
            if t == 3:
                # sigma3 pair tiles: DMA mid-layer-1 (queue is idle then;
                # landing them early keeps the phi AllReduce DMAs and the
                # ACT drain stream unblocked at the layer boundaries)
                s3_pairs = []
                for m in range(NP):
                    sg = p_sA.tile([128, 2, H], FP8, name=f"s3_{m}",
                                   tag="s3", bufs=NP)
                    dma_in(sg[:], sg3_t[m])
                    s3_pairs.append(sg)

        for _e in range(N_EXTRA_L1):
            for t in range(JT):
                pss = [ps_main.tile([128, CHUNK], F32,
                                    name=f"px1_{_e}_{t}_{c}", tag="ps_main")
                       for c in range(nch)]
                for kt in range(KT1):
                    lhs = w1_of(kt)[:, 128*t:128*(t+1)]
                    for c in range(nch):
                        nc.tensor.matmul(pss[c], lhs,
                                         x_tiles[kt][:, CHUNK*c:CHUNK*(c+1)],
                                         start=(kt == 0),
                                         stop=(kt == KT1 - 1))

        pl1_cm.__exit__(None, None, None)

        p_gB = es.enter_context(tc.tile_pool(name="p_gB", bufs=8))
        brB_cm = tc.tile_pool(name="brB", bufs=1)
        brB = brB_cm.__enter__()

        if NO_THETA:
            th2_cb = lambda: theta_scales(zero_theta("zth2")[:], "th2")
        else:
            rd1 = digitize_r(r1, "rd1")
            phi2 = matvec_T_dr(rd1, s2_pairs, "th2")
            D2 = allreduce_rows(phi2[0:3, :], 3, "phi2")
            th2_cb = lambda: theta_scales(combine_digits(D2, "th2")[:],
                                          "th2")

        g2_pairs = alloc_g_pairs(p_gB, "gB", "l2")
        r2 = layer_dr(s2_pairs, g1_pairs, th2_cb, g2_pairs, True, "l2",
                      bridge_pool=brB, bridge_t=5, bridge_bufs=20)

        if NO_THETA:
            th3_cb = lambda: theta_scales(zero_theta("zth3")[:], "th3")
        else:
            rd2 = digitize_r(r2, "rd2")
            phi3 = matvec_T_dr(rd2, s3_pairs, "th3")
            D3 = allreduce_rows(phi3[0:3, :], 3, "phi3")
            th3_cb = lambda: theta_scales(combine_digits(D3, "th3")[:],
                                          "th3")

        g3_pairs = alloc_g_pairs(p_gA, "gA", "l3")
        th3_memo = []

        def th3_once():
            if not th3_memo:
                th3_memo.append(th3_cb())
            return th3_memo[0]

        layer_dr(s3_pairs, g2_pairs, th3_once, g3_pairs, False, "l3",
                 bridge_pool=brB, bridge_t=5, bridge_bufs=20)

        if N_EXTRA:
            # dummy layers recycle dead pools: s2 (dead after L2) and
            # gB/g2 (dead once dummy0 replaces g3 as the consumer)
            assert N_EXTRA <= 2
            gin = g3_pairs
            for e in range(N_EXTRA):
                if MM_ONLY_EXTRA:
                    for t in range(JT):
                        pss = [ps_main.tile([128, CHUNK], F32,
                                            name=f"px_{e}_{t}_{c}",
                                            tag="ps_main")
                               for c in range(nch)]
                        for m in range(NP):
                            lhs = s3_pairs[m][:, :, 128*t:128*(t+1)]
                            for c in range(nch):
                                nc.tensor.matmul(
                                    pss[c], lhs,
                                    gin[m][:, :, CHUNK*c:CHUNK*(c+1)],
                                    start=(m == 0), stop=(m == NP - 1),
                                    perf_mode=DR)
                    continue
                pool, tag = (p_sA, "s2") if e == 0 else (p_gB, "gB")
                gx = [pool.tile([128, 2, n_loc], FP8, name=f"g_x{e}_{m}",
                                tag=tag, bufs=NP) for m in range(NP)]
                layer_dr(s3_pairs, gin, th3_once, gx, False, f"lx{e}")
                gin = gx

        brB_cm.__exit__(None, None, None)
        p_l4 = es.enter_context(tc.tile_pool(name="p_l4", bufs=1))

        # ---------------- layer 4 + BatchNorm ----------------
        # k4 = sign(W4) @ h3 (exact small integers); BN statistics of
        # y4 = s4*k4 are folded into per-channel affine coefficients so the
        # only full-width op after the AllReduce is one tensor_scalar.
        k4 = p_l4.tile([C, n_loc], F32, name="k4", tag="k4")
        ksum = misc.tile([C, nch], F32, name="ksum", tag="ksum")
        ksq = misc.tile([C, nch], F32, name="ksq", tag="ksq")
        for c in range(nch):
            ps = ps_small.tile([16, CHUNK], F32, name=f"ps_l4_{c}", tag="ps_main", bufs=8)
            for m in range(NP):
                nc.tensor.matmul(ps[:], s4_pairs[m][:],
                                 g3_pairs[m][:, :, CHUNK*c:CHUNK*(c+1)],
                                 start=(m == 0), stop=(m == NP - 1),
                                 perf_mode=DR)
            nc.vector.tensor_scalar(k4[:, CHUNK*c:CHUNK*(c+1)], ps[0:C, :],
                                    1.0, None, ALU.mult, ALU.add,
                                    accum_out=ksum[:, c:c+1])
            sqsc = p_l4.tile([C, CHUNK], F32, name=f"sq_{c}", tag="sqsc",
                             bufs=2)
            nc.scalar.activation(sqsc[:], ps[0:C, :], AF.Square,
                                 accum_out=ksq[:, c:c+1])

        # pre-scale the stats by 1/B before the AllReduce: the reduced
        # result is then directly (mu_k, E[k^2])
        p4 = misc.tile([C, 2], F32, name="p4", tag="p4")
        nc.vector.tensor_reduce(p4[:, 0:1], ksum[:], axis=AX.X, op=ALU.add)
        nc.vector.tensor_reduce(p4[:, 1:2], ksq[:], axis=AX.X, op=ALU.add)
        p4s = misc.tile([C, 2], F32, name="p4s", tag="p4s")
        nc.vector.tensor_scalar_mul(p4s[:], p4[:], inv_B)

        G4 = allreduce(p4s[:], [C, 2], "p4")
        # -var = mu^2 - E[k^2];  veps = (-var)*(-s4^2) + eps
        nvar = misc.tile([C, 1], F32, name="nvar", tag="nvar")
        nc.vector.tensor_scalar(nvar[:], G4[:, 0:1], G4[:, 0:1], G4[:, 1:2],
                                ALU.mult, ALU.subtract)
        veps = misc.tile([C, 1], F32, name="veps", tag="veps")
        nc.vector.tensor_scalar(veps[:], nvar[:], ns4sq[:], 1e-5,
                                ALU.mult, ALU.add)
        sd = misc.tile([C, 1], F32, name="sd", tag="sd")
        nc.scalar.activation(sd[:], veps[:], AF.Sqrt)
        inv_sd = misc.tile([C, 1], F32, name="inv_sd", tag="inv_sd")
        nc.vector.reciprocal(inv_sd[:], sd[:])
        # A = g*s4/sd(y);  B0 = b - mu_k*A
        A4 = misc.tile([C, 1], F32, name="A4", tag="A4")
        nc.vector.tensor_scalar(A4[:], inv_sd[:], g4sb[:], s4[:],
                                ALU.mult, ALU.mult)
        muA = misc.tile([C, 1], F32, name="muA", tag="muA")
        nc.vector.tensor_mul(muA[:], G4[:, 0:1], A4[:])
        B4 = misc.tile([C, 1], F32, name="B4", tag="B4")
        nc.vector.tensor_sub(B4[:], b4sb[:], muA[:])

        # final affine: split halves across DVE and ACT, DMA out per half
        yo = p_l4.tile([C, n_loc], F32, name="yo", tag="yo")
        hn = n_loc // 2
        nc.vector.tensor_scalar(yo[:, 0:hn], k4[:, 0:hn], A4[:], B4[:],
                                ALU.mult, ALU.add)
        nc.sync.dma_start(yout[:, 0:hn], yo[:, 0:hn])
        nc.scalar.activation(yo[:, hn:], k4[:, hn:], AF.Identity,
                             bias=B4[:], scale=A4[:])
        nc.sync.dma_start(yout[:, hn:], yo[:, hn:])


# --------------------------------------------------------------------------
def prep_inputs(x, w1, w2, w3, w4, g4, b4, n_loc):
    n_cores = N_CORES
    B = x.shape[0]
    assert B == n_loc * n_cores

    f16 = np.float16

    def signs(w):
        return np.where(np.asarray(w) >= 0, np.float32(1.0),
                        np.float32(-1.0))

    # sg1: [768, H], row k = sign(w1[:, k]); sgm: [128, H] mixed tail plane
    # (rows 0-15 = sign rows 768-783, rows 16-31 the same, rest zero)
    s1 = signs(w1).T.astype(NP8)          # [784, 2048]
    sg1 = np.ascontiguousarray(s1[:KF*128])
    sgm = np.zeros((128, H), dtype=NP8)
    sgm[0:D_IN - KF*128] = s1[KF*128:]
    sgm[16:16 + D_IN - KF*128] = s1[KF*128:]

    def paired(w, width):
        # [H, width]: rows (m, k, p) -> sign(w[j, 128*(2m+p)+k]), j < width.
        # Odd input-tile planes (p=1) are scaled x2: those tiles' activations
        # are stored as (h+1)/2 in {0,1} by the DVE drain path.
        s = signs(w)                       # [out, H]
        out = np.zeros((NP, 128, 2, width), dtype=NP8)
        for m in range(NP):
            for p in (0, 1):
                blk = s[:, 128*(2*m+p):128*(2*m+p+1)] * (1.0 + p)
                out[m, :, p, :blk.shape[0]] = blk.T.astype(NP8)
        return out.reshape(H, width)

    sg2 = paired(w2, H)
    sg3 = paired(w3, H)
    sg4 = paired(w4, 16)
    w4n = np.ascontiguousarray(np.asarray(w4).astype(np.float32))
    g4v = np.ascontiguousarray(np.asarray(g4).reshape(C, 1).astype(np.float32))
    b4v = np.ascontiguousarray(np.asarray(b4).reshape(C, 1).astype(np.float32))

    in_maps = []
    ntail = D_IN - KF*128                  # 16 tail rows
    for cidx in range(n_cores):
        xs = x[n_loc*cidx:n_loc*(cidx+1)]
        xT = np.ascontiguousarray(xs.T.astype(np.float32))   # [784, n_loc]
        xhp = xT.astype(f16)
        xlp = (xT - xhp.astype(np.float32)).astype(f16)
        xmv = np.zeros((128, n_loc), dtype=f16)
        xmv[0:ntail] = xhp[KF*128:]
        xmv[16:16 + ntail] = xlp[KF*128:]
        m = {
            "xh": np.ascontiguousarray(xhp[:KF*128]),
            "xl": np.ascontiguousarray(xlp[:KF*128]),
            "xm": xmv,
            "sg1": sg1, "sgm": sgm, "sg2": sg2, "sg3": sg3,
            "sg4": sg4, "w4n": w4n, "g4v": g4v, "b4v": b4v,
        }
        in_maps.append(m)
    return in_maps


_NC_CACHE = {}


def kernel(x, w1, w2, w3, w4, g1, b1, g2, b2, g3, b3, g4, b4):
    x = np.asarray(x); w1 = np.asarray(w1); w2 = np.asarray(w2)
    w3 = np.asarray(w3); w4 = np.asarray(w4)
    g4 = np.asarray(g4); b4 = np.asarray(b4)
    # layers 1-3 BN params: scales cancel inside sign() only when gamma>0, beta=0
    for g in (g1, g2, g3):
        assert np.all(np.asarray(g) > 0), "kernel assumes gamma > 0 for hidden BNs"
    for b in (b1, b2, b3):
        assert np.all(np.asarray(b) == 0), "kernel assumes beta == 0 for hidden BNs"
    for w in (w1, w2, w3, w4):
        assert not np.any(w == 0.0), "exact-zero weight would break Sign()"

    n_loc = x.shape[0] // N_CORES
    if n_loc not in _NC_CACHE:
        _NC_CACHE[n_loc] = build(n_loc)
    nc = _NC_CACHE[n_loc]

    in_maps = prep_inputs(x, w1, w2, w3, w4, g4, b4, n_loc)
    res = bass_utils.run_bass_kernel_spmd(nc, in_maps,
                                          core_ids=list(range(N_CORES)))
    out = np.concatenate([res.results[c]["yout"].T for c in range(N_CORES)],
                         axis=0)
    return out.astype(np.float32)


# revision 39
# speedup vs baseline: 1.0554x; 1.0287x over previous
"""BinaryNet MLP forward on 8 TRN2 NeuronCores.

Strategy: data-parallel over batch (2048 rows/core), feature-major on-chip
layout (activations stored [channel, batch]).  For layers 1-3 the positive
per-row weight scales and the BatchNorm variance cancel inside sign(), so
each layer reduces to:  g_l = 1{ A_l >= mean_batch(A_l) }  where
A_l = sign(W_l) @ h_{l-1} is an exact small integer computed with fp8 {+-1}
activations x fp8 {+-1} weights on the TensorEngine.  h_l is produced by
the Activation engine as Sign(A - mean) (integer margins >= 1/16384 make
the fp32 subtract sign-safe).  Layers 2-4 run fp8 DoubleRow (two
K-subtiles per pass).  Layer 1 (continuous x) uses a 2-term fp16 split of
x that reproduces the reference's fp32 sign decisions; the hi and lo
tails of the 7th (7/8-padding) k-tile share one mixed k-tile, so layer 1
runs 13 K-passes instead of 14 while the six full-width sign-weight tiles
stay shared between the hi and lo passes.  Layer 4 applies the real
BatchNorm with weight scales.

Batch means: colsum(A_l) = sigma_l @ rowsum(h_{l-1}) distributes over the
AllReduce, so each core computes a LOCAL transposed matvec
phi = rowsum_local^T @ sigma  (rowsum stationary: LDWEIGHTS ~free; sigma
planes stream as the moving operand, DoubleRow pairs for layers 2/3) and
the AllReduce carries the phi rows instead of the rowsums -- similar
payload, but the LDWEIGHTS-bound PE matvec of the old formulation
disappears and the matvec no longer sits between the AllReduce and the
drains.  Theta is read back from the AR buffer with transposing DMAs
("r (t p) -> p t") and the phi rows are combined after, in
partition-aligned DVE ops.  For layers 2/3 the local rowsums are exact
integers |r| <= 2048, shipped to the PE as 3 balanced base-16 fp8
digits, so the threshold psums accumulate exact small integers; layer
1's theta uses an 8-digit base-16 fixed-point split of the local x
rowsums (2^-17 resolution, hierarchically extracted with exact fp32
steps) for the same reason -- the sign margins bottom out at ~1e-6 and
ANY accumulation-ordering dice in theta flips h1 entries, which the
binary net amplifies ~36x per layer (10 flips => 6% final error).

Engine-queue discipline at the AllReduce boundaries: the ACT/DVE queues
are strict in-order FIFOs, so theta-dependent ops must not be enqueued
ahead of ready work.  Each layer emits: matmuls for the first BRIDGE_T
j-tiles with psum->SBUF bridge copies (fp16 for the integer DR layers,
f32 for layer 1; even tiles copy on ACT, odd on DVE), THEN the theta
combine/scale algebra, THEN the deferred drains -- so the bridges free
psum banks while the collective is still in flight and the PE never
head-of-line blocks (worth ~30us over the naive order).  PSUM runs with
7 main accumulation banks + 1 for the theta matvecs/layer 4.

All sign-weights ship pre-signed from the host as fp8 {+-1} planes
(pre-paired for DoubleRow), so the ACT engine never runs sign-prep and
the DMA queue never serializes weight staging against the AllReduce path.

Measured (8-core axon TRN2, slope-of-n_rep method): ~477-490us vs the
538.8us/519.5us baseline; rel err 3.09e-07, zero flipped rows.
"""
import sys, os
sys.path.insert(0, '/opt/trn_rl_repo')
import numpy as np
import ml_dtypes

import concourse.bass as bass
import concourse.bacc as bacc
import concourse.tile as tile
import concourse.mybir as mybir
from concourse import bass_utils

F32 = mybir.dt.float32
BF16 = mybir.dt.bfloat16
FP16 = mybir.dt.float16
FP8 = mybir.dt.float8e4
AF = mybir.ActivationFunctionType
ALU = mybir.AluOpType
AX = mybir.AxisListType
DR = mybir.MatmulPerfMode.DoubleRow

NP8 = mybir.dt.np(mybir.dt.float8e4)

N_CORES = 8
D_IN, H, C = 784, 2048, 10
KF = 6                 # full 128-row k-tiles of x (768 rows)
KT1 = 2 * KF + 1       # 13 layer-1 K-passes (6 hi + 6 lo + 1 mixed)
NW1 = KF + 1           # 7 distinct layer-1 weight planes
KT = H // 128          # 16 k-tiles for layers 2-4
NP = KT // 2           # 8 DoubleRow k-pairs
JT = H // 128          # 16 output-channel tiles
CHUNK = 512
NCHNK = H // CHUNK     # 4 j-chunks of 512 for the theta matvecs
BRIDGE_BUFS = 8        # SBUF bridge slots for psum->sbuf theta-decoupling
BRIDGE_T = 2           # j-tiles per layer whose psums get bridged
MAGIC = float(3 << 22)  # fp32 round-to-nearest-int magic constant

# Timing-experiment knobs (leave defaults for correct results)
SKIP_DMA_REP = False   # skip input DMAs for rep>0 (garbage data, timing only)
SKIP_AR = False        # replace AllReduces with local DMA copies (wrong
                       # results on 8 cores, timing only)
DMA_ONLY = False       # emit only the input DMAs (timing the DMA floor)
NO_THETA = False       # constant thresholds: no matvec/AR/theta path at all
N_EXTRA = 0            # extra dummy DR layers after L3 (timing calibration)
MM_ONLY_EXTRA = False  # dummy layers emit only the matmuls (no drains)
N_EXTRA_L1 = 0         # extra MM-only replays of the L1 loop (timing)
LDW_OPT = False        # pass --enable-ldw-opt=true to walrus (dedup LDWs)


def _install_ldw_opt_patch():
    from concourse import bass_utils as _bu
    if getattr(_bu, "_ldw_patch", None):
        return
    _orig = _bu.run_command

    def _patched(argv, **kw):
        if LDW_OPT and any("walrus" in str(a) for a in argv[:1]):
            argv = ["--enable-ldw-opt=true" if a == "--enable-ldw-opt=false"
                    else a for a in argv]
        return _orig(argv, **kw)

    _bu.run_command = _patched
    _bu._ldw_patch = True


_install_ldw_opt_patch()


def build(n_loc: int, single: bool = False, n_rep: int = 1):
    """Emit the SPMD program for one core (all 8 run it on their own shard).

    single=True builds a 1-core variant with AllReduces replaced by plain
    copies (for cost-model timeline analysis).  n_rep repeats the whole
    forward pass back-to-back (device-time benchmarking)."""
    nch = n_loc // CHUNK
    assert n_loc % CHUNK == 0
    inv_B = 1.0 / float(n_loc * N_CORES)   # exact: power of two
    inv_H = 1.0 / float(H)

    nc = bacc.Bacc("TRN2", target_bir_lowering=False, debug=False,
                   num_devices=1 if single else N_CORES)
    nc._single_fake_ar = single

    xh = nc.dram_tensor("xh", [KF * 128, n_loc], FP16, kind="ExternalInput")
    xl = nc.dram_tensor("xl", [KF * 128, n_loc], FP16, kind="ExternalInput")
    xm = nc.dram_tensor("xm", [128, n_loc], FP16, kind="ExternalInput")
    sg1 = nc.dram_tensor("sg1", [KF * 128, H], FP8, kind="ExternalInput")
    sgm = nc.dram_tensor("sgm", [128, H], FP8, kind="ExternalInput")
    sg2 = nc.dram_tensor("sg2", [H, H], FP8, kind="ExternalInput")
    sg3 = nc.dram_tensor("sg3", [H, H], FP8, kind="ExternalInput")
    sg4 = nc.dram_tensor("sg4", [H, 16], FP8, kind="ExternalInput")
    w4n = nc.dram_tensor("w4n", [C, H], F32, kind="ExternalInput")
    g4v = nc.dram_tensor("g4v", [C, 1], F32, kind="ExternalInput")
    b4v = nc.dram_tensor("b4v", [C, 1], F32, kind="ExternalInput")
    yout = nc.dram_tensor("yout", [C, n_loc], F32, kind="ExternalOutput")

    xh_t = xh[:].rearrange("(t p) i -> t p i", p=128)
    xl_t = xl[:].rearrange("(t p) i -> t p i", p=128)
    sg1_t = sg1[:].rearrange("(t p) j -> t p j", p=128)
    sg2_t = sg2[:].rearrange("(m k p) j -> m k p j", k=128, p=2)
    sg3_t = sg3[:].rearrange("(m k p) j -> m k p j", k=128, p=2)
    sg4_t = sg4[:].rearrange("(m k p) j -> m k p j", k=128, p=2)

    salt = os.environ.get("BASS_SALT", "")
    if salt:
        nc.dram_tensor(f"salt_{salt}", [1, 4], F32, kind="Internal")
    with tile.TileContext(nc) as tc:
        for _rep in range(n_rep):
            _emit(tc, nc, n_loc, nch, inv_B, inv_H,
                  xh_t, xl_t, xm, sg1_t, sgm, sg2_t, sg3_t, sg4_t,
                  w4n, g4v, b4v, yout, _rep)
    nc.compile()
    return nc


def _emit(tc, nc, n_loc, nch, inv_B, inv_H,
          xh_t, xl_t, xm, sg1_t, sgm, sg2_t, sg3_t, sg4_t,
          w4n, g4v, b4v, yout, rep_idx=0):
    skip_in = SKIP_DMA_REP and rep_idx > 0
    def dma_in(dst, src):
        if not skip_in:
            nc.sync.dma_start(dst, src)
    import contextlib
    es = contextlib.ExitStack()
    with es:
        misc = es.enter_context(tc.tile_pool(name="misc", bufs=1))
        dram = es.enter_context(tc.tile_pool(name="dram", bufs=1, space="DRAM"))
        ps_main = es.enter_context(tc.tile_pool(name="ps_main", bufs=8, space="PSUM"))
        ps_small = ps_main
        # paired activation tiles [128, 2, n_loc]; g1/g3 rotate through p_gA,
        # g2 lives in p_gB (opened after the layer-1 pool closes)
        p_gA = es.enter_context(tc.tile_pool(name="p_gA", bufs=8))
        # sigma pool A: s1 (7x [128,H] fp8) + s2 (8 pairs); s4 has own pool
        p_sA = es.enter_context(tc.tile_pool(name="p_sA", bufs=1))
        p_s4 = es.enter_context(tc.tile_pool(name="p_s4", bufs=1))
        p_l4c = es.enter_context(tc.tile_pool(name="p_l4c", bufs=1))

        def allreduce_rows(phi_src, nrows, name):
            # AllReduce a [nrows, H] f32 block of local colsum rows, then
            # read it back transposed as [128, JT, nrows] in one DMA.
            bi = dram.tile([nrows, H], F32, name=f"{name}_bi", tag=f"{name}_bi")
            nc.sync.dma_start(bi[:], phi_src)
            dst = misc.tile([128, JT, nrows], F32, name=f"{name}_ar",
                            tag=f"{name}_ar")
            if getattr(nc, "_single_fake_ar", False) or SKIP_AR:
                src = bi
            else:
                bo = dram.tile([nrows, H], F32, addr_space="Shared",
                               name=f"{name}_bo", tag=f"{name}_bo")
                nc.gpsimd.collective_compute(
                    "AllReduce", ALU.add,
                    replica_groups=[list(range(N_CORES))],
                    ins=[bi.opt()], outs=[bo.opt()],
                )
                src = bo
            for rrow in range(nrows):
                nc.sync.dma_start(
                    dst[:, :, rrow],
                    src[rrow:rrow+1, :].rearrange("r (t p) -> (r p) t", p=128))
            return dst

        def allreduce(sbuf_src, shape, name):
            # plain AllReduce (layer-4 BN stats)
            bi = dram.tile(shape, F32, name=f"{name}_bi", tag=f"{name}_bi")
            nc.sync.dma_start(bi[:], sbuf_src)
            dst = misc.tile(shape, F32, name=f"{name}_ar", tag=f"{name}_ar")
            if getattr(nc, "_single_fake_ar", False) or SKIP_AR:
                nc.sync.dma_start(dst[:], bi[:])
                return dst
            bo = dram.tile(shape, F32, addr_space="Shared",
                           name=f"{name}_bo", tag=f"{name}_bo")
            nc.gpsimd.collective_compute(
                "AllReduce", ALU.add,
                replica_groups=[list(range(N_CORES))],
                ins=[bi.opt()], outs=[bo.opt()],
            )
            nc.sync.dma_start(dst[:], bo[:])
            return dst

        def theta_scales(raw, name):
            # negated global mean (ACT Sign bias) and positive mean (DVE
            # is_ge threshold) from the combined colsum vector [128, JT]
            thn = misc.tile([128, JT], F32, name=f"{name}_n", tag=f"{name}_n")
            nc.vector.tensor_scalar_mul(thn[:], raw, -inv_B)
            thp = misc.tile([128, JT], F32, name=f"{name}_p", tag=f"{name}_p")
            nc.vector.tensor_scalar_mul(thp[:], raw, inv_B)
            return thn, thp

        def digitize_r(r, name):
            # local rowsums (exact ints, |r|<=2048) -> 3 balanced base-16
            # fp8 digits laid out [128, KT, 16] (digit dim padded to 16 so
            # the DR weight AP step is 16-byte aligned)
            rd = misc.tile([128, KT, 16], FP8, name=f"{name}_d", tag=f"{name}_d")
            t2 = misc.tile([128, KT], F32, name=f"{name}_t2", tag=f"{name}_t2")
            nc.vector.tensor_scalar(t2[:], r[:], 1.0 / 256.0, MAGIC,
                                    ALU.mult, ALU.add)
            q2 = misc.tile([128, KT], F32, name=f"{name}_q2", tag=f"{name}_q2")
            nc.vector.tensor_scalar(q2[:], t2[:], MAGIC, None, ALU.subtract)
            rem = misc.tile([128, KT], F32, name=f"{name}_rm", tag=f"{name}_rm")
            nc.vector.scalar_tensor_tensor(rem[:], q2[:], -256.0, r[:],
                                           ALU.mult, ALU.add)
            t1 = misc.tile([128, KT], F32, name=f"{name}_t1", tag=f"{name}_t1")
            nc.vector.tensor_scalar(t1[:], rem[:], 1.0 / 16.0, MAGIC,
                                    ALU.mult, ALU.add)
            q1 = misc.tile([128, KT], F32, name=f"{name}_q1", tag=f"{name}_q1")
            nc.vector.tensor_scalar(q1[:], t1[:], MAGIC, None, ALU.subtract)
            q0 = misc.tile([128, KT], F32, name=f"{name}_q0", tag=f"{name}_q0")
            nc.vector.scalar_tensor_tensor(q0[:], q1[:], -16.0, rem[:],
                                           ALU.mult, ALU.add)
            nc.vector.tensor_copy(rd[:, :, 0], q0[:])
            nc.vector.tensor_copy(rd[:, :, 1], q1[:])
            nc.vector.tensor_copy(rd[:, :, 2], q2[:])
            return rd

        def phi_tile(name):
            # one shared [8, H] row-block for all three theta matvecs (each
            # is DMA'd to the AR input buffer long before the next layer's
            # matvec runs, so a single buffer is WAR-safe)
            return misc.tile([8, H], F32, name=name, tag="phi", bufs=1)

        def matvec_T_dr(rd, sig_pairs, name):
            # local transposed matvec, DoubleRow: phi digit rows
            # phi[d, j] = sum_k digit_d(r[k]) * sig[j, k]
            phi = phi_tile(f"{name}_phi")
            for cj in range(NCHNK):
                ps = ps_small.tile([3, CHUNK], F32, name=f"mvT_{name}_{cj}",
                                   tag="ps_main", bufs=8)
                for m in range(NP):
                    nc.tensor.matmul(ps[:], rd[:, 2*m:2*m+2, 0:3],
                                     sig_pairs[m][:, :, CHUNK*cj:CHUNK*(cj+1)],
                                     start=(m == 0), stop=(m == NP - 1),
                                     perf_mode=DR)
                nc.vector.tensor_copy(phi[0:3, CHUNK*cj:CHUNK*(cj+1)],
                                      ps[:])
            return phi

        def combine_digits(dst, name):
            # dst: [128, JT, 3] AR'd digit planes -> [128, JT] colsums
            t01 = misc.tile([128, JT], F32, name=f"{name}_c1", tag=f"{name}_c1")
            nc.vector.scalar_tensor_tensor(t01[:], dst[:, :, 2], 16.0,
                                           dst[:, :, 1], ALU.mult, ALU.add)
            raw = misc.tile([128, JT], F32, name=f"{name}_c0", tag=f"{name}_c0")
            nc.vector.scalar_tensor_tensor(raw[:], t01[:], 16.0,
                                           dst[:, :, 0], ALU.mult, ALU.add)
            return raw

        def drains(gp_of, t, srcs, thn, r, accs_tag, lname, thp=None):
            # h = Sign(A - mean) in {-1,+1} fp8, on the Activation engine
            # (margins are >= 1/16384 with |A| << 1024, so the fp32 subtract
            # never rounds to exactly 0 and Sign never emits 0).
            # Odd j-tiles drain on DVE as (h+1)/2 in {0,1} via is_ge: the
            # per-tile affine encoding cancels in every downstream
            # mean-compare (and in the final BatchNorm).
            on_dve = thp is not None and (t % 2 == 1)
            accs = misc.tile([128, nch], F32, name=f"acc_{lname}_{t}",
                             tag=accs_tag, bufs=4) if r is not None else None
            for c in range(nch):
                sl = gp_of(t, c)
                if on_dve:
                    if r is not None:
                        nc.vector.tensor_scalar(sl, srcs[c], thp[:, t:t+1],
                                                None, ALU.is_ge, ALU.add,
                                                accum_out=accs[:, c:c+1])
                    else:
                        nc.vector.tensor_scalar(sl, srcs[c], thp[:, t:t+1],
                                                None, ALU.is_ge)
                elif r is not None:
                    nc.scalar.activation(sl, srcs[c], AF.Sign,
                                         bias=thn[:, t:t+1],
                                         accum_out=accs[:, c:c+1])
                else:
                    nc.scalar.activation(sl, srcs[c], AF.Sign,
                                         bias=thn[:, t:t+1])
            if r is not None:
                nc.vector.tensor_reduce(r[:, t:t+1], accs[:], axis=AX.X,
                                        op=ALU.add)

        def alloc_g_pairs(pool, tag, lname):
            return [pool.tile([128, 2, n_loc], FP8, name=f"g_{lname}_{m}",
                              tag=tag) for m in range(NP)]

        def bridge_tile(t, pss, lname, bridge_pool, bridge_bufs,
                        bdt=F32):
            # copy psums to SBUF right away (no theta dep) so the banks
            # free up while the AllReduce for theta is still in flight.
            # Even tiles copy on ACT, odd on DVE, matching the engine that
            # will drain them -- and these copies are emitted BEFORE any
            # theta-dependent op so the in-order queues never head-of-line
            # block on the collective.
            srcs = []
            for c in range(nch):
                tb = bridge_pool.tile([128, CHUNK], bdt,
                                      name=f"br_{lname}_{t}_{c}",
                                      tag="bridge", bufs=bridge_bufs)
                if t % 2 == 0:
                    nc.scalar.activation(tb[:], pss[c], AF.Identity)
                else:
                    nc.vector.tensor_copy(tb[:], pss[c])
                srcs.append(tb)
            return srcs

        def layer_dr(sig_pairs, gin_pairs, theta_cb, gout_pairs, want_r,
                     lname, bridge_pool=None, bridge_t=None,
                     bridge_bufs=None):
            if bridge_t is None:
                bridge_t = BRIDGE_T
            if bridge_bufs is None:
                bridge_bufs = BRIDGE_BUFS
            # DoubleRow fp8 layer: A = sigma @ (prev g), drained via ACT Sign
            r = misc.tile([128, JT], F32, name=f"r_{lname}", tag=f"r_{lname}") \
                if want_r else None
            gp_of = lambda tt, cc: gout_pairs[tt//2][:, tt % 2,
                                                     CHUNK*cc:CHUNK*(cc+1)]
            pend = []
            theta = thp = None
            for t in range(JT):
                pss = [ps_main.tile([128, CHUNK], F32,
                                    name=f"ps_{lname}_{t}_{c}", tag="ps_main")
                       for c in range(nch)]
                for m in range(NP):
                    lhs = sig_pairs[m][:, :, 128*t:128*(t+1)]
                    for c in range(nch):
                        nc.tensor.matmul(pss[c], lhs,
                                         gin_pairs[m][:, :, CHUNK*c:CHUNK*(c+1)],
                                         start=(m == 0), stop=(m == NP - 1),
                                         perf_mode=DR)
                if t < bridge_t:
                    pend.append((t, bridge_tile(t, pss, lname, bridge_pool,
                                                bridge_bufs, bdt=FP16)))
                    continue
                if theta is None:
                    # theta algebra lands in the queues only now, after all
                    # bridge copies, then the deferred drains
                    theta, thp = theta_cb()
                    for (tt, ss) in pend:
                        drains(gp_of, tt, ss, theta, r, "accs", lname,
                               thp=thp)
                    pend = []
                drains(gp_of, t, pss, theta, r, "accs", lname, thp=thp)
            return r

        # ---------------- layer 1: x load (chunked), sums, weight DMAs ------
        pl1_cm = tc.tile_pool(name="pl1", bufs=1)
        pl1 = pl1_cm.__enter__()

        # startup-critical DMA order: sg1 k-tile 0 first so the PE can begin
        # the first j-tile as soon as x k-tile 0 lands; x tiles next (they
        # pace the psum accumulation); bulk sigma tiles after.
        # Layer-1 K-pass kt: 2w -> (s1[w], xh[w]); 2w+1 -> (s1[w], xl[w]);
        # 12 -> (sgm, xm) mixed hi/lo tail tile.  The interleaved hi/lo
        # order keeps the psum accumulation close to the reference's
        # k-major fp32 summation order.
        s1_tiles = []
        xs_loc = misc.tile([128, KT1], F32, name="xs_loc", tag="xs_loc")
        x_tiles = []
        for w in range(KF):
            sgt = p_sA.tile([128, H], FP8, name=f"s1_{w}", tag="s1",
                            bufs=NW1)
            dma_in(sgt[:], sg1_t[w])
            s1_tiles.append(sgt)
            hi = pl1.tile([128, n_loc], FP16, name=f"xt0_{w}", tag="xhl",
                          bufs=KT1)
            dma_in(hi[:], xh_t[w])
            nc.vector.tensor_reduce(xs_loc[:, 2*w:2*w+1], hi[:], axis=AX.X,
                                    op=ALU.add)
            x_tiles.append(hi)
            lo = pl1.tile([128, n_loc], FP16, name=f"xt1_{w}", tag="xhl",
                          bufs=KT1)
            dma_in(lo[:], xl_t[w])
            nc.vector.tensor_reduce(xs_loc[:, 2*w+1:2*w+2], lo[:],
                                    axis=AX.X, op=ALU.add)
            x_tiles.append(lo)
        sgt = p_sA.tile([128, H], FP8, name="s1_m", tag="s1", bufs=NW1)
        dma_in(sgt[:], sgm[:])
        s1_tiles.append(sgt)          # s1_tiles[KF] = mixed weight plane
        xmt = pl1.tile([128, n_loc], FP16, name="xt_m", tag="xhl", bufs=KT1)
        dma_in(xmt[:], xm[:])
        nc.vector.tensor_reduce(xs_loc[:, 2*KF:2*KF+1], xmt[:], axis=AX.X,
                                op=ALU.add)
        x_tiles.append(xmt)

        if DMA_ONLY:
            # land every input, then bail out with a token output write
            for m in range(NP):
                sg = p_sA.tile([128, 2, H], FP8, name=f"s2_{m}", tag="s2",
                               bufs=NP)
                dma_in(sg[:], sg2_t[m])
            for m in range(NP):
                sg = p_sA.tile([128, 2, H], FP8, name=f"s3_{m}", tag="s3",
                               bufs=NP)
                dma_in(sg[:], sg3_t[m])
            for m in range(NP):
                sg = p_s4.tile([128, 2, 16], FP8, name=f"s4_{m}", tag="s4",
                               bufs=NP)
                dma_in(sg[:], sg4_t[m])
            w4sb = p_l4c.tile([C, H], F32, name="w4sb", tag="w4sb")
            dma_in(w4sb[:], w4n[:])
            nc.sync.dma_start(yout[0:10, 0:13], xs_loc[0:10, :])
            pl1_cm.__exit__(None, None, None)
            return

        def w1_of(kt):
            return s1_tiles[kt // 2] if kt < 2*KF else s1_tiles[KF]

        # per-weight-plane x rowsums: the hi and lo k-tiles of plane w
        # share sign weights, so their rowsums sum before the theta matvec
        xsv = misc.tile([128, NW1], F32, name="xsv", tag="xsv")
        xsi = xs_loc[:, 0:2*KF].rearrange("p (a b) -> p a b", b=2)
        nc.vector.tensor_add(xsv[:, 0:KF], xsi[:, :, 0], xsi[:, :, 1])
        nc.vector.tensor_copy(xsv[:, KF:NW1], xs_loc[:, 2*KF:2*KF+1])

        # exact fixed-point digitization: 8 balanced base-16 fp8 digits of
        # xsv * 2^17 (hierarchical RN extraction; every step exact in fp32,
        # only the final fractional drop rounds: <= 2^-18 per value).  The
        # theta matvec psums then accumulate exact small integers, so the
        # threshold has no accumulation-order rounding dice at all.
        xs2 = misc.tile([128, NW1, 8], FP8, name="xs2", tag="xs2")
        rk = misc.tile([128, NW1], F32, name="th1_rk", tag="th1_rk")
        nc.vector.tensor_scalar_mul(rk[:], xsv[:], float(2.0 ** -11))
        for d in range(7, -1, -1):
            tm = misc.tile([128, NW1], F32, name=f"th1_t{d}", tag="th1_tm",
                           bufs=2)
            nc.vector.tensor_scalar(tm[:], rk[:], MAGIC, None, ALU.add)
            dg = misc.tile([128, NW1], F32, name=f"th1_d{d}", tag="th1_dg",
                           bufs=2)
            nc.vector.tensor_scalar(dg[:], tm[:], MAGIC, None, ALU.subtract)
            nc.vector.tensor_copy(xs2[:, :, d], dg[:])
            if d > 0:
                rem = misc.tile([128, NW1], F32, name=f"th1_r{d}",
                                tag="th1_rm", bufs=2)
                nc.vector.scalar_tensor_tensor(rem[:], dg[:], -1.0, rk[:],
                                               ALU.mult, ALU.add)
                rk = misc.tile([128, NW1], F32, name=f"th1_k{d}",
                               tag="th1_rk2", bufs=2)
                nc.vector.tensor_scalar_mul(rk[:], rem[:], 16.0)

        def zero_theta(name):
            z = misc.tile([128, JT], F32, name=name, tag=name)
            nc.vector.tensor_scalar_mul(z[:, 0:KT1], xs_loc[:], 0.0)
            nc.vector.tensor_scalar_mul(z[:, KT1:JT], xs_loc[:, 0:JT-KT1], 0.0)
            return z

        # local transposed matvec for theta1 over the 7 weight planes.
        # Emitted from inside the layer-1 main loop (after tile 2's
        # matmuls) so the in-order PE queue does not head-of-line block
        # layer 1's first tiles -- which can pace with the x DMAs --
        # behind a matvec that needs ALL x tiles landed.
        th1_state = {}

        def emit_phi1():
            phi1 = phi_tile("phi1")
            for cj in range(NCHNK):
                ps = ps_small.tile([8, CHUNK], F32, name=f"mvT_th1_{cj}",
                                   tag="ps_main", bufs=8)
                for w in range(NW1):
                    nc.tensor.matmul(ps[:], xs2[:, w, :],
                                     s1_tiles[w][:, CHUNK*cj:CHUNK*(cj+1)],
                                     start=(w == 0), stop=(w == NW1 - 1))
                nc.vector.tensor_copy(phi1[0:8, CHUNK*cj:CHUNK*(cj+1)],
                                      ps[:])
            if not NO_THETA:
                th1_state["D1"] = allreduce_rows(phi1[0:8, :], 8, "phi1")

        if NO_THETA:
            def th1_cb():
                z1 = zero_theta("zth1")
                return theta_scales(z1[:], "th1")
        else:
            def th1_cb():
                D1 = th1_state["D1"]
                # theta = sum_d D1[..d] * 16^d * 2^-17 / B, smallest digit
                # first so the rounding stays at ~2 ulp of the result
                acc = misc.tile([128, JT], F32, name="th1_a0", tag="th1_acc",
                                bufs=2)
                nc.vector.tensor_scalar_mul(acc[:], D1[:, :, 0],
                                            float(2.0 ** -17) * inv_B)
                for d in range(1, 8):
                    nxt = misc.tile([128, JT], F32, name=f"th1_a{d}",
                                    tag="th1_acc", bufs=2)
                    nc.vector.scalar_tensor_tensor(
                        nxt[:], D1[:, :, d], float(2.0 ** (4*d - 17)) * inv_B,
                        acc[:], ALU.mult, ALU.add)
                    acc = nxt
                thn = misc.tile([128, JT], F32, name="th1_n", tag="th1_n")
                nc.vector.tensor_scalar_mul(thn[:], acc[:], -1.0)
                return thn, acc

        # sigma2 pair tiles: direct DMA of host-signed fp8 planes
        s2_pairs = []
        for m in range(NP):
            sg = p_sA.tile([128, 2, H], FP8, name=f"s2_{m}", tag="s2", bufs=NP)
            dma_in(sg[:], sg2_t[m])
            s2_pairs.append(sg)

        # layer-4 statics (tiny): land them during layer 1 so the DMA queue
        # is empty around every AllReduce
        s4_pairs = []
        for m in range(NP):
            sg = p_s4.tile([128, 2, 16], FP8, name=f"s4_{m}", tag="s4", bufs=NP)
            dma_in(sg[:], sg4_t[m])
            s4_pairs.append(sg)
        w4sb = p_l4c.tile([C, H], F32, name="w4sb", tag="w4sb")
        dma_in(w4sb[:], w4n[:])
        g4sb = misc.tile([C, 1], F32, name="g4sb", tag="g4sb")
        dma_in(g4sb[:], g4v[:])
        b4sb = misc.tile([C, 1], F32, name="b4sb", tag="b4sb")
        dma_in(b4sb[:], b4v[:])

        s4raw = misc.tile([C, 1], F32, name="s4raw", tag="s4raw")
        nc.vector.tensor_reduce(s4raw[:], w4sb[:], axis=AX.X, op=ALU.add,
                                apply_absolute_value=True)
        s4 = misc.tile([C, 1], F32, name="s4", tag="s4sc")
        nc.vector.tensor_scalar_mul(s4[:], s4raw[:], inv_H)     # mean|w4|
        ns4sq = misc.tile([C, 1], F32, name="ns4sq", tag="ns4sq")
        nc.vector.tensor_scalar(ns4sq[:], s4[:], s4[:], -1.0,
                                ALU.mult, ALU.mult)              # -s4^2

        # ---------------- layer 1 main (13-pass 2-term fp16 x fp8 sign) -----
        g1_pairs = alloc_g_pairs(p_gA, "gA", "l1")
        r1 = misc.tile([128, JT], F32, name="r_l1", tag="r_l1")
        gp1 = lambda tt, cc: g1_pairs[tt//2][:, tt % 2,
                                             CHUNK*cc:CHUNK*(cc+1)]
        pend1 = []
        theta1 = thp1 = None
        for t in range(JT):
            pss = [ps_main.tile([128, CHUNK], F32, name=f"ps_l1_{t}_{c}",
                                tag="ps_main") for c in range(nch)]
            for kt in range(KT1):
                lhs = w1_of(kt)[:, 128*t:128*(t+1)]
                for c in range(nch):
                    nc.tensor.matmul(pss[c], lhs,
                                     x_tiles[kt][:, CHUNK*c:CHUNK*(c+1)],
                                     start=(kt == 0), stop=(kt == KT1 - 1))
            if t == 1:
                emit_phi1()
            if t < BRIDGE_T:
                pend1.append((t, bridge_tile(t, pss, "l1", pl1,
                                             BRIDGE_BUFS)))
            else:
                if theta1 is None:
                    theta1, thp1 = th1_cb()
                    for (tt, ss) in pend1:
                        drains(gp1, tt, ss, theta1, r1, "accs", "l1",
                               thp=thp1)
                    pend1 = []
                drains(gp1, t, pss, theta1, r1, "accs", "l1", thp=thp1)
            if t == 3:
                # sigma3 pair tiles: DMA mid-layer-1 (queue is idle then;
                # landing them early keeps the phi AllReduce DMAs and the
                # ACT drain stream unblocked at the layer boundaries)
                s3_pairs = []
                for m in range(NP):
                    sg = p_sA.tile([128, 2, H], FP8, name=f"s3_{m}",
                                   tag="s3", bufs=NP)
                    dma_in(sg[:], sg3_t[m])
                    s3_pairs.append(sg)

        for _e in range(N_EXTRA_L1):
            for t in range(JT):
                pss = [ps_main.tile([128, CHUNK], F32,
                                    name=f"px1_{_e}_{t}_{c}", tag="ps_main")
                       for c in range(nch)]
                for kt in range(KT1):
                    lhs = w1_of(kt)[:, 128*t:128*(t+1)]
                    for c in range(nch):
                        nc.tensor.matmul(pss[c], lhs,
                                         x_tiles[kt][:, CHUNK*c:CHUNK*(c+1)],
                                         start=(kt == 0),
                                         stop=(kt == KT1 - 1))

        pl1_cm.__exit__(None, None, None)

        p_gB = es.enter_context(tc.tile_pool(name="p_gB", bufs=8))
        brB_cm = tc.tile_pool(name="brB", bufs=1)
        brB = brB_cm.__enter__()

        if NO_THETA:
            th2_cb = lambda: theta_scales(zero_theta("zth2")[:], "th2")
        else:
            rd1 = digitize_r(r1, "rd1")
            phi2 = matvec_T_dr(rd1, s2_pairs, "th2")
            D2 = allreduce_rows(phi2[0:3, :], 3, "phi2")
            th2_cb = lambda: theta_scales(combine_digits(D2, "th2")[:],
                                          "th2")

        g2_pairs = alloc_g_pairs(p_gB, "gB", "l2")
        r2 = layer_dr(s2_pairs, g1_pairs, th2_cb, g2_pairs, True, "l2",
                      bridge_pool=brB, bridge_t=5, bridge_bufs=20)

        if NO_THETA:
            th3_cb = lambda: theta_scales(zero_theta("zth3")[:], "th3")
        else:
            rd2 = digitize_r(r2, "rd2")
            phi3 = matvec_T_dr(rd2, s3_pairs, "th3")
            D3 = allreduce_rows(phi3[0:3, :], 3, "phi3")
            th3_cb = lambda: theta_scales(combine_digits(D3, "th3")[:],
                                          "th3")

        g3_pairs = alloc_g_pairs(p_gA, "gA", "l3")
        th3_memo = []

        def th3_once():
            if not th3_memo:
                th3_memo.append(th3_cb())
            return th3_memo[0]

        layer_dr(s3_pairs, g2_pairs, th3_once, g3_pairs, False, "l3",
                 bridge_pool=brB, bridge_t=5, bridge_bufs=20)

        if N_EXTRA:
            # dummy layers recycle dead pools: s2 (dead after L2) and
            # gB/g2 (dead once dummy0 replaces g3 as the consumer)
            assert N_EXTRA <= 2
            gin = g3_pairs
            for e in range(N_EXTRA):
                if MM_ONLY_EXTRA:
                    for t in range(JT):
                        pss = [ps_main.tile([128, CHUNK], F32,
                                            name=f"px_{e}_{t}_{c}",
                                            tag="ps_main")
                               for c in range(nch)]
                        for m in range(NP):
                            lhs = s3_pairs[m][:, :, 128*t:128*(t+1)]
                            for c in range(nch):
                                nc.tensor.matmul(
                                    pss[c], lhs,
                                    gin[m][:, :, CHUNK*c:CHUNK*(c+1)],
                                    start=(m == 0), stop=(m == NP - 1),
                                    perf_mode=DR)
                    continue
                pool, tag = (p_sA, "s2") if e == 0 else (p_gB, "gB")
                gx = [pool.tile([128, 2, n_loc], FP8, name=f"g_x{e}_{m}",
                                tag=tag, bufs=NP) for m in range(NP)]
                layer_dr(s3_pairs, gin, th3_once, gx, False, f"lx{e}")
                gin = gx

        brB_cm.__exit__(None, None, None)
        p_l4 = es.enter_context(tc.tile_pool(name="p_l4", bufs=1))

        # ---------------- layer 4 + BatchNorm ----------------
        # k4 = sign(W4) @ h3 (exact small integers); BN statistics of
        # y4 = s4*k4 are folded into per-channel affine coefficients so the
        # only full-width op after the AllReduce is one tensor_scalar.
        k4 = p_l4.tile([C, n_loc], F32, name="k4", tag="k4")
        ksum = misc.tile([C, nch], F32, name="ksum", tag="ksum")
        ksq = misc.tile([C, nch], F32, name="ksq", tag="ksq")
        for c in range(nch):
            ps = ps_small.tile([16, CHUNK], F32, name=f"ps_l4_{c}", tag="ps_main", bufs=8)
            for m in range(NP):
                nc.tensor.matmul(ps[:], s4_pairs[m][:],
                                 g3_pairs[m][:, :, CHUNK*c:CHUNK*(c+1)],
                                 start=(m == 0), stop=(m == NP - 1),
                                 perf_mode=DR)
            nc.vector.tensor_scalar(k4[:, CHUNK*c:CHUNK*(c+1)], ps[0:C, :],
                                    1.0, None, ALU.mult, ALU.add,
                                    accum_out=ksum[:, c:c+1])
            sqsc = p_l4.tile([C, CHUNK], F32, name=f"sq_{c}", tag="sqsc",
                             bufs=2)
            nc.scalar.activation(sqsc[:], ps[0:C, :], AF.Square,
                                 accum_out=ksq[:, c:c+1])

        # pre-scale the stats by 1/B before the AllReduce: the reduced
        # result is then directly (mu_k, E[k^2])
        p4 = misc.tile([C, 2], F32, name="p4", tag="p4")
        nc.vector.tensor_reduce(p4[:, 0:1], ksum[:], axis=AX.X, op=ALU.add)
        nc.vector.tensor_reduce(p4[:, 1:2], ksq[:], axis=AX.X, op=ALU.add)
        p4s = misc.tile([C, 2], F32, name="p4s", tag="p4s")
        nc.vector.tensor_scalar_mul(p4s[:], p4[:], inv_B)

        G4 = allreduce(p4s[:], [C, 2], "p4")
        # -var = mu^2 - E[k^2];  veps = (-var)*(-s4^2) + eps
        nvar = misc.tile([C, 1], F32, name="nvar", tag="nvar")
        nc.vector.tensor_scalar(nvar[:], G4[:, 0:1], G4[:, 0:1], G4[:, 1:2],
                                ALU.mult, ALU.subtract)
        veps = misc.tile([C, 1], F32, name="veps", tag="veps")
        nc.vector.tensor_scalar(veps[:], nvar[:], ns4sq[:], 1e-5,
                                ALU.mult, ALU.add)
        sd = misc.tile([C, 1], F32, name="sd", tag="sd")
        nc.scalar.activation(sd[:], veps[:], AF.Sqrt)
        inv_sd = misc.tile([C, 1], F32, name="inv_sd", tag="inv_sd")
        nc.vector.reciprocal(inv_sd[:], sd[:])
        # A = g*s4/sd(y);  B0 = b - mu_k*A
        A4 = misc.tile([C, 1], F32, name="A4", tag="A4")
        nc.vector.tensor_scalar(A4[:], inv_sd[:], g4sb[:], s4[:],
                                ALU.mult, ALU.mult)
        muA = misc.tile([C, 1], F32, name="muA", tag="muA")
        nc.vector.tensor_mul(muA[:], G4[:, 0:1], A4[:])
        B4 = misc.tile([C, 1], F32, name="B4", tag="B4")
        nc.vector.tensor_sub(B4[:], b4sb[:], muA[:])

        # final affine: split halves across DVE and ACT, DMA out per half
        yo = p_l4.tile([C, n_loc], F32, name="yo", tag="yo")
        hn = n_loc // 2
        nc.vector.tensor_scalar(yo[:, 0:hn], k4[:, 0:hn], A4[:], B4[:],
                                ALU.mult, ALU.add)
        nc.sync.dma_start(yout[:, 0:hn], yo[:, 0:hn])
        nc.scalar.activation(yo[:, hn:], k4[:, hn:], AF.Identity,
                             bias=B4[:], scale=A4[:])
        nc.sync.dma_start(yout[:, hn:], yo[:, hn:])


# --------------------------------------------------------------------------
def prep_inputs(x, w1, w2, w3, w4, g4, b4, n_loc):
    n_cores = N_CORES
    B = x.shape[0]
    assert B == n_loc * n_cores

    f16 = np.float16

    def signs(w):
        return np.where(np.asarray(w) >= 0, np.float32(1.0),
                        np.float32(-1.0))

    # sg1: [768, H], row k = sign(w1[:, k]); sgm: [128, H] mixed tail plane
    # (rows 0-15 = sign rows 768-783, rows 16-31 the same, rest zero)
    s1 = signs(w1).T.astype(NP8)          # [784, 2048]
    sg1 = np.ascontiguousarray(s1[:KF*128])
    sgm = np.zeros((128, H), dtype=NP8)
    sgm[0:D_IN - KF*128] = s1[KF*128:]
    sgm[16:16 + D_IN - KF*128] = s1[KF*128:]

    def paired(w, width):
        # [H, width]: rows (m, k, p) -> sign(w[j, 128*(2m+p)+k]), j < width.
        # Odd input-tile planes (p=1) are scaled x2: those tiles' activations
        # are stored as (h+1)/2 in {0,1} by the DVE drain path.
        s = signs(w)                       # [out, H]
        out = np.zeros((NP, 128, 2, width), dtype=NP8)
        for m in range(NP):
            for p in (0, 1):
                blk = s[:, 128*(2*m+p):128*(2*m+p+1)] * (1.0 + p)
                out[m, :, p, :blk.shape[0]] = blk.T.astype(NP8)
        return out.reshape(H, width)

    sg2 = paired(w2, H)
    sg3 = paired(w3, H)
    sg4 = paired(w4, 16)
    w4n = np.ascontiguousarray(np.asarray(w4).astype(np.float32))
    g4v = np.ascontiguousarray(np.asarray(g4).reshape(C, 1).astype(np.float32))
    b4v = np.ascontiguousarray(np.asarray(b4).reshape(C, 1).astype(np.float32))

    in_maps = []
    ntail = D_IN - KF*128                  # 16 tail rows
    for cidx in range(n_cores):
        xs = x[n_loc*cidx:n_loc*(cidx+1)]
        xT = np.ascontiguousarray(xs.T.astype(np.float32))   # [784, n_loc]
        xhp = xT.astype(f16)
        xlp = (xT - xhp.astype(np.float32)).astype(f16)
        xmv = np.zeros((128, n_loc), dtype=f16)
        xmv[0:ntail] = xhp[KF*128:]
        xmv[16:16 + ntail] = xlp[KF*128:]
        m = {
            "xh": np.ascontiguousarray(xhp[:KF*128]),
            "xl": np.ascontiguousarray(xlp[:KF*128]),
            "xm": xmv,
            "sg1": sg1, "sgm": sgm, "sg2": sg2, "sg3": sg3,
            "sg4": sg4, "w4n": w4n, "g4v": g4v, "b4v": b4v,
        }
        in_maps.append(m)
    return in_maps


_NC_CACHE = {}


def kernel(x, w1, w2, w3, w4, g1, b1, g2, b2, g3, b3, g4, b4):
    x = np.asarray(x); w1 = np.asarray(w1); w2 = np.asarray(w2)
    w3 = np.asarray(w3); w4 = np.asarray(w4)
    g4 = np.asarray(g4); b4 = np.asarray(b4)
    # layers 1-3 BN params: scales cancel inside sign() only when gamma>0, beta=0
    for g in (g1, g2, g3):
        assert np.all(np.asarray(g) > 0), "kernel assumes gamma > 0 for hidden BNs"
    for b in (b1, b2, b3):
        assert np.all(np.asarray(b) == 0), "kernel assumes beta == 0 for hidden BNs"
    for w in (w1, w2, w3, w4):
        assert not np.any(w == 0.0), "exact-zero weight would break Sign()"

    n_loc = x.shape[0] // N_CORES
    if n_loc not in _NC_CACHE:
        _NC_CACHE[n_loc] = build(n_loc)
    nc = _NC_CACHE[n_loc]

    in_maps = prep_inputs(x, w1, w2, w3, w4, g4, b4, n_loc)
    res = bass_utils.run_bass_kernel_spmd(nc, in_maps,
                                          core_ids=list(range(N_CORES)))
    out = np.concatenate([res.results[c]["yout"].T for c in range(N_CORES)],
                         axis=0)
    return out.astype(np.float32)


# revision 40
# speedup vs baseline: 1.0935x; 1.0361x over previous
"""BinaryNet MLP forward on 8 TRN2 NeuronCores.

Strategy: data-parallel over batch (2048 rows/core), feature-major on-chip
layout (activations stored [channel, batch]).  For layers 1-3 the positive
per-row weight scales and the BatchNorm variance cancel inside sign(), so
each layer reduces to:  g_l = 1{ A_l >= mean_batch(A_l) }  where
A_l = sign(W_l) @ h_{l-1} is an exact small integer computed with fp8 {+-1}
activations x fp8 {+-1} weights on the TensorEngine.  h_l is produced by
the Activation engine as Sign(A - mean) (integer margins >= 1/16384 make
the fp32 subtract sign-safe).  Layers 2-4 run fp8 DoubleRow (two
K-subtiles per pass).  Layer 1 (continuous x) uses a 2-term fp16 split of
x that reproduces the reference's fp32 sign decisions; the hi and lo
tails of the 7th (7/8-padding) k-tile share one mixed k-tile, so layer 1
runs 13 K-passes instead of 14 while the six full-width sign-weight tiles
stay shared between the hi and lo passes.  Layer 4 applies the real
BatchNorm with weight scales.

Batch means: colsum(A_l) = sigma_l @ rowsum(h_{l-1}) distributes over the
AllReduce, so each core computes a LOCAL transposed matvec
phi = rowsum_local^T @ sigma  (rowsum stationary: LDWEIGHTS ~free; sigma
planes stream as the moving operand, DoubleRow pairs for layers 2/3) and
the AllReduce carries the phi rows instead of the rowsums -- similar
payload, but the LDWEIGHTS-bound PE matvec of the old formulation
disappears and the matvec no longer sits between the AllReduce and the
drains.  Theta is read back from the AR buffer with transposing DMAs
("r (t p) -> p t") and the phi rows are combined after, in
partition-aligned DVE ops.  For layers 2/3 the local rowsums are exact
integers |r| <= 2048, shipped to the PE as 3 balanced base-16 fp8
digits, so the threshold psums accumulate exact small integers; layer
1's theta uses an 8-digit base-16 fixed-point split of the local x
rowsums (2^-17 resolution, hierarchically extracted with exact fp32
steps) for the same reason -- the sign margins bottom out at ~1e-6 and
ANY accumulation-ordering dice in theta flips h1 entries, which the
binary net amplifies ~36x per layer (10 flips => 6% final error).

Engine-queue discipline at the AllReduce boundaries: the ACT/DVE queues
are strict in-order FIFOs, so theta-dependent ops must not be enqueued
ahead of ready work.  Each layer emits: matmuls for the first BRIDGE_T
j-tiles with psum->SBUF bridge copies (fp16 for the integer DR layers,
f32 for layer 1; even tiles copy on ACT, odd on DVE), THEN the theta
combine/scale algebra, THEN the deferred drains -- so the bridges free
psum banks while the collective is still in flight and the PE never
head-of-line blocks (worth ~30us over the naive order).  PSUM runs with
7 main accumulation banks + 1 for the theta matvecs/layer 4.

All sign-weights ship pre-signed from the host as fp8 {+-1} planes
(pre-paired for DoubleRow), so the ACT engine never runs sign-prep and
the DMA queue never serializes weight staging against the AllReduce path.

Measured (8-core axon TRN2, slope-of-n_rep method): ~477-490us vs the
538.8us/519.5us baseline; rel err 3.09e-07, zero flipped rows.
"""
import sys, os
sys.path.insert(0, '/opt/trn_rl_repo')
import numpy as np
import ml_dtypes

import concourse.bass as bass
import concourse.bacc as bacc
import concourse.tile as tile
import concourse.mybir as mybir
from concourse import bass_utils

F32 = mybir.dt.float32
BF16 = mybir.dt.bfloat16
FP16 = mybir.dt.float16
FP8 = mybir.dt.float8e4
AF = mybir.ActivationFunctionType
ALU = mybir.AluOpType
AX = mybir.AxisListType
DR = mybir.MatmulPerfMode.DoubleRow

NP8 = mybir.dt.np(mybir.dt.float8e4)

N_CORES = 8
D_IN, H, C = 784, 2048, 10
KF = 6                 # full 128-row k-tiles of x (768 rows)
KT1 = 2 * KF + 1       # 13 layer-1 K-passes (6 hi + 6 lo + 1 mixed)
NW1 = KF + 1           # 7 distinct layer-1 weight planes
KT = H // 128          # 16 k-tiles for layers 2-4
NP = KT // 2           # 8 DoubleRow k-pairs
JT = H // 128          # 16 output-channel tiles
CHUNK = 512
NCHNK = H // CHUNK     # 4 j-chunks of 512 for the theta matvecs
BRIDGE_BUFS = 8        # SBUF bridge slots for psum->sbuf theta-decoupling
BRIDGE_T = 2           # j-tiles per layer whose psums get bridged
MAGIC = float(3 << 22)  # fp32 round-to-nearest-int magic constant

# Timing-experiment knobs (leave defaults for correct results)
SKIP_DMA_REP = False   # skip input DMAs for rep>0 (garbage data, timing only)
SKIP_AR = False        # replace AllReduces with local DMA copies (wrong
                       # results on 8 cores, timing only)
DMA_ONLY = False       # emit only the input DMAs (timing the DMA floor)
NO_THETA = False       # constant thresholds: no matvec/AR/theta path at all
N_EXTRA = 0            # extra dummy DR layers after L3 (timing calibration)
MM_ONLY_EXTRA = False  # dummy layers emit only the matmuls (no drains)
N_EXTRA_L1 = 0         # extra MM-only replays of the L1 loop (timing)
LDW_OPT = False        # pass --enable-ldw-opt=true to walrus (dedup LDWs)


def _install_ldw_opt_patch():
    from concourse import bass_utils as _bu
    if getattr(_bu, "_ldw_patch", None):
        return
    _orig = _bu.run_command

    def _patched(argv, **kw):
        if LDW_OPT and any("walrus" in str(a) for a in argv[:1]):
            argv = ["--enable-ldw-opt=true" if a == "--enable-ldw-opt=false"
                    else a for a in argv]
        return _orig(argv, **kw)

    _bu.run_command = _patched
    _bu._ldw_patch = True


_install_ldw_opt_patch()


def build(n_loc: int, single: bool = False, n_rep: int = 1):
    """Emit the SPMD program for one core (all 8 run it on their own shard).

    single=True builds a 1-core variant with AllReduces replaced by plain
    copies (for cost-model timeline analysis).  n_rep repeats the whole
    forward pass back-to-back (device-time benchmarking)."""
    nch = n_loc // CHUNK
    assert n_loc % CHUNK == 0
    inv_B = 1.0 / float(n_loc * N_CORES)   # exact: power of two
    inv_H = 1.0 / float(H)

    nc = bacc.Bacc("TRN2", target_bir_lowering=False, debug=False,
                   num_devices=1 if single else N_CORES)
    nc._single_fake_ar = single

    xh = nc.dram_tensor("xh", [KF * 128, n_loc], FP16, kind="ExternalInput")
    xl = nc.dram_tensor("xl", [KF * 128, n_loc], FP16, kind="ExternalInput")
    xm = nc.dram_tensor("xm", [128, n_loc], FP16, kind="ExternalInput")
    sg1 = nc.dram_tensor("sg1", [KF * 128, H], FP8, kind="ExternalInput")
    sgm = nc.dram_tensor("sgm", [128, H], FP8, kind="ExternalInput")
    sg2 = nc.dram_tensor("sg2", [H, H], FP8, kind="ExternalInput")
    sg3 = nc.dram_tensor("sg3", [H, H], FP8, kind="ExternalInput")
    sg4 = nc.dram_tensor("sg4", [H, 16], FP8, kind="ExternalInput")
    w4n = nc.dram_tensor("w4n", [C, H], F32, kind="ExternalInput")
    g4v = nc.dram_tensor("g4v", [C, 1], F32, kind="ExternalInput")
    b4v = nc.dram_tensor("b4v", [C, 1], F32, kind="ExternalInput")
    yout = nc.dram_tensor("yout", [C, n_loc], F32, kind="ExternalOutput")

    xh_t = xh[:].rearrange("(t p) i -> t p i", p=128)
    xl_t = xl[:].rearrange("(t p) i -> t p i", p=128)
    sg1_t = sg1[:].rearrange("(t p) j -> t p j", p=128)
    sg2_t = sg2[:].rearrange("(m k p) j -> m k p j", k=128, p=2)
    sg3_t = sg3[:].rearrange("(m k p) j -> m k p j", k=128, p=2)
    sg4_t = sg4[:].rearrange("(m k p) j -> m k p j", k=128, p=2)

    salt = os.environ.get("BASS_SALT", "")
    if salt:
        nc.dram_tensor(f"salt_{salt}", [1, 4], F32, kind="Internal")
    with tile.TileContext(nc) as tc:
        for _rep in range(n_rep):
            _emit(tc, nc, n_loc, nch, inv_B, inv_H,
                  xh_t, xl_t, xm, sg1_t, sgm, sg2_t, sg3_t, sg4_t,
                  w4n, g4v, b4v, yout, _rep)
    nc.compile()
    return nc


def _emit(tc, nc, n_loc, nch, inv_B, inv_H,
          xh_t, xl_t, xm, sg1_t, sgm, sg2_t, sg3_t, sg4_t,
          w4n, g4v, b4v, yout, rep_idx=0):
    skip_in = SKIP_DMA_REP and rep_idx > 0
    def dma_in(dst, src):
        if not skip_in:
            nc.sync.dma_start(dst, src)
    import contextlib
    es = contextlib.ExitStack()
    with es:
        misc = es.enter_context(tc.tile_pool(name="misc", bufs=1))
        dram = es.enter_context(tc.tile_pool(name="dram", bufs=1, space="DRAM"))
        ps_main = es.enter_context(tc.tile_pool(name="ps_main", bufs=8, space="PSUM"))
        ps_small = ps_main
        # paired activation tiles [128, 2, n_loc]; g1/g3 rotate through p_gA,
        # g2 lives in p_gB (opened after the layer-1 pool closes)
        p_gA = es.enter_context(tc.tile_pool(name="p_gA", bufs=8))
        # sigma pool A: s1 (7x [128,H] fp8) + s2 (8 pairs); s4 has own pool
        p_sA = es.enter_context(tc.tile_pool(name="p_sA", bufs=1))
        p_s4 = es.enter_context(tc.tile_pool(name="p_s4", bufs=1))
        p_l4c = es.enter_context(tc.tile_pool(name="p_l4c", bufs=1))

        def allreduce_rows(phi_src, nrows, name):
            # AllReduce a [nrows, H] f32 block of local colsum rows, then
            # read it back transposed as [128, JT, nrows] in one DMA.
            bi = dram.tile([nrows, H], F32, name=f"{name}_bi", tag=f"{name}_bi")
            nc.sync.dma_start(bi[:], phi_src)
            dst = misc.tile([128, JT, nrows], F32, name=f"{name}_ar",
                            tag=f"{name}_ar")
            if getattr(nc, "_single_fake_ar", False) or SKIP_AR:
                src = bi
            else:
                bo = dram.tile([nrows, H], F32, addr_space="Shared",
                               name=f"{name}_bo", tag=f"{name}_bo")
                nc.gpsimd.collective_compute(
                    "AllReduce", ALU.add,
                    replica_groups=[list(range(N_CORES))],
                    ins=[bi.opt()], outs=[bo.opt()],
                )
                src = bo
            for rrow in range(nrows):
                nc.sync.dma_start(
                    dst[:, :, rrow],
                    src[rrow:rrow+1, :].rearrange("r (t p) -> (r p) t", p=128))
            return dst

        def allreduce(sbuf_src, shape, name):
            # plain AllReduce (layer-4 BN stats)
            bi = dram.tile(shape, F32, name=f"{name}_bi", tag=f"{name}_bi")
            nc.sync.dma_start(bi[:], sbuf_src)
            dst = misc.tile(shape, F32, name=f"{name}_ar", tag=f"{name}_ar")
            if getattr(nc, "_single_fake_ar", False) or SKIP_AR:
                nc.sync.dma_start(dst[:], bi[:])
                return dst
            bo = dram.tile(shape, F32, addr_space="Shared",
                           name=f"{name}_bo", tag=f"{name}_bo")
            nc.gpsimd.collective_compute(
                "AllReduce", ALU.add,
                replica_groups=[list(range(N_CORES))],
                ins=[bi.opt()], outs=[bo.opt()],
            )
            nc.sync.dma_start(dst[:], bo[:])
            return dst

        def theta_scales(raw, name):
            # negated global mean (ACT Sign bias) and positive mean (DVE
            # is_ge threshold) from the combined colsum vector [128, JT]
            thn = misc.tile([128, JT], F32, name=f"{name}_n", tag=f"{name}_n")
            nc.vector.tensor_scalar_mul(thn[:], raw, -inv_B)
            thp = misc.tile([128, JT], F32, name=f"{name}_p", tag=f"{name}_p")
            nc.vector.tensor_scalar_mul(thp[:], raw, inv_B)
            return thn, thp

        def digitize_r(r, name):
            # local rowsums (exact ints, |r|<=2048) -> 3 balanced base-16
            # fp8 digits laid out [128, KT, 16] (digit dim padded to 16 so
            # the DR weight AP step is 16-byte aligned)
            rd = misc.tile([128, KT, 16], FP8, name=f"{name}_d", tag=f"{name}_d")
            t2 = misc.tile([128, KT], F32, name=f"{name}_t2", tag=f"{name}_t2")
            nc.vector.tensor_scalar(t2[:], r[:], 1.0 / 256.0, MAGIC,
                                    ALU.mult, ALU.add)
            q2 = misc.tile([128, KT], F32, name=f"{name}_q2", tag=f"{name}_q2")
            nc.vector.tensor_scalar(q2[:], t2[:], MAGIC, None, ALU.subtract)
            rem = misc.tile([128, KT], F32, name=f"{name}_rm", tag=f"{name}_rm")
            nc.vector.scalar_tensor_tensor(rem[:], q2[:], -256.0, r[:],
                                           ALU.mult, ALU.add)
            t1 = misc.tile([128, KT], F32, name=f"{name}_t1", tag=f"{name}_t1")
            nc.vector.tensor_scalar(t1[:], rem[:], 1.0 / 16.0, MAGIC,
                                    ALU.mult, ALU.add)
            q1 = misc.tile([128, KT], F32, name=f"{name}_q1", tag=f"{name}_q1")
            nc.vector.tensor_scalar(q1[:], t1[:], MAGIC, None, ALU.subtract)
            q0 = misc.tile([128, KT], F32, name=f"{name}_q0", tag=f"{name}_q0")
            nc.vector.scalar_tensor_tensor(q0[:], q1[:], -16.0, rem[:],
                                           ALU.mult, ALU.add)
            nc.vector.tensor_copy(rd[:, :, 0], q0[:])
            nc.vector.tensor_copy(rd[:, :, 1], q1[:])
            nc.vector.tensor_copy(rd[:, :, 2], q2[:])
            return rd

        def phi_tile(name):
            # one shared [8, H] row-block for all three theta matvecs (each
            # is DMA'd to the AR input buffer long before the next layer's
            # matvec runs, so a single buffer is WAR-safe)
            return misc.tile([8, H], F32, name=name, tag="phi", bufs=1)

        def matvec_T_dr(rd, sig_pairs, name):
            # local transposed matvec, DoubleRow: phi digit rows
            # phi[d, j] = sum_k digit_d(r[k]) * sig[j, k]
            phi = phi_tile(f"{name}_phi")
            for cj in range(NCHNK):
                ps = ps_small.tile([3, CHUNK], F32, name=f"mvT_{name}_{cj}",
                                   tag="ps_main", bufs=8)
                for m in range(NP):
                    nc.tensor.matmul(ps[:], rd[:, 2*m:2*m+2, 0:3],
                                     sig_pairs[m][:, :, CHUNK*cj:CHUNK*(cj+1)],
                                     start=(m == 0), stop=(m == NP - 1),
                                     perf_mode=DR)
                nc.vector.tensor_copy(phi[0:3, CHUNK*cj:CHUNK*(cj+1)],
                                      ps[:])
            return phi

        def combine_digits(dst, name):
            # dst: [128, JT, 3] AR'd digit planes -> [128, JT] colsums
            t01 = misc.tile([128, JT], F32, name=f"{name}_c1", tag=f"{name}_c1")
            nc.vector.scalar_tensor_tensor(t01[:], dst[:, :, 2], 16.0,
                                           dst[:, :, 1], ALU.mult, ALU.add)
            raw = misc.tile([128, JT], F32, name=f"{name}_c0", tag=f"{name}_c0")
            nc.vector.scalar_tensor_tensor(raw[:], t01[:], 16.0,
                                           dst[:, :, 0], ALU.mult, ALU.add)
            return raw

        def drains(gp_of, t, srcs, thn, r, accs_tag, lname, thp=None):
            # h = Sign(A - mean) in {-1,+1} fp8, on the Activation engine
            # (margins are >= 1/16384 with |A| << 1024, so the fp32 subtract
            # never rounds to exactly 0 and Sign never emits 0).
            # Odd j-tiles drain on DVE as (h+1)/2 in {0,1} via is_ge: the
            # per-tile affine encoding cancels in every downstream
            # mean-compare (and in the final BatchNorm).
            on_dve = thp is not None and (t % 2 == 1)
            accs = misc.tile([128, nch], F32, name=f"acc_{lname}_{t}",
                             tag=accs_tag, bufs=4) if r is not None else None
            for c in range(nch):
                sl = gp_of(t, c)
                if on_dve:
                    if r is not None:
                        nc.vector.tensor_scalar(sl, srcs[c], thp[:, t:t+1],
                                                None, ALU.is_ge, ALU.add,
                                                accum_out=accs[:, c:c+1])
                    else:
                        nc.vector.tensor_scalar(sl, srcs[c], thp[:, t:t+1],
                                                None, ALU.is_ge)
                elif r is not None:
                    nc.scalar.activation(sl, srcs[c], AF.Sign,
                                         bias=thn[:, t:t+1],
                                         accum_out=accs[:, c:c+1])
                else:
                    nc.scalar.activation(sl, srcs[c], AF.Sign,
                                         bias=thn[:, t:t+1])
            if r is not None:
                nc.vector.tensor_reduce(r[:, t:t+1], accs[:], axis=AX.X,
                                        op=ALU.add)

        def alloc_g_pairs(pool, tag, lname):
            return [pool.tile([128, 2, n_loc], FP8, name=f"g_{lname}_{m}",
                              tag=tag) for m in range(NP)]

        def bridge_tile(t, pss, lname, bridge_pool, bridge_bufs,
                        bdt=F32):
            # copy psums to SBUF right away (no theta dep) so the banks
            # free up while the AllReduce for theta is still in flight.
            # Even tiles copy on ACT, odd on DVE, matching the engine that
            # will drain them -- and these copies are emitted BEFORE any
            # theta-dependent op so the in-order queues never head-of-line
            # block on the collective.
            srcs = []
            for c in range(nch):
                tb = bridge_pool.tile([128, CHUNK], bdt,
                                      name=f"br_{lname}_{t}_{c}",
                                      tag="bridge", bufs=bridge_bufs)
                if t % 2 == 0:
                    nc.scalar.activation(tb[:], pss[c], AF.Identity)
                else:
                    nc.vector.tensor_copy(tb[:], pss[c])
                srcs.append(tb)
            return srcs

        def layer_dr(sig_pairs, gin_pairs, theta_cb, gout_pairs, want_r,
                     lname, bridge_pool=None, bridge_t=None,
                     bridge_bufs=None):
            if bridge_t is None:
                bridge_t = BRIDGE_T
            if bridge_bufs is None:
                bridge_bufs = BRIDGE_BUFS
            # DoubleRow fp8 layer: A = sigma @ (prev g), drained via ACT Sign
            r = misc.tile([128, JT], F32, name=f"r_{lname}", tag=f"r_{lname}") \
                if want_r else None
            gp_of = lambda tt, cc: gout_pairs[tt//2][:, tt % 2,
                                                     CHUNK*cc:CHUNK*(cc+1)]
            pend = []
            theta = thp = None
            for t in range(JT):
                pss = [ps_main.tile([128, CHUNK], F32,
                                    name=f"ps_{lname}_{t}_{c}", tag="ps_main")
                       for c in range(nch)]
                for m in range(NP):
                    lhs = sig_pairs[m][:, :, 128*t:128*(t+1)]
                    for c in range(nch):
                        nc.tensor.matmul(pss[c], lhs,
                                         gin_pairs[m][:, :, CHUNK*c:CHUNK*(c+1)],
                                         start=(m == 0), stop=(m == NP - 1),
                                         perf_mode=DR)
                if t < bridge_t:
                    pend.append((t, bridge_tile(t, pss, lname, bridge_pool,
                                                bridge_bufs, bdt=FP16)))
                    continue
                if theta is None:
                    # theta algebra lands in the queues only now, after all
                    # bridge copies, then the deferred drains
                    theta, thp = theta_cb()
                    for (tt, ss) in pend:
                        drains(gp_of, tt, ss, theta, r, "accs", lname,
                               thp=thp)
                    pend = []
                drains(gp_of, t, pss, theta, r, "accs", lname, thp=thp)
            return r

        # ---------------- layer 1: x load (chunked), sums, weight DMAs ------
        pl1_cm = tc.tile_pool(name="pl1", bufs=1)
        pl1 = pl1_cm.__enter__()

        # startup-critical DMA order: sg1 k-tile 0 first so the PE can begin
        # the first j-tile as soon as x k-tile 0 lands; x tiles next (they
        # pace the psum accumulation); bulk sigma tiles after.
        # Layer-1 K-pass kt: 2w -> (s1[w], xh[w]); 2w+1 -> (s1[w], xl[w]);
        # 12 -> (sgm, xm) mixed hi/lo tail tile.  The interleaved hi/lo
        # order keeps the psum accumulation close to the reference's
        # k-major fp32 summation order.
        s1_tiles = []
        xs_loc = misc.tile([128, KT1], F32, name="xs_loc", tag="xs_loc")
        x_tiles = []
        for w in range(KF):
            sgt = p_sA.tile([128, H], FP8, name=f"s1_{w}", tag="s1",
                            bufs=NW1)
            dma_in(sgt[:], sg1_t[w])
            s1_tiles.append(sgt)
            hi = pl1.tile([128, n_loc], FP16, name=f"xt0_{w}", tag="xhl",
                          bufs=KT1)
            dma_in(hi[:], xh_t[w])
            nc.vector.tensor_reduce(xs_loc[:, 2*w:2*w+1], hi[:], axis=AX.X,
                                    op=ALU.add)
            x_tiles.append(hi)
            lo = pl1.tile([128, n_loc], FP16, name=f"xt1_{w}", tag="xhl",
                          bufs=KT1)
            dma_in(lo[:], xl_t[w])
            nc.vector.tensor_reduce(xs_loc[:, 2*w+1:2*w+2], lo[:],
                                    axis=AX.X, op=ALU.add)
            x_tiles.append(lo)
        sgt = p_sA.tile([128, H], FP8, name="s1_m", tag="s1", bufs=NW1)
        dma_in(sgt[:], sgm[:])
        s1_tiles.append(sgt)          # s1_tiles[KF] = mixed weight plane
        xmt = pl1.tile([128, n_loc], FP16, name="xt_m", tag="xhl", bufs=KT1)
        dma_in(xmt[:], xm[:])
        nc.vector.tensor_reduce(xs_loc[:, 2*KF:2*KF+1], xmt[:], axis=AX.X,
                                op=ALU.add)
        x_tiles.append(xmt)

        if DMA_ONLY:
            # land every input, then bail out with a token output write
            for m in range(NP):
                sg = p_sA.tile([128, 2, H], FP8, name=f"s2_{m}", tag="s2",
                               bufs=NP)
                dma_in(sg[:], sg2_t[m])
            for m in range(NP):
                sg = p_sA.tile([128, 2, H], FP8, name=f"s3_{m}", tag="s3",
                               bufs=NP)
                dma_in(sg[:], sg3_t[m])
            for m in range(NP):
                sg = p_s4.tile([128, 2, 16], FP8, name=f"s4_{m}", tag="s4",
                               bufs=NP)
                dma_in(sg[:], sg4_t[m])
            w4sb = p_l4c.tile([C, H], F32, name="w4sb", tag="w4sb")
            dma_in(w4sb[:], w4n[:])
            nc.sync.dma_start(yout[0:10, 0:13], xs_loc[0:10, :])
            pl1_cm.__exit__(None, None, None)
            return

        def w1_of(kt):
            return s1_tiles[kt // 2] if kt < 2*KF else s1_tiles[KF]

        # per-weight-plane x rowsums: the hi and lo k-tiles of plane w
        # share sign weights, so their rowsums sum before the theta matvec
        xsv = misc.tile([128, NW1], F32, name="xsv", tag="xsv")
        xsi = xs_loc[:, 0:2*KF].rearrange("p (a b) -> p a b", b=2)
        nc.vector.tensor_add(xsv[:, 0:KF], xsi[:, :, 0], xsi[:, :, 1])
        nc.vector.tensor_copy(xsv[:, KF:NW1], xs_loc[:, 2*KF:2*KF+1])

        # exact fixed-point digitization: 8 balanced base-16 fp8 digits of
        # xsv * 2^17 (hierarchical RN extraction; every step exact in fp32,
        # only the final fractional drop rounds: <= 2^-18 per value).  The
        # theta matvec psums then accumulate exact small integers, so the
        # threshold has no accumulation-order rounding dice at all.
        xs2 = misc.tile([128, NW1, 8], FP8, name="xs2", tag="xs2")
        rk = misc.tile([128, NW1], F32, name="th1_rk", tag="th1_rk")
        nc.vector.tensor_scalar_mul(rk[:], xsv[:], float(2.0 ** -11))
        for d in range(7, -1, -1):
            tm = misc.tile([128, NW1], F32, name=f"th1_t{d}", tag="th1_tm",
                           bufs=2)
            nc.vector.tensor_scalar(tm[:], rk[:], MAGIC, None, ALU.add)
            dg = misc.tile([128, NW1], F32, name=f"th1_d{d}", tag="th1_dg",
                           bufs=2)
            nc.vector.tensor_scalar(dg[:], tm[:], MAGIC, None, ALU.subtract)
            nc.vector.tensor_copy(xs2[:, :, d], dg[:])
            if d > 0:
                rem = misc.tile([128, NW1], F32, name=f"th1_r{d}",
                                tag="th1_rm", bufs=2)
                nc.vector.scalar_tensor_tensor(rem[:], dg[:], -1.0, rk[:],
                                               ALU.mult, ALU.add)
                rk = misc.tile([128, NW1], F32, name=f"th1_k{d}",
                               tag="th1_rk2", bufs=2)
                nc.vector.tensor_scalar_mul(rk[:], rem[:], 16.0)

        def zero_theta(name):
            z = misc.tile([128, JT], F32, name=name, tag=name)
            nc.vector.tensor_scalar_mul(z[:, 0:KT1], xs_loc[:], 0.0)
            nc.vector.tensor_scalar_mul(z[:, KT1:JT], xs_loc[:, 0:JT-KT1], 0.0)
            return z

        # local transposed matvec for theta1 over the 7 weight planes.
        # Emitted from inside the layer-1 main loop (after tile 2's
        # matmuls) so the in-order PE queue does not head-of-line block
        # layer 1's first tiles -- which can pace with the x DMAs --
        # behind a matvec that needs ALL x tiles landed.
        th1_state = {}

        def emit_phi1():
            phi1 = phi_tile("phi1")
            for cj in range(NCHNK):
                ps = ps_small.tile([8, CHUNK], F32, name=f"mvT_th1_{cj}",
                                   tag="ps_main", bufs=8)
                for w in range(NW1):
                    nc.tensor.matmul(ps[:], xs2[:, w, :],
                                     s1_tiles[w][:, CHUNK*cj:CHUNK*(cj+1)],
                                     start=(w == 0), stop=(w == NW1 - 1))
                nc.vector.tensor_copy(phi1[0:8, CHUNK*cj:CHUNK*(cj+1)],
                                      ps[:])
            if not NO_THETA:
                th1_state["D1"] = allreduce_rows(phi1[0:8, :], 8, "phi1")

        if NO_THETA:
            def th1_cb():
                z1 = zero_theta("zth1")
                return theta_scales(z1[:], "th1")
        else:
            def th1_cb():
                D1 = th1_state["D1"]
                # theta = sum_d D1[..d] * 16^d * 2^-17 / B, smallest digit
                # first so the rounding stays at ~2 ulp of the result
                acc = misc.tile([128, JT], F32, name="th1_a0", tag="th1_acc",
                                bufs=2)
                nc.vector.tensor_scalar_mul(acc[:], D1[:, :, 0],
                                            float(2.0 ** -17) * inv_B)
                for d in range(1, 8):
                    nxt = misc.tile([128, JT], F32, name=f"th1_a{d}",
                                    tag="th1_acc", bufs=2)
                    nc.vector.scalar_tensor_tensor(
                        nxt[:], D1[:, :, d], float(2.0 ** (4*d - 17)) * inv_B,
                        acc[:], ALU.mult, ALU.add)
                    acc = nxt
                thn = misc.tile([128, JT], F32, name="th1_n", tag="th1_n")
                nc.vector.tensor_scalar_mul(thn[:], acc[:], -1.0)
                return thn, acc

        # sigma2 pair tiles: direct DMA of host-signed fp8 planes
        s2_pairs = []
        for m in range(NP):
            sg = p_sA.tile([128, 2, H], FP8, name=f"s2_{m}", tag="s2", bufs=NP)
            dma_in(sg[:], sg2_t[m])
            s2_pairs.append(sg)

        # layer-4 statics (tiny): land them during layer 1 so the DMA queue
        # is empty around every AllReduce
        s4_pairs = []
        for m in range(NP):
            sg = p_s4.tile([128, 2, 16], FP8, name=f"s4_{m}", tag="s4", bufs=NP)
            dma_in(sg[:], sg4_t[m])
            s4_pairs.append(sg)
        w4sb = p_l4c.tile([C, H], F32, name="w4sb", tag="w4sb")
        dma_in(w4sb[:], w4n[:])
        g4sb = misc.tile([C, 1], F32, name="g4sb", tag="g4sb")
        dma_in(g4sb[:], g4v[:])
        b4sb = misc.tile([C, 1], F32, name="b4sb", tag="b4sb")
        dma_in(b4sb[:], b4v[:])

        s4raw = misc.tile([C, 1], F32, name="s4raw", tag="s4raw")
        nc.vector.tensor_reduce(s4raw[:], w4sb[:], axis=AX.X, op=ALU.add,
                                apply_absolute_value=True)
        s4 = misc.tile([C, 1], F32, name="s4", tag="s4sc")
        nc.vector.tensor_scalar_mul(s4[:], s4raw[:], inv_H)     # mean|w4|
        ns4sq = misc.tile([C, 1], F32, name="ns4sq", tag="ns4sq")
        nc.vector.tensor_scalar(ns4sq[:], s4[:], s4[:], -1.0,
                                ALU.mult, ALU.mult)              # -s4^2

        # ---------------- layer 1 main (13-pass 2-term fp16 x fp8 sign) -----
        g1_pairs = alloc_g_pairs(p_gA, "gA", "l1")
        r1 = misc.tile([128, JT], F32, name="r_l1", tag="r_l1")
        gp1 = lambda tt, cc: g1_pairs[tt//2][:, tt % 2,
                                             CHUNK*cc:CHUNK*(cc+1)]
        pend1 = []
        theta1 = thp1 = None
        for t in range(JT):
            pss = [ps_main.tile([128, CHUNK], F32, name=f"ps_l1_{t}_{c}",
                                tag="ps_main") for c in range(nch)]
            for kt in range(KT1):
                lhs = w1_of(kt)[:, 128*t:128*(t+1)]
                for c in range(nch):
                    nc.tensor.matmul(pss[c], lhs,
                                     x_tiles[kt][:, CHUNK*c:CHUNK*(c+1)],
                                     start=(kt == 0), stop=(kt == KT1 - 1))
            if t == 1:
                emit_phi1()
            if t < BRIDGE_T:
                pend1.append((t, bridge_tile(t, pss, "l1", pl1,
                                             BRIDGE_BUFS)))
            else:
                if theta1 is None:
                    theta1, thp1 = th1_cb()
                    for (tt, ss) in pend1:
                        drains(gp1, tt, ss, theta1, r1, "accs", "l1",
                               thp=thp1)
                    pend1 = []
                drains(gp1, t, pss, theta1, r1, "accs", "l1", thp=thp1)
            if t == 3:
                # sigma3 pair tiles: DMA mid-layer-1 (queue is idle then;
                # landing them early keeps the phi AllReduce DMAs and the
                # ACT drain stream unblocked at the layer boundaries)
                s3_pairs = []
                for m in range(NP):
                    sg = p_sA.tile([128, 2, H], FP8, name=f"s3_{m}",
                                   tag="s3", bufs=NP)
                    dma_in(sg[:], sg3_t[m])
                    s3_pairs.append(sg)

        for _e in range(N_EXTRA_L1):
            for t in range(JT):
                pss = [ps_main.tile([128, CHUNK], F32,
                                    name=f"px1_{_e}_{t}_{c}", tag="ps_main")
                       for c in range(nch)]
                for kt in range(KT1):
                    lhs = w1_of(kt)[:, 128*t:128*(t+1)]
                    for c in range(nch):
                        nc.tensor.matmul(pss[c], lhs,
                                         x_tiles[kt][:, CHUNK*c:CHUNK*(c+1)],
                                         start=(kt == 0),
                                         stop=(kt == KT1 - 1))

        pl1_cm.__exit__(None, None, None)

        p_gB = es.enter_context(tc.tile_pool(name="p_gB", bufs=8))
        brB_cm = tc.tile_pool(name="brB", bufs=1)
        brB = brB_cm.__enter__()

        if NO_THETA:
            th2_cb = lambda: theta_scales(zero_theta("zth2")[:], "th2")
        else:
            rd1 = digitize_r(r1, "rd1")
            phi2 = matvec_T_dr(rd1, s2_pairs, "th2")
            D2 = allreduce_rows(phi2[0:3, :], 3, "phi2")
            th2_cb = lambda: theta_scales(combine_digits(D2, "th2")[:],
                                          "th2")

        g2_pairs = alloc_g_pairs(p_gB, "gB", "l2")
        r2 = layer_dr(s2_pairs, g1_pairs, th2_cb, g2_pairs, True, "l2",
                      bridge_pool=brB, bridge_t=5, bridge_bufs=20)

        if NO_THETA:
            th3_cb = lambda: theta_scales(zero_theta("zth3")[:], "th3")
        else:
            rd2 = digitize_r(r2, "rd2")
            phi3 = matvec_T_dr(rd2, s3_pairs, "th3")
            D3 = allreduce_rows(phi3[0:3, :], 3, "phi3")
            th3_cb = lambda: theta_scales(combine_digits(D3, "th3")[:],
                                          "th3")

        g3_pairs = alloc_g_pairs(p_gA, "gA", "l3")
        th3_memo = []

        def th3_once():
            if not th3_memo:
                th3_memo.append(th3_cb())
            return th3_memo[0]

        layer_dr(s3_pairs, g2_pairs, th3_once, g3_pairs, False, "l3",
                 bridge_pool=brB, bridge_t=5, bridge_bufs=20)

        if N_EXTRA:
            # dummy layers recycle dead pools: s2 (dead after L2) and
            # gB/g2 (dead once dummy0 replaces g3 as the consumer)
            assert N_EXTRA <= 2
            gin = g3_pairs
            for e in range(N_EXTRA):
                if MM_ONLY_EXTRA:
                    for t in range(JT):
                        pss = [ps_main.tile([128, CHUNK], F32,
                                            name=f"px_{e}_{t}_{c}",
                                            tag="ps_main")
                               for c in range(nch)]
                        for m in range(NP):
                            lhs = s3_pairs[m][:, :, 128*t:128*(t+1)]
                            for c in range(nch):
                                nc.tensor.matmul(
                                    pss[c], lhs,
                                    gin[m][:, :, CHUNK*c:CHUNK*(c+1)],
                                    start=(m == 0), stop=(m == NP - 1),
                                    perf_mode=DR)
                    continue
                pool, tag = (p_sA, "s2") if e == 0 else (p_gB, "gB")
                gx = [pool.tile([128, 2, n_loc], FP8, name=f"g_x{e}_{m}",
                                tag=tag, bufs=NP) for m in range(NP)]
                layer_dr(s3_pairs, gin, th3_once, gx, False, f"lx{e}")
                gin = gx

        brB_cm.__exit__(None, None, None)
        p_l4 = es.enter_context(tc.tile_pool(name="p_l4", bufs=1))

        # ---------------- layer 4 + BatchNorm ----------------
        # k4 = sign(W4) @ h3 (exact small integers); BN statistics of
        # y4 = s4*k4 are folded into per-channel affine coefficients so the
        # only full-width op after the AllReduce is one tensor_scalar.
        k4 = p_l4.tile([C, n_loc], F32, name="k4", tag="k4")
        ksum = misc.tile([C, nch], F32, name="ksum", tag="ksum")
        ksq = misc.tile([C, nch], F32, name="ksq", tag="ksq")
        for c in range(nch):
            ps = ps_small.tile([16, CHUNK], F32, name=f"ps_l4_{c}", tag="ps_main", bufs=8)
            for m in range(NP):
                nc.tensor.matmul(ps[:], s4_pairs[m][:],
                                 g3_pairs[m][:, :, CHUNK*c:CHUNK*(c+1)],
                                 start=(m == 0), stop=(m == NP - 1),
                                 perf_mode=DR)
            nc.vector.tensor_scalar(k4[:, CHUNK*c:CHUNK*(c+1)], ps[0:C, :],
                                    1.0, None, ALU.mult, ALU.add,
                                    accum_out=ksum[:, c:c+1])
            sqsc = p_l4.tile([C, CHUNK], F32, name=f"sq_{c}", tag="sqsc",
                             bufs=2)
            nc.scalar.activation(sqsc[:], ps[0:C, :], AF.Square,
                                 accum_out=ksq[:, c:c+1])

        # pre-scale the stats by 1/B before the AllReduce: the reduced
        # result is then directly (mu_k, E[k^2])
        p4 = misc.tile([C, 2], F32, name="p4", tag="p4")
        nc.vector.tensor_reduce(p4[:, 0:1], ksum[:], axis=AX.X, op=ALU.add)
        nc.vector.tensor_reduce(p4[:, 1:2], ksq[:], axis=AX.X, op=ALU.add)
        p4s = misc.tile([C, 2], F32, name="p4s", tag="p4s")
        nc.vector.tensor_scalar_mul(p4s[:], p4[:], inv_B)

        G4 = allreduce(p4s[:], [C, 2], "p4")
        # -var = mu^2 - E[k^2];  veps = (-var)*(-s4^2) + eps
        nvar = misc.tile([C, 1], F32, name="nvar", tag="nvar")
        nc.vector.tensor_scalar(nvar[:], G4[:, 0:1], G4[:, 0:1], G4[:, 1:2],
                                ALU.mult, ALU.subtract)
        veps = misc.tile([C, 1], F32, name="veps", tag="veps")
        nc.vector.tensor_scalar(veps[:], nvar[:], ns4sq[:], 1e-5,
                                ALU.mult, ALU.add)
        sd = misc.tile([C, 1], F32, name="sd", tag="sd")
        nc.scalar.activation(sd[:], veps[:], AF.Sqrt)
        inv_sd = misc.tile([C, 1], F32, name="inv_sd", tag="inv_sd")
        nc.vector.reciprocal(inv_sd[:], sd[:])
        # A = g*s4/sd(y);  B0 = b - mu_k*A
        A4 = misc.tile([C, 1], F32, name="A4", tag="A4")
        nc.vector.tensor_scalar(A4[:], inv_sd[:], g4sb[:], s4[:],
                                ALU.mult, ALU.mult)
        muA = misc.tile([C, 1], F32, name="muA", tag="muA")
        nc.vector.tensor_mul(muA[:], G4[:, 0:1], A4[:])
        B4 = misc.tile([C, 1], F32, name="B4", tag="B4")
        nc.vector.tensor_sub(B4[:], b4sb[:], muA[:])

        # final affine: 4 chunks alternating DVE/ACT, DMA out per chunk so
        # the output transfer overlaps the remaining affine compute
        yo = p_l4.tile([C, n_loc], F32, name="yo", tag="yo")
        qn = n_loc // 4
        for q in range(4):
            sl = slice(q * qn, (q + 1) * qn)
            if q % 2 == 0:
                nc.vector.tensor_scalar(yo[:, sl], k4[:, sl], A4[:], B4[:],
                                        ALU.mult, ALU.add)
            else:
                nc.scalar.activation(yo[:, sl], k4[:, sl], AF.Identity,
                                     bias=B4[:], scale=A4[:])
            nc.sync.dma_start(yout[:, sl], yo[:, sl])


# --------------------------------------------------------------------------
def prep_inputs(x, w1, w2, w3, w4, g4, b4, n_loc):
    n_cores = N_CORES
    B = x.shape[0]
    assert B == n_loc * n_cores

    f16 = np.float16

    def signs(w):
        return np.where(np.asarray(w) >= 0, np.float32(1.0),
                        np.float32(-1.0))

    # sg1: [768, H], row k = sign(w1[:, k]); sgm: [128, H] mixed tail plane
    # (rows 0-15 = sign rows 768-783, rows 16-31 the same, rest zero)
    s1 = signs(w1).T.astype(NP8)          # [784, 2048]
    sg1 = np.ascontiguousarray(s1[:KF*128])
    sgm = np.zeros((128, H), dtype=NP8)
    sgm[0:D_IN - KF*128] = s1[KF*128:]
    sgm[16:16 + D_IN - KF*128] = s1[KF*128:]

    def paired(w, width):
        # [H, width]: rows (m, k, p) -> sign(w[j, 128*(2m+p)+k]), j < width.
        # Odd input-tile planes (p=1) are scaled x2: those tiles' activations
        # are stored as (h+1)/2 in {0,1} by the DVE drain path.
        s = signs(w)                       # [out, H]
        out = np.zeros((NP, 128, 2, width), dtype=NP8)
        for m in range(NP):
            for p in (0, 1):
                blk = s[:, 128*(2*m+p):128*(2*m+p+1)] * (1.0 + p)
                out[m, :, p, :blk.shape[0]] = blk.T.astype(NP8)
        return out.reshape(H, width)

    sg2 = paired(w2, H)
    sg3 = paired(w3, H)
    sg4 = paired(w4, 16)
    w4n = np.ascontiguousarray(np.asarray(w4).astype(np.float32))
    g4v = np.ascontiguousarray(np.asarray(g4).reshape(C, 1).astype(np.float32))
    b4v = np.ascontiguousarray(np.asarray(b4).reshape(C, 1).astype(np.float32))

    in_maps = []
    ntail = D_IN - KF*128                  # 16 tail rows
    for cidx in range(n_cores):
        xs = x[n_loc*cidx:n_loc*(cidx+1)]
        xT = np.ascontiguousarray(xs.T.astype(np.float32))   # [784, n_loc]
        xhp = xT.astype(f16)
        xlp = (xT - xhp.astype(np.float32)).astype(f16)
        xmv = np.zeros((128, n_loc), dtype=f16)
        xmv[0:ntail] = xhp[KF*128:]
        xmv[16:16 + ntail] = xlp[KF*128:]
        m = {
            "xh": np.ascontiguousarray(xhp[:KF*128]),
            "xl": np.ascontiguousarray(xlp[:KF*128]),
            "xm": xmv,
            "sg1": sg1, "sgm": sgm, "sg2": sg2, "sg3": sg3,
            "sg4": sg4, "w4n": w4n, "g4v": g4v, "b4v": b4v,
        }
        in_maps.append(m)
    return in_maps


_NC_CACHE = {}


def kernel(x, w1, w2, w3, w4, g1, b1, g2, b2, g3, b3, g4, b4):
    x = np.asarray(x); w1 = np.asarray(w1); w2 = np.asarray(w2)
    w3 = np.asarray(w3); w4 = np.asarray(w4)
    g4 = np.asarray(g4); b4 = np.asarray(b4)
    # layers 1-3 BN params: scales cancel inside sign() only when gamma>0, beta=0
    for g in (g1, g2, g3):
        assert np.all(np.asarray(g) > 0), "kernel assumes gamma > 0 for hidden BNs"
    for b in (b1, b2, b3):
        assert np.all(np.asarray(b) == 0), "kernel assumes beta == 0 for hidden BNs"
    for w in (w1, w2, w3, w4):
        assert not np.any(w == 0.0), "exact-zero weight would break Sign()"

    n_loc = x.shape[0] // N_CORES
    if n_loc not in _NC_CACHE:
        _NC_CACHE[n_loc] = build(n_loc)
    nc = _NC_CACHE[n_loc]

    in_maps = prep_inputs(x, w1, w2, w3, w4, g4, b4, n_loc)
    res = bass_utils.run_bass_kernel_spmd(nc, in_maps,
                                          core_ids=list(range(N_CORES)))
    out = np.concatenate([res.results[c]["yout"].T for c in range(N_CORES)],
                         axis=0)
    return out.astype(np.float32)
